# revision 4
# baseline (speedup 1.0000x reference)
"""AttnDecoderRNN step on 8 Trainium2 NeuronCores (Bass/Tile, tensor-parallel).

Strategy (all large operands column-sharded over the 8 cores; vectors stitched
back with AllGather):
  - emb sharded over H columns: each core indirect-gathers its 128-wide slice
    of the token row; AG0 rebuilds the full embedding row.
  - attn scores: attn_W sharded over S columns (512/core); AG1 -> softmax
    computed (partition-major) on every core.
  - attn_applied: encoder_outputs sharded over H columns; AG2.
  - comb: comb_W sharded over H columns; AG3.
  - GRU (x2, shared weights): gate outputs sharded over H (128/core, all 3
    gates); h re-gathered with AG4/AG5 for the next contraction.
  - logits: out_W sharded over vocab columns (6400/core after padding to
    51200); log_softmax denominator via AG of per-core exp-sums (AG6); each
    core writes its own log-prob shard, host concatenates.

All compute is f32 except the out_W projection which is stored/streamed in
bf16 (halves the dominant HBM traffic; logit abs error ~4e-3 on values ~-10.8).
"""

import numpy as np

import concourse.bacc as bacc
import concourse.bass as bass
import concourse.tile as tile
from concourse import mybir
from concourse.bass import IndirectOffsetOnAxis
from concourse.bass_utils import run_bass_kernel_spmd
from concourse.masks import make_identity

H = 1024
S = 4096
V = 50257
L = 2
NC = 8
VP = 51200          # padded vocab
VS = VP // NC       # 6400 vocab cols per core
MCH = VS // 128     # 50 m-chunks per core
SS = S // NC        # 512 attn cols per core
KH = H // 128       # 8 k-chunks for H contraction
K2H = 2 * H // 128  # 16 k-chunks for 2H contraction
KS = S // 128       # 32 k-chunks for S contraction

F32 = mybir.dt.float32
BF16 = mybir.dt.bfloat16
OW_BF16 = True      # stream out_W in bf16

_CACHE = {}


def _gru_layer(nc, tc, pools, x_src_dram, h_sb, h_col, wih_sb, whh_sb,
               bi_sb, bh_sb, bsum_sb, ident):
    """One GRU cell step; returns (h_new_col [128,1] sbuf AP)."""
    sb, ps = pools
    gi_ps = ps.tile([128, 3], F32, name="gi_ps", tag="scr")
    gh_ps = ps.tile([128, 3], F32, name="gh_ps", tag="scr")
    for g in range(3):
        for k in range(KH):
            nc.tensor.matmul(gi_ps[:, g:g + 1], lhsT=wih_sb[:, k, g, :],
                             rhs=x_src_dram[:, k:k + 1],
                             start=(k == 0), stop=(k == KH - 1))
    for g in range(3):
        for k in range(KH):
            nc.tensor.matmul(gh_ps[:, g:g + 1], lhsT=whh_sb[:, k, g, :],
                             rhs=h_sb[:, k:k + 1],
                             start=(k == 0), stop=(k == KH - 1))
    rz_sb = sb.tile([128, 2], F32, name="rz_sb")
    nc.vector.tensor_add(rz_sb[:], gi_ps[:, 0:2], gh_ps[:, 0:2])
    r = sb.tile([128, 1], F32, name="gru_r")
    z = sb.tile([128, 1], F32, name="gru_z")
    nc.scalar.activation(r[:], rz_sb[:, 0:1],
                         mybir.ActivationFunctionType.Sigmoid, bias=bsum_sb[:, 0:1])
    nc.scalar.activation(z[:], rz_sb[:, 1:2],
                         mybir.ActivationFunctionType.Sigmoid, bias=bsum_sb[:, 1:2])
    hnb = sb.tile([128, 1], F32, name="gru_hnb")
    nc.scalar.activation(hnb[:], gh_ps[:, 2:3],
                         mybir.ActivationFunctionType.Identity, bias=bh_sb[:, 2:3])
    rn = sb.tile([128, 1], F32, name="gru_rn")
    nc.vector.tensor_mul(rn[:], r[:], hnb[:])
    tin = sb.tile([128, 1], F32, name="gru_tin")
    nc.vector.tensor_add(tin[:], gi_ps[:, 2:3], rn[:])
    n = sb.tile([128, 1], F32, name="gru_n")
    nc.scalar.activation(n[:], tin[:],
                         mybir.ActivationFunctionType.Tanh, bias=bi_sb[:, 2:3])
    d = sb.tile([128, 1], F32, name="gru_d")
    nc.vector.tensor_sub(d[:], h_col[:], n[:])
    zd = sb.tile([128, 1], F32, name="gru_zd")
    nc.vector.tensor_mul(zd[:], z[:], d[:])
    h_new = sb.tile([128, 1], F32, name="gru_hnew")
    nc.vector.tensor_add(h_new[:], n[:], zd[:])
    return h_new


def build_program():
    ow_dt = BF16 if OW_BF16 else F32
    nc = bacc.Bacc("TRN2", target_bir_lowering=False, debug=False, num_devices=NC)
    AF = mybir.ActivationFunctionType

    # ---- kernel I/O (per core) ----
    idx2 = nc.dram_tensor("idx2", [2, 1], mybir.dt.int32, kind="ExternalInput")
    emb_cols = nc.dram_tensor("emb_cols", [V, 128], F32, kind="ExternalInput")
    hidden8 = nc.dram_tensor("hidden8", [KH, 128], F32, kind="ExternalInput")
    h0col = nc.dram_tensor("h0col", [128, 1], F32, kind="ExternalInput")
    attn_w = nc.dram_tensor("attn_w", [2 * H, SS], F32, kind="ExternalInput")
    attn_b = nc.dram_tensor("attn_b", [1, SS], F32, kind="ExternalInput")
    enc = nc.dram_tensor("enc", [S, 128], F32, kind="ExternalInput")
    comb_w = nc.dram_tensor("comb_w", [2 * H, 128], F32, kind="ExternalInput")
    comb_b = nc.dram_tensor("comb_b", [1, 128], F32, kind="ExternalInput")
    wih = nc.dram_tensor("wih", [H, 3, 128], F32, kind="ExternalInput")
    whh = nc.dram_tensor("whh", [H, 3, 128], F32, kind="ExternalInput")
    bi = nc.dram_tensor("bi", [128, 3], F32, kind="ExternalInput")
    bh = nc.dram_tensor("bh", [128, 3], F32, kind="ExternalInput")
    ow = nc.dram_tensor("ow", [H, VS], ow_dt, kind="ExternalInput")
    ob = nc.dram_tensor("ob", [128, MCH], F32, kind="ExternalInput")

    attn_out = nc.dram_tensor("attn_out", [128, KS], F32, kind="ExternalOutput")
    h_out = nc.dram_tensor("h_out", [128, 1], F32, kind="ExternalOutput")
    logp_out = nc.dram_tensor("logp_out", [128, MCH], F32, kind="ExternalOutput")

    rg = [list(range(NC))]

    with tile.TileContext(nc) as tc:
        with (
            tc.tile_pool(name="w", bufs=1) as wp,
            tc.tile_pool(name="sb", bufs=2) as sb,
            tc.tile_pool(name="owp", bufs=3) as owp,
            tc.tile_pool(name="ps", bufs=4, space="PSUM") as ps,
            tc.tile_pool(name="lgps", bufs=2, space="PSUM") as lgps,
            tc.tile_pool(name="dram", bufs=1, space="DRAM") as dram,
        ):
            # ---- persistent weights / constants ----
            ident = wp.tile([128, 128], F32)
            make_identity(nc, ident[:])
            ones_p = wp.tile([128, 1], F32)
            nc.vector.memset(ones_p[:], 1.0)
            ones_1 = wp.tile([1, 128], F32)
            nc.vector.memset(ones_1[:], 1.0)

            attn_sb = wp.tile([128, K2H, SS], F32)
            nc.sync.dma_start(out=attn_sb[:],
                              in_=attn_w[:, :].rearrange("(k p) n -> p k n", p=128))
            enc_sb = wp.tile([128, KS, 128], F32)
            nc.sync.dma_start(out=enc_sb[:],
                              in_=enc[:, :].rearrange("(k p) n -> p k n", p=128))
            comb_sb = wp.tile([128, K2H, 128], F32)
            nc.sync.dma_start(out=comb_sb[:],
                              in_=comb_w[:, :].rearrange("(k p) n -> p k n", p=128))
            wih_sb = wp.tile([128, KH, 3, 128], F32)
            nc.sync.dma_start(out=wih_sb[:],
                              in_=wih[:, :, :].rearrange("(k p) g n -> p k g n", p=128))
            whh_sb = wp.tile([128, KH, 3, 128], F32)
            nc.sync.dma_start(out=whh_sb[:],
                              in_=whh[:, :, :].rearrange("(k p) g n -> p k g n", p=128))
            ab_sb = wp.tile([1, SS], F32)
            nc.sync.dma_start(out=ab_sb[:], in_=attn_b[:, :])
            cbias_sb = wp.tile([1, 128], F32)
            nc.sync.dma_start(out=cbias_sb[:], in_=comb_b[:, :])
            bi_sb = wp.tile([128, 3], F32)
            nc.sync.dma_start(out=bi_sb[:], in_=bi[:, :])
            bh_sb = wp.tile([128, 3], F32)
            nc.sync.dma_start(out=bh_sb[:], in_=bh[:, :])
            ob_sb = wp.tile([128, MCH], F32)
            nc.sync.dma_start(out=ob_sb[:], in_=ob[:, :])
            h0_sb = wp.tile([128, 1], F32)
            nc.sync.dma_start(out=h0_sb[:], in_=h0col[:, :])
            bsum_sb = wp.tile([128, 2], F32)
            nc.vector.tensor_add(bsum_sb[:], bi_sb[:, 0:2], bh_sb[:, 0:2])

            # ---- step 0: embedding gather + AG0 ----
            idx_sb = sb.tile([2, 1], mybir.dt.int32, name="idx_sb")
            nc.sync.dma_start(out=idx_sb[:], in_=idx2[:, :])
            er2 = sb.tile([2, 128], F32, name="er2")
            nc.gpsimd.indirect_dma_start(
                out=er2[:], out_offset=None,
                in_=emb_cols[:, :],
                in_offset=IndirectOffsetOnAxis(ap=idx_sb[:, :1], axis=0))
            ag0_in = dram.tile([1, 128], F32)
            ag0_out = dram.tile([NC, 128], F32)
            nc.sync.dma_start(out=ag0_in[:], in_=er2[0:1, :])
            nc.gpsimd.collective_compute(
                "AllGather", mybir.AluOpType.bypass, replica_groups=rg,
                ins=[ag0_in.opt()], outs=[ag0_out.opt()])

            # ---- step 1: x2h = [emb ; h] in partition-major chunks ----
            xh16 = sb.tile([K2H, 128], F32, name="xh16")
            nc.sync.dma_start(out=xh16[0:KH, :], in_=ag0_out[:])
            nc.sync.dma_start(out=xh16[KH:K2H, :], in_=hidden8[:, :])
            tp_xh = ps.tile([128, K2H], F32, name="tp_xh", tag="scr")
            nc.tensor.transpose(tp_xh[:], xh16[:], ident[:K2H, :K2H])
            x2h_sb = sb.tile([128, K2H], F32, name="x2h_sb")
            nc.vector.tensor_copy(x2h_sb[:], tp_xh[:])

            # ---- step 2: attn scores (this core's 512 cols of S) + AG1 ----
            t1_ps = ps.tile([1, SS], F32, name="t1_ps", tag="scr")
            for k in range(K2H):
                nc.tensor.matmul(t1_ps[:], lhsT=x2h_sb[:, k:k + 1],
                                 rhs=attn_sb[:, k, :],
                                 start=(k == 0), stop=(k == K2H - 1))
            t1_sb = sb.tile([1, SS], F32, name="t1_sb")
            nc.vector.tensor_add(t1_sb[:], t1_ps[:], ab_sb[:])
            ag1_in = dram.tile([1, SS], F32)
            ag1_out = dram.tile([NC, SS], F32)
            nc.sync.dma_start(out=ag1_in[:], in_=t1_sb[:])
            nc.gpsimd.collective_compute(
                "AllGather", mybir.AluOpType.bypass, replica_groups=rg,
                ins=[ag1_in.opt()], outs=[ag1_out.opt()])

            # ---- step 3: softmax over full S (partition-major) ----
            t1_32 = sb.tile([KS, 128], F32, name="t1_32")
            nc.sync.dma_start(out=t1_32[:],
                              in_=ag1_out[:].rearrange("a b -> (a b)")
                              .rearrange("(k p) -> k p", p=128))
            tp_t1 = ps.tile([128, KS], F32, name="tp_t1", tag="scr")
            nc.tensor.transpose(tp_t1[:], t1_32[:], ident[:KS, :KS])
            u_sb = sb.tile([128, KS], F32, name="u_sb")
            srow = sb.tile([128, 1], F32, name="srow")
            nc.scalar.activation(u_sb[:], tp_t1[:], AF.Exp, accum_out=srow[:])
            s1_ps = ps.tile([1, 1], F32, name="s1_ps", tag="scr")
            nc.tensor.matmul(s1_ps[:], lhsT=srow[:], rhs=ones_p[:],
                             start=True, stop=True)
            rs_sb = sb.tile([1, 1], F32, name="rs_sb")
            nc.vector.reciprocal(rs_sb[:], s1_ps[:])
            rb_ps = ps.tile([128, 1], F32, name="rb_ps", tag="scr")
            nc.tensor.matmul(rb_ps[:], lhsT=ones_1[:], rhs=rs_sb[:],
                             start=True, stop=True)
            rs_bc = sb.tile([128, 1], F32, name="rs_bc")
            nc.vector.tensor_copy(rs_bc[:], rb_ps[:])
            aw_sb = sb.tile([128, KS], F32, name="aw_sb")
            nc.vector.tensor_scalar_mul(aw_sb[:], u_sb[:], rs_bc[:])
            nc.sync.dma_start(out=attn_out[:, :], in_=aw_sb[:])

            # ---- step 4: attn_applied (this core's 128 cols of H) + AG2 ----
            aa_ps = ps.tile([1, 128], F32, name="aa_ps", tag="scr")
            for k in range(KS):
                nc.tensor.matmul(aa_ps[:], lhsT=aw_sb[:, k:k + 1],
                                 rhs=enc_sb[:, k, :],
                                 start=(k == 0), stop=(k == KS - 1))
            aa_sb = sb.tile([1, 128], F32, name="aa_sb")
            nc.vector.tensor_copy(aa_sb[:], aa_ps[:])
            ag2_in = dram.tile([1, 128], F32)
            ag2_out = dram.tile([NC, 128], F32)
            nc.sync.dma_start(out=ag2_in[:], in_=aa_sb[:])
            nc.gpsimd.collective_compute(
                "AllGather", mybir.AluOpType.bypass, replica_groups=rg,
                ins=[ag2_in.opt()], outs=[ag2_out.opt()])

            # ---- step 5: comb + AG3 ----
            ya16 = sb.tile([K2H, 128], F32, name="ya16")
            nc.sync.dma_start(out=ya16[0:KH, :], in_=ag0_out[:])
            nc.sync.dma_start(out=ya16[KH:K2H, :], in_=ag2_out[:])
            tp_ya = ps.tile([128, K2H], F32, name="tp_ya", tag="scr")
            nc.tensor.transpose(tp_ya[:], ya16[:], ident[:K2H, :K2H])
            y2h_sb = sb.tile([128, K2H], F32, name="y2h_sb")
            nc.vector.tensor_copy(y2h_sb[:], tp_ya[:])
            cb_ps = ps.tile([1, 128], F32, name="cb_ps", tag="scr")
            for k in range(K2H):
                nc.tensor.matmul(cb_ps[:], lhsT=y2h_sb[:, k:k + 1],
                                 rhs=comb_sb[:, k, :],
                                 start=(k == 0), stop=(k == K2H - 1))
            cbo_sb = sb.tile([1, 128], F32, name="cbo_sb")
            nc.vector.tensor_add(cbo_sb[:], cb_ps[:], cbias_sb[:])
            ag3_in = dram.tile([1, 128], F32)
            ag3_out = dram.tile([NC, 128], F32)
            nc.sync.dma_start(out=ag3_in[:], in_=cbo_sb[:])
            nc.gpsimd.collective_compute(
                "AllGather", mybir.AluOpType.bypass, replica_groups=rg,
                ins=[ag3_in.opt()], outs=[ag3_out.opt()])

            # ---- step 6: GRU layer 1 ----
            hid_sb = sb.tile([128, KH], F32, name="hid_sb")
            tp_h0 = ps.tile([128, KH], F32, name="tp_h0", tag="scr")
            h08 = sb.tile([KH, 128], F32, name="h08")
            nc.sync.dma_start(out=h08[:], in_=hidden8[:, :])
            nc.tensor.transpose(tp_h0[:], h08[:], ident[:KH, :KH])
            nc.vector.tensor_copy(hid_sb[:], tp_h0[:])

            x18 = sb.tile([KH, 128], F32, name="x18")
            nc.sync.dma_start(out=x18[:], in_=ag3_out[:])
            tp_x1 = ps.tile([128, KH], F32, name="tp_x1", tag="scr")
            nc.tensor.transpose(tp_x1[:], x18[:], ident[:KH, :KH])
            x1_sb = sb.tile([128, KH], F32, name="x1_sb")
            nc.scalar.activation(x1_sb[:], tp_x1[:], AF.Relu)

            h1_col = _gru_layer(nc, tc, (sb, ps), x1_sb, hid_sb, h0_sb,
                                wih_sb, whh_sb, bi_sb, bh_sb, bsum_sb, ident)
            ag4_in = dram.tile([128, 1], F32)
            ag4_out = dram.tile([NC * 128, 1], F32)
            nc.sync.dma_start(out=ag4_in[:], in_=h1_col[:])
            nc.gpsimd.collective_compute(
                "AllGather", mybir.AluOpType.bypass, replica_groups=rg,
                ins=[ag4_in.opt()], outs=[ag4_out.opt()])

            # ---- step 7: GRU layer 2 ----
            h18 = sb.tile([KH, 128], F32, name="h18")
            nc.sync.dma_start(out=h18[:],
                              in_=ag4_out[:].rearrange("(a b) o -> a (b o)", b=128))
            tp_h1 = ps.tile([128, KH], F32, name="tp_h1", tag="scr")
            nc.tensor.transpose(tp_h1[:], h18[:], ident[:KH, :KH])
            x2_sb = sb.tile([128, KH], F32, name="x2_sb")
            nc.scalar.activation(x2_sb[:], tp_h1[:], AF.Relu)
            h1f_sb = sb.tile([128, KH], F32, name="h1f_sb")
            nc.vector.tensor_copy(h1f_sb[:], tp_h1[:])

            h2_col = _gru_layer(nc, tc, (sb, ps), x2_sb, h1f_sb, h1_col,
                                wih_sb, whh_sb, bi_sb, bh_sb, bsum_sb, ident)
            nc.sync.dma_start(out=h_out[:, :], in_=h2_col[:])
            ag5_in = dram.tile([128, 1], F32)
            ag5_out = dram.tile([NC * 128, 1], F32)
            nc.sync.dma_start(out=ag5_in[:], in_=h2_col[:])
            nc.gpsimd.collective_compute(
                "AllGather", mybir.AluOpType.bypass, replica_groups=rg,
                ins=[ag5_in.opt()], outs=[ag5_out.opt()])

            # ---- step 8: logits (this core's 6400 padded vocab cols) ----
            h28 = sb.tile([KH, 128], F32, name="h28")
            nc.sync.dma_start(out=h28[:],
                              in_=ag5_out[:].rearrange("(a b) o -> a (b o)", b=128))
            tp_h2 = ps.tile([128, KH], F32, name="tp_h2", tag="scr")
            nc.tensor.transpose(tp_h2[:], h28[:], ident[:KH, :KH])
            h2w_sb = sb.tile([128, KH], ow_dt, name="h2w_sb")
            nc.vector.tensor_copy(h2w_sb[:], tp_h2[:])

            lg_sb = sb.tile([128, MCH], F32, name="lg_sb", bufs=1)
            nc.vector.tensor_copy(lg_sb[:], ob_sb[:])
            for k in range(KH):
                ow_k = owp.tile([128, MCH, 128], ow_dt, name="ow_k")
                nc.sync.dma_start(
                    out=ow_k[:],
                    in_=ow[k * 128:(k + 1) * 128, :].rearrange("p (m n) -> p m n", n=128))
                lg_ps = lgps.tile([128, MCH], F32, name="lg_ps", tag="lg")
                for m in range(MCH):
                    nc.tensor.matmul(lg_ps[:, m:m + 1], lhsT=ow_k[:, m, :],
                                     rhs=h2w_sb[:, k:k + 1],
                                     start=True, stop=True)
                nc.vector.tensor_add(lg_sb[:], lg_sb[:], lg_ps[:])

            # ---- step 9: log-softmax denominator via AG6 ----
            elg = sb.tile([128, MCH], F32, name="elg")
            srl = sb.tile([128, 1], F32, name="srl")
            nc.scalar.activation(elg[:], lg_sb[:], AF.Exp, accum_out=srl[:])
            st_ps = ps.tile([1, 1], F32, name="st_ps", tag="scr")
            nc.tensor.matmul(st_ps[:], lhsT=srl[:], rhs=ones_p[:],
                             start=True, stop=True)
            sc8 = sb.tile([1, 8], F32, name="sc8")
            nc.vector.memset(sc8[:], 0.0)
            nc.vector.tensor_copy(sc8[:, 0:1], st_ps[:])
            ag6_in = dram.tile([1, 8], F32)
            ag6_out = dram.tile([NC, 8], F32)
            nc.sync.dma_start(out=ag6_in[:], in_=sc8[:])
            nc.gpsimd.collective_compute(
                "AllGather", mybir.AluOpType.bypass, replica_groups=rg,
                ins=[ag6_in.opt()], outs=[ag6_out.opt()])
            sg = sb.tile([1, NC, 8], F32, name="sg")
            nc.sync.dma_start(out=sg[:],
                              in_=ag6_out[:].rearrange("a b -> (a b)")[None, :]
                              .rearrange("o (a b) -> o a b", b=8))
            tot = sb.tile([1, 1], F32, name="tot")
            nc.vector.tensor_reduce(tot[:], sg[:, :, 0:1],
                                    axis=mybir.AxisListType.XY, op=mybir.AluOpType.add)
            lse = sb.tile([1, 1], F32, name="lse")
            nc.scalar.activation(lse[:], tot[:], AF.Ln)
            lb_ps = ps.tile([128, 1], F32, name="lb_ps", tag="scr")
            nc.tensor.matmul(lb_ps[:], lhsT=ones_1[:], rhs=lse[:],
                             start=True, stop=True)
            lse_bc = sb.tile([128, 1], F32, name="lse_bc")
            nc.vector.tensor_copy(lse_bc[:], lb_ps[:])
            lp_sb = sb.tile([128, MCH], F32, name="lp_sb")
            nc.vector.tensor_scalar(lp_sb[:], lg_sb[:], lse_bc[:], None,
                                    op0=mybir.AluOpType.subtract)
            nc.sync.dma_start(out=logp_out[:, :], in_=lp_sb[:])

    nc.compile()
    return nc


def shard_inputs(inputs):
    """FULL numpy inputs -> list of 8 per-core input maps."""
    f = lambda k: np.ascontiguousarray(np.asarray(inputs[k], np.float32))
    idx = int(np.asarray(inputs["input"]).ravel()[0])
    emb = f("emb")
    hidden = f("hidden").reshape(H)
    attn_W, attn_b = f("attn_W"), f("attn_b")
    enc = f("encoder_outputs")
    comb_W, comb_b = f("comb_W"), f("comb_b")
    Wih, Whh = f("gru_Wih"), f("gru_Whh")
    bih, bhh = f("gru_bih"), f("gru_bhh")
    out_W, out_b = f("out_W"), f("out_b")

    owp = np.zeros((H, VP), np.float32)
    owp[:, :V] = out_W
    obp = np.full((VP,), -1e30, np.float32)
    obp[:V] = out_b
    ow_dt = np.dtype("bfloat16") if OW_BF16 else np.float32
    try:
        owp = owp.astype(ow_dt)
    except TypeError:
        import ml_dtypes
        owp = owp.astype(ml_dtypes.bfloat16)

    A_ih = np.ascontiguousarray(Wih.T.reshape(H, 3, H))
    A_hh = np.ascontiguousarray(Whh.T.reshape(H, 3, H))
    bi3 = bih.reshape(3, H)
    bh3 = bhh.reshape(3, H)

    idx2 = np.array([[idx], [idx]], np.int32)
    hidden8 = np.ascontiguousarray(hidden.reshape(KH, 128))

    in_maps = []
    for c in range(NC):
        sl = slice(c * 128, (c + 1) * 128)
        ssl = slice(c * SS, (c + 1) * SS)
        vsl = slice(c * VS, (c + 1) * VS)
        in_maps.append({
            "idx2": idx2,
            "emb_cols": np.ascontiguousarray(emb[:, sl]),
            "hidden8": hidden8,
            "h0col": np.ascontiguousarray(hidden[sl, None]),
            "attn_w": np.ascontiguousarray(attn_W[:, ssl]),
            "attn_b": np.ascontiguousarray(attn_b[None, ssl]),
            "enc": np.ascontiguousarray(enc[:, sl]),
            "comb_w": np.ascontiguousarray(comb_W[:, sl]),
            "comb_b": np.ascontiguousarray(comb_b[None, sl]),
            "wih": np.ascontiguousarray(A_ih[:, :, sl]),
            "whh": np.ascontiguousarray(A_hh[:, :, sl]),
            "bi": np.ascontiguousarray(bi3[:, sl].T),
            "bh": np.ascontiguousarray(bh3[:, sl].T),
            "ow": np.ascontiguousarray(owp[:, vsl]),
            "ob": np.ascontiguousarray(obp[vsl].reshape(MCH, 128).T),
        })
    return in_maps


def unshard_outputs(results):
    """list of per-core {name: array} -> (log_probs, hidden, attn_weights)."""
    logp = np.concatenate(
        [np.asarray(r["logp_out"]).T.reshape(-1) for r in results])[:V][None, :]
    h = np.concatenate([np.asarray(r["h_out"]).ravel() for r in results])[None, None, :]
    aw = np.asarray(results[0]["attn_out"]).T.reshape(-1)[None, :]
    return (np.ascontiguousarray(logp, np.float32),
            np.ascontiguousarray(h, np.float32),
            np.ascontiguousarray(aw, np.float32))


def _get_program():
    if "nc" not in _CACHE:
        _CACHE["nc"] = build_program()
    return _CACHE["nc"]


def kernel(**inputs):
    nc = _get_program()
    in_maps = shard_inputs(inputs)
    res = run_bass_kernel_spmd(nc, in_maps, core_ids=list(range(NC)), trace=False)
    return unshard_outputs(res.results)


# revision 7
# speedup vs baseline: 1.5659x; 1.5659x over previous
"""AttnDecoderRNN step on 8 Trainium2 NeuronCores (Bass/Tile, tensor-parallel).

Sharding (8 cores, stitched with AllGather where a full vector is needed):
  - embedding: table replicated (viewed [V*8, 128]); each core indirect-DMAs
    the token row as 8 chunk-rows (4KB HBM read) -- no collective.
  - attn scores: attn_W sharded over S columns (512/core); AG1 -> softmax
    computed partition-major on every core.
  - attn_applied: encoder_outputs sharded over H columns (128/core); AG2.
  - comb: comb_W sharded over H columns; AG3.
  - GRU (x2, shared weights): gate outputs sharded over H (128/core for each
    of the 3 gates); h re-gathered with AG4/AG5 for the next contraction.
  - logits: out_W sharded over vocab columns (6400/core after padding V to
    51200); log_softmax denominator from an AG of per-core exp-sums (AG6);
    each core writes its own log-prob shard, the host concatenates.

Precision: f32 everywhere except out_W / encoder_outputs / comb_W (and the
activation vectors feeding those three matmuls), which run in bf16.
Verified end-to-end error vs the f32 reference: log_probs ~2e-4, hidden and
attn_weights ~2e-5 (max relative).

Queue plan: all weight streaming on the sync (SP) DMA queue in consumption
order; collective bounce-buffer traffic and gather loads on the gpsimd queue
so they interleave with the collectives; compute engines stay DMA-free.
"""

import numpy as np

import concourse.bacc as bacc
import concourse.bass as bass
import concourse.tile as tile
from concourse import mybir
from concourse.bass import IndirectOffsetOnAxis
from concourse.bass_utils import run_bass_kernel_spmd
from concourse.masks import make_identity

H = 1024
S = 4096
V = 50257
L = 2
NC = 8
VP = 51200          # padded vocab
VS = VP // NC       # 6400 vocab cols per core
MCH = VS // 128     # 50 m-chunks per core
SS = S // NC        # 512 attn cols per core
KH = H // 128       # 8 k-chunks for H contraction
K2H = 2 * H // 128  # 16 k-chunks for 2H contraction
KS = S // 128       # 32 k-chunks for S contraction

F32 = mybir.dt.float32
BF16 = mybir.dt.bfloat16

_CACHE = {}


def _gru_layer(nc, sb, ps, x_sb, h_sb, h_col, wih_sb, whh_sb,
               bi_sb, bh_sb, bsum_sb):
    """One GRU cell step; returns h_new [128,1] (this core's slice)."""
    AF = mybir.ActivationFunctionType
    gi_ps = ps.tile([128, 3], F32, name="gi_ps", tag="scr")
    gh_ps = ps.tile([128, 3], F32, name="gh_ps", tag="scr")
    for g in range(3):
        for k in range(KH):
            nc.tensor.matmul(gi_ps[:, g:g + 1], lhsT=wih_sb[:, k, g, :],
                             rhs=x_sb[:, k:k + 1],
                             start=(k == 0), stop=(k == KH - 1))
    for g in range(3):
        for k in range(KH):
            nc.tensor.matmul(gh_ps[:, g:g + 1], lhsT=whh_sb[:, k, g, :],
                             rhs=h_sb[:, k:k + 1],
                             start=(k == 0), stop=(k == KH - 1))
    gh_sb = sb.tile([128, 3], F32, name="gh_sb")
    nc.vector.tensor_copy(gh_sb[:], gh_ps[:])
    rz_sb = sb.tile([128, 2], F32, name="rz_sb")
    nc.vector.tensor_add(rz_sb[:], gi_ps[:, 0:2], gh_sb[:, 0:2])
    r = sb.tile([128, 1], F32, name="gru_r")
    z = sb.tile([128, 1], F32, name="gru_z")
    nc.scalar.activation(r[:], rz_sb[:, 0:1], AF.Sigmoid, bias=bsum_sb[:, 0:1])
    nc.scalar.activation(z[:], rz_sb[:, 1:2], AF.Sigmoid, bias=bsum_sb[:, 1:2])
    hnb = sb.tile([128, 1], F32, name="gru_hnb")
    nc.scalar.activation(hnb[:], gh_sb[:, 2:3], AF.Identity, bias=bh_sb[:, 2:3])
    rn = sb.tile([128, 1], F32, name="gru_rn")
    nc.vector.tensor_mul(rn[:], r[:], hnb[:])
    tin = sb.tile([128, 1], F32, name="gru_tin")
    nc.vector.tensor_add(tin[:], gi_ps[:, 2:3], rn[:])
    n = sb.tile([128, 1], F32, name="gru_n")
    nc.scalar.activation(n[:], tin[:], AF.Tanh, bias=bi_sb[:, 2:3])
    d = sb.tile([128, 1], F32, name="gru_d")
    nc.vector.tensor_sub(d[:], h_col[:], n[:])
    zd = sb.tile([128, 1], F32, name="gru_zd")
    nc.vector.tensor_mul(zd[:], z[:], d[:])
    h_new = sb.tile([128, 1], F32, name="gru_hnew")
    nc.vector.tensor_add(h_new[:], n[:], zd[:])
    return h_new


def build_program():
    nc = bacc.Bacc("TRN2", target_bir_lowering=False, debug=False, num_devices=NC)
    AF = mybir.ActivationFunctionType

    # ---- kernel I/O (per core) ----
    idx8 = nc.dram_tensor("idx8", [KH, 1], mybir.dt.int32, kind="ExternalInput")
    embr = nc.dram_tensor("embr", [V * KH, 128], F32, kind="ExternalInput")
    hidden8 = nc.dram_tensor("hidden8", [KH, 128], F32, kind="ExternalInput")
    h0col = nc.dram_tensor("h0col", [128, 1], F32, kind="ExternalInput")
    attn_w = nc.dram_tensor("attn_w", [2 * H, SS], F32, kind="ExternalInput")
    attn_b = nc.dram_tensor("attn_b", [1, SS], F32, kind="ExternalInput")
    enc = nc.dram_tensor("enc", [S, 128], BF16, kind="ExternalInput")
    comb_w = nc.dram_tensor("comb_w", [2 * H, 128], BF16, kind="ExternalInput")
    comb_b = nc.dram_tensor("comb_b", [1, 128], F32, kind="ExternalInput")
    wih = nc.dram_tensor("wih", [H, 3, 128], F32, kind="ExternalInput")
    whh = nc.dram_tensor("whh", [H, 3, 128], F32, kind="ExternalInput")
    bi = nc.dram_tensor("bi", [128, 3], F32, kind="ExternalInput")
    bh = nc.dram_tensor("bh", [128, 3], F32, kind="ExternalInput")
    ow = nc.dram_tensor("ow", [H, VS], BF16, kind="ExternalInput")
    ob = nc.dram_tensor("ob", [128, MCH], F32, kind="ExternalInput")

    attn_out = nc.dram_tensor("attn_out", [128, KS], F32, kind="ExternalOutput")
    h_out = nc.dram_tensor("h_out", [128, 1], F32, kind="ExternalOutput")
    logp_out = nc.dram_tensor("logp_out", [128, MCH], F32, kind="ExternalOutput")

    rg = [list(range(NC))]

    with tile.TileContext(nc) as tc:
        with (
            tc.tile_pool(name="w", bufs=1) as wp,
            tc.tile_pool(name="sb", bufs=2) as sb,
            tc.tile_pool(name="owp", bufs=KH) as owp,
            tc.tile_pool(name="ps", bufs=4, space="PSUM") as ps,
            tc.tile_pool(name="lgps", bufs=2, space="PSUM") as lgps,
            tc.tile_pool(name="dram", bufs=1, space="DRAM") as dram,
        ):
            # ---- tiny chain-critical loads first (SP queue) ----
            idx_sb = sb.tile([KH, 1], mybir.dt.int32, name="idx_sb", bufs=1)
            nc.sync.dma_start(out=idx_sb[:], in_=idx8[:, :])
            h08 = sb.tile([KH, 128], F32, name="h08", bufs=1)
            nc.sync.dma_start(out=h08[:], in_=hidden8[:, :])
            h0_sb = wp.tile([128, 1], F32)
            nc.sync.dma_start(out=h0_sb[:], in_=h0col[:, :])
            bi_sb = wp.tile([128, 3], F32)
            nc.sync.dma_start(out=bi_sb[:], in_=bi[:, :])
            bh_sb = wp.tile([128, 3], F32)
            nc.sync.dma_start(out=bh_sb[:], in_=bh[:, :])
            ab_sb = wp.tile([1, SS], F32)
            nc.sync.dma_start(out=ab_sb[:], in_=attn_b[:, :])
            cbias_sb = wp.tile([1, 128], F32)
            nc.sync.dma_start(out=cbias_sb[:], in_=comb_b[:, :])
            ob_sb = wp.tile([128, MCH], F32)
            nc.sync.dma_start(out=ob_sb[:], in_=ob[:, :])

            # ---- weight streams (SP queue, consumption order) ----
            attn_sb = wp.tile([128, K2H, SS], F32)
            nc.sync.dma_start(out=attn_sb[:],
                              in_=attn_w[:, :].rearrange("(k p) n -> p k n", p=128))
            wih_sb = wp.tile([128, KH, 3, 128], F32)
            nc.sync.dma_start(out=wih_sb[:],
                              in_=wih[:, :, :].rearrange("(k p) g n -> p k g n", p=128))
            whh_sb = wp.tile([128, KH, 3, 128], F32)
            nc.sync.dma_start(out=whh_sb[:],
                              in_=whh[:, :, :].rearrange("(k p) g n -> p k g n", p=128))
            enc_sb = wp.tile([128, KS, 128], BF16)
            nc.sync.dma_start(out=enc_sb[:],
                              in_=enc[:, :].rearrange("(k p) n -> p k n", p=128))
            comb_sb = wp.tile([128, K2H, 128], BF16)
            nc.sync.dma_start(out=comb_sb[:],
                              in_=comb_w[:, :].rearrange("(k p) n -> p k n", p=128))
            ow_tiles = []
            for k in range(KH):
                ow_k = owp.tile([128, MCH, 128], BF16, name="ow_k")
                nc.sync.dma_start(
                    out=ow_k[:],
                    in_=ow[k * 128:(k + 1) * 128, :].rearrange("p (m n) -> p m n", n=128))
                ow_tiles.append(ow_k)

            # ---- constants ----
            ident = wp.tile([128, 128], F32)
            make_identity(nc, ident[:])
            ones_p = wp.tile([128, 1], F32)
            nc.vector.memset(ones_p[:], 1.0)
            ones_1 = wp.tile([1, 128], F32)
            nc.vector.memset(ones_1[:], 1.0)
            bsum_sb = wp.tile([128, 2], F32)
            nc.vector.tensor_add(bsum_sb[:], bi_sb[:, 0:2], bh_sb[:, 0:2])

            # ---- embedding row gather (gpsimd) + partition-major transposes --
            er8 = sb.tile([KH, 128], F32, name="er8", bufs=1)
            nc.gpsimd.indirect_dma_start(
                out=er8[:], out_offset=None,
                in_=embr[:, :],
                in_offset=IndirectOffsetOnAxis(ap=idx_sb[:, :1], axis=0))
            tp_e = ps.tile([128, KH], F32, name="tp_e", tag="scr")
            nc.tensor.transpose(tp_e[:], er8[:], ident[:KH, :KH])
            emb_pm = sb.tile([128, KH], F32, name="emb_pm", bufs=1)
            nc.vector.tensor_copy(emb_pm[:], tp_e[:])
            embc_bf = sb.tile([128, KH], BF16, name="embc_bf", bufs=1)
            nc.vector.tensor_copy(embc_bf[:], tp_e[:])
            tp_h0 = ps.tile([128, KH], F32, name="tp_h0", tag="scr")
            nc.tensor.transpose(tp_h0[:], h08[:], ident[:KH, :KH])
            hid_sb = sb.tile([128, KH], F32, name="hid_sb", bufs=1)
            nc.vector.tensor_copy(hid_sb[:], tp_h0[:])

            # ---- attn scores (this core's 512 cols of S) + AG1 ----
            t1_ps = ps.tile([1, SS], F32, name="t1_ps", tag="scr")
            for k in range(K2H):
                lhs = emb_pm[:, k:k + 1] if k < KH else hid_sb[:, k - KH:k - KH + 1]
                nc.tensor.matmul(t1_ps[:], lhsT=lhs, rhs=attn_sb[:, k, :],
                                 start=(k == 0), stop=(k == K2H - 1))
            t1_sb = sb.tile([1, SS], F32, name="t1_sb")
            nc.vector.tensor_add(t1_sb[:], t1_ps[:], ab_sb[:])
            ag1_in = dram.tile([1, SS], F32)
            ag1_out = dram.tile([NC, SS], F32)
            nc.gpsimd.dma_start(out=ag1_in[:], in_=t1_sb[:])
            nc.gpsimd.collective_compute(
                "AllGather", mybir.AluOpType.bypass, replica_groups=rg,
                ins=[ag1_in.opt()], outs=[ag1_out.opt()])

            # ---- softmax over full S (partition-major, no max-sub needed) ----
            t1_32 = sb.tile([KS, 128], F32, name="t1_32")
            nc.gpsimd.dma_start(out=t1_32[:],
                                in_=ag1_out[:].rearrange("a b -> (a b)")
                                .rearrange("(k p) -> k p", p=128))
            tp_t1 = ps.tile([128, KS], F32, name="tp_t1", tag="scr")
            nc.tensor.transpose(tp_t1[:], t1_32[:], ident[:KS, :KS])
            u_sb = sb.tile([128, KS], F32, name="u_sb")
            srow = sb.tile([128, 1], F32, name="srow")
            nc.scalar.activation(u_sb[:], tp_t1[:], AF.Exp, accum_out=srow[:])
            s1_ps = ps.tile([1, 1], F32, name="s1_ps", tag="scr")
            nc.tensor.matmul(s1_ps[:], lhsT=srow[:], rhs=ones_p[:],
                             start=True, stop=True)
            rs_sb = sb.tile([1, 1], F32, name="rs_sb")
            nc.vector.reciprocal(rs_sb[:], s1_ps[:])
            rb_ps = ps.tile([128, 1], F32, name="rb_ps", tag="scr")
            nc.tensor.matmul(rb_ps[:], lhsT=ones_1[:], rhs=rs_sb[:],
                             start=True, stop=True)
            rs_bc = sb.tile([128, 1], F32, name="rs_bc")
            nc.vector.tensor_copy(rs_bc[:], rb_ps[:])
            aw_sb = sb.tile([128, KS], F32, name="aw_sb")
            nc.vector.tensor_scalar_mul(aw_sb[:], u_sb[:], rs_bc[:])
            aw_bf = sb.tile([128, KS], BF16, name="aw_bf")
            nc.vector.tensor_copy(aw_bf[:], aw_sb[:])
            nc.gpsimd.dma_start(out=attn_out[:, :], in_=aw_sb[:])

            # ---- attn_applied (this core's 128 cols of H) + AG2 ----
            aa_ps = ps.tile([1, 128], F32, name="aa_ps", tag="scr")
            for k in range(KS):
                nc.tensor.matmul(aa_ps[:], lhsT=aw_bf[:, k:k + 1],
                                 rhs=enc_sb[:, k, :],
                                 start=(k == 0), stop=(k == KS - 1))
            aa_sb = sb.tile([1, 128], F32, name="aa_sb")
            nc.vector.tensor_copy(aa_sb[:], aa_ps[:])
            ag2_in = dram.tile([1, 128], F32)
            ag2_out = dram.tile([NC, 128], F32)
            nc.gpsimd.dma_start(out=ag2_in[:], in_=aa_sb[:])
            nc.gpsimd.collective_compute(
                "AllGather", mybir.AluOpType.bypass, replica_groups=rg,
                ins=[ag2_in.opt()], outs=[ag2_out.opt()])

            # ---- comb + AG3 ----
            aa8 = sb.tile([KH, 128], F32, name="aa8")
            nc.gpsimd.dma_start(out=aa8[:], in_=ag2_out[:])
            tp_aa = ps.tile([128, KH], F32, name="tp_aa", tag="scr")
            nc.tensor.transpose(tp_aa[:], aa8[:], ident[:KH, :KH])
            aa_bf = sb.tile([128, KH], BF16, name="aa_bf")
            nc.vector.tensor_copy(aa_bf[:], tp_aa[:])
            cb_ps = ps.tile([1, 128], F32, name="cb_ps", tag="scr")
            for k in range(K2H):
                lhs = embc_bf[:, k:k + 1] if k < KH else aa_bf[:, k - KH:k - KH + 1]
                nc.tensor.matmul(cb_ps[:], lhsT=lhs, rhs=comb_sb[:, k, :],
                                 start=(k == 0), stop=(k == K2H - 1))
            cbo_sb = sb.tile([1, 128], F32, name="cbo_sb")
            nc.vector.tensor_add(cbo_sb[:], cb_ps[:], cbias_sb[:])
            ag3_in = dram.tile([1, 128], F32)
            ag3_out = dram.tile([NC, 128], F32)
            nc.gpsimd.dma_start(out=ag3_in[:], in_=cbo_sb[:])
            nc.gpsimd.collective_compute(
                "AllGather", mybir.AluOpType.bypass, replica_groups=rg,
                ins=[ag3_in.opt()], outs=[ag3_out.opt()])

            # ---- GRU layer 1 ----
            x18 = sb.tile([KH, 128], F32, name="x18")
            nc.gpsimd.dma_start(out=x18[:], in_=ag3_out[:])
            tp_x1 = ps.tile([128, KH], F32, name="tp_x1", tag="scr")
            nc.tensor.transpose(tp_x1[:], x18[:], ident[:KH, :KH])
            x1_sb = sb.tile([128, KH], F32, name="x1_sb")
            nc.scalar.activation(x1_sb[:], tp_x1[:], AF.Relu)
            h1_col = _gru_layer(nc, sb, ps, x1_sb, hid_sb, h0_sb,
                                wih_sb, whh_sb, bi_sb, bh_sb, bsum_sb)
            ag4_in = dram.tile([128, 1], F32)
            ag4_out = dram.tile([NC * 128, 1], F32)
            nc.gpsimd.dma_start(out=ag4_in[:], in_=h1_col[:])
            nc.gpsimd.collective_compute(
                "AllGather", mybir.AluOpType.bypass, replica_groups=rg,
                ins=[ag4_in.opt()], outs=[ag4_out.opt()])

            # ---- GRU layer 2 ----
            h18 = sb.tile([KH, 128], F32, name="h18")
            nc.gpsimd.dma_start(out=h18[:],
                                in_=ag4_out[:].rearrange("(a b) o -> a (b o)", b=128))
            tp_h1 = ps.tile([128, KH], F32, name="tp_h1", tag="scr")
            nc.tensor.transpose(tp_h1[:], h18[:], ident[:KH, :KH])
            x2_sb = sb.tile([128, KH], F32, name="x2_sb")
            nc.scalar.activation(x2_sb[:], tp_h1[:], AF.Relu)
            h1f_sb = sb.tile([128, KH], F32, name="h1f_sb")
            nc.vector.tensor_copy(h1f_sb[:], tp_h1[:])
            h2_col = _gru_layer(nc, sb, ps, x2_sb, h1f_sb, h1_col,
                                wih_sb, whh_sb, bi_sb, bh_sb, bsum_sb)
            nc.gpsimd.dma_start(out=h_out[:, :], in_=h2_col[:])
            ag5_in = dram.tile([128, 1], F32)
            ag5_out = dram.tile([NC * 128, 1], F32)
            nc.gpsimd.dma_start(out=ag5_in[:], in_=h2_col[:])
            nc.gpsimd.collective_compute(
                "AllGather", mybir.AluOpType.bypass, replica_groups=rg,
                ins=[ag5_in.opt()], outs=[ag5_out.opt()])

            # ---- logits over this core's 6400 padded vocab cols ----
            h28 = sb.tile([KH, 128], F32, name="h28")
            nc.gpsimd.dma_start(out=h28[:],
                                in_=ag5_out[:].rearrange("(a b) o -> a (b o)", b=128))
            tp_h2 = ps.tile([128, KH], F32, name="tp_h2", tag="scr")
            nc.tensor.transpose(tp_h2[:], h28[:], ident[:KH, :KH])
            h2w_sb = sb.tile([128, KH], BF16, name="h2w_sb")
            nc.vector.tensor_copy(h2w_sb[:], tp_h2[:])

            lg_sb = sb.tile([128, MCH], F32, name="lg_sb", bufs=1)
            nc.vector.tensor_copy(lg_sb[:], ob_sb[:])
            for k in range(KH):
                lg_ps = lgps.tile([128, MCH], F32, name="lg_ps", tag="lg")
                for m in range(MCH):
                    nc.tensor.matmul(lg_ps[:, m:m + 1], lhsT=ow_tiles[k][:, m, :],
                                     rhs=h2w_sb[:, k:k + 1],
                                     start=True, stop=True)
                nc.vector.tensor_add(lg_sb[:], lg_sb[:], lg_ps[:])

            # ---- log-softmax denominator via AG6 ----
            elg = sb.tile([128, MCH], F32, name="elg")
            srl = sb.tile([128, 1], F32, name="srl")
            nc.scalar.activation(elg[:], lg_sb[:], AF.Exp, accum_out=srl[:])
            st_ps = ps.tile([1, 1], F32, name="st_ps", tag="scr")
            nc.tensor.matmul(st_ps[:], lhsT=srl[:], rhs=ones_p[:],
                             start=True, stop=True)
            sc8 = sb.tile([1, 8], F32, name="sc8")
            nc.vector.memset(sc8[:], 0.0)
            nc.vector.tensor_copy(sc8[:, 0:1], st_ps[:])
            ag6_in = dram.tile([1, 8], F32)
            ag6_out = dram.tile([NC, 8], F32)
            nc.gpsimd.dma_start(out=ag6_in[:], in_=sc8[:])
            nc.gpsimd.collective_compute(
                "AllGather", mybir.AluOpType.bypass, replica_groups=rg,
                ins=[ag6_in.opt()], outs=[ag6_out.opt()])
            sg = sb.tile([1, NC, 8], F32, name="sg")
            nc.gpsimd.dma_start(out=sg[:],
                                in_=ag6_out[:].rearrange("a b -> (a b)")[None, :]
                                .rearrange("o (a b) -> o a b", b=8))
            tot = sb.tile([1, 1], F32, name="tot")
            nc.vector.tensor_reduce(tot[:], sg[:, :, 0:1],
                                    axis=mybir.AxisListType.XY, op=mybir.AluOpType.add)
            lse = sb.tile([1, 1], F32, name="lse")
            nc.scalar.activation(lse[:], tot[:], AF.Ln)
            lb_ps = ps.tile([128, 1], F32, name="lb_ps", tag="scr")
            nc.tensor.matmul(lb_ps[:], lhsT=ones_1[:], rhs=lse[:],
                             start=True, stop=True)
            lse_bc = sb.tile([128, 1], F32, name="lse_bc")
            nc.vector.tensor_copy(lse_bc[:], lb_ps[:])
            lp_sb = sb.tile([128, MCH], F32, name="lp_sb")
            nc.vector.tensor_scalar(lp_sb[:], lg_sb[:], lse_bc[:], None,
                                    op0=mybir.AluOpType.subtract)
            nc.sync.dma_start(out=logp_out[:, :], in_=lp_sb[:])

    nc.compile()
    return nc


def shard_inputs(inputs):
    """FULL numpy inputs -> list of 8 per-core input maps."""
    import ml_dtypes
    bf16 = ml_dtypes.bfloat16

    f = lambda k: np.ascontiguousarray(np.asarray(inputs[k], np.float32))
    idx = int(np.asarray(inputs["input"]).ravel()[0])
    emb = f("emb")
    hidden = f("hidden").reshape(H)
    attn_W, attn_b = f("attn_W"), f("attn_b")
    enc = f("encoder_outputs")
    comb_W, comb_b = f("comb_W"), f("comb_b")
    Wih, Whh = f("gru_Wih"), f("gru_Whh")
    bih, bhh = f("gru_bih"), f("gru_bhh")
    out_W, out_b = f("out_W"), f("out_b")

    owp = np.zeros((H, VP), np.float32)
    owp[:, :V] = out_W
    owp = owp.astype(bf16)
    obp = np.full((VP,), -1e30, np.float32)
    obp[:V] = out_b

    A_ih = np.ascontiguousarray(Wih.T.reshape(H, 3, H))
    A_hh = np.ascontiguousarray(Whh.T.reshape(H, 3, H))
    bi3 = bih.reshape(3, H)
    bh3 = bhh.reshape(3, H)

    embr = emb.reshape(V * KH, 128)
    idx8 = (idx * KH + np.arange(KH, dtype=np.int32)).reshape(KH, 1)
    hidden8 = np.ascontiguousarray(hidden.reshape(KH, 128))
    enc_bf = enc.astype(bf16)
    comb_bf = comb_W.astype(bf16)

    in_maps = []
    for c in range(NC):
        sl = slice(c * 128, (c + 1) * 128)
        ssl = slice(c * SS, (c + 1) * SS)
        vsl = slice(c * VS, (c + 1) * VS)
        in_maps.append({
            "idx8": idx8,
            "embr": embr,
            "hidden8": hidden8,
            "h0col": np.ascontiguousarray(hidden[sl, None]),
            "attn_w": np.ascontiguousarray(attn_W[:, ssl]),
            "attn_b": np.ascontiguousarray(attn_b[None, ssl]),
            "enc": np.ascontiguousarray(enc_bf[:, sl]),
            "comb_w": np.ascontiguousarray(comb_bf[:, sl]),
            "comb_b": np.ascontiguousarray(comb_b[None, sl]),
            "wih": np.ascontiguousarray(A_ih[:, :, sl]),
            "whh": np.ascontiguousarray(A_hh[:, :, sl]),
            "bi": np.ascontiguousarray(bi3[:, sl].T),
            "bh": np.ascontiguousarray(bh3[:, sl].T),
            "ow": np.ascontiguousarray(owp[:, vsl]),
            "ob": np.ascontiguousarray(obp[vsl].reshape(MCH, 128).T),
        })
    return in_maps


def unshard_outputs(results):
    """list of per-core {name: array} -> (log_probs, hidden, attn_weights)."""
    logp = np.concatenate(
        [np.asarray(r["logp_out"]).T.reshape(-1) for r in results])[:V][None, :]
    h = np.concatenate([np.asarray(r["h_out"]).ravel() for r in results])[None, None, :]
    aw = np.asarray(results[0]["attn_out"]).T.reshape(-1)[None, :]
    return (np.ascontiguousarray(logp, np.float32),
            np.ascontiguousarray(h, np.float32),
            np.ascontiguousarray(aw, np.float32))


def _get_program():
    if "nc" not in _CACHE:
        _CACHE["nc"] = build_program()
    return _CACHE["nc"]


def kernel(**inputs):
    nc = _get_program()
    in_maps = shard_inputs(inputs)
    res = run_bass_kernel_spmd(nc, in_maps, core_ids=list(range(NC)), trace=False)
    return unshard_outputs(res.results)


# revision 11
# speedup vs baseline: 1.7420x; 1.1125x over previous
"""AttnDecoderRNN step on 8 Trainium2 NeuronCores (Bass/Tile, tensor-parallel).

Sharding (8 cores, stitched with AllGather where a full vector is needed):
  - embedding: table replicated (viewed [V*8, 128]); each core indirect-DMAs
    the token row as 8 chunk-rows (4KB HBM read) -- no collective.
  - attn scores: attn_W sharded over S columns (512/core); AG1 -> softmax
    computed partition-major on every core.
  - attn_applied: encoder_outputs sharded over H columns (128/core); AG2.
  - comb: comb_W sharded over H columns; AG3.
  - GRU (x2, shared weights): gate outputs sharded over H (128/core for each
    of the 3 gates); h re-gathered with AG4/AG5 for the next contraction.
  - logits: out_W sharded over vocab columns (6400/core after padding V to
    51200); log_softmax denominator from an AG of per-core exp-sums (AG6);
    each core writes its own log-prob shard, the host concatenates.

Precision: f32 everywhere except out_W / encoder_outputs / comb_W (and the
activation vectors feeding those three matmuls), which run in bf16.
Verified end-to-end error vs the f32 reference: log_probs ~2e-4, hidden and
attn_weights ~2e-5 (max relative).

Queue plan: all weight streaming on the sync (SP) DMA queue in consumption
order; collective bounce-buffer traffic and gather loads on the gpsimd queue
so they interleave with the collectives; compute engines stay DMA-free.
"""

import numpy as np

import concourse.bacc as bacc
import concourse.bass as bass
import concourse.tile as tile
from concourse import mybir
from concourse.bass import IndirectOffsetOnAxis
from concourse.bass_utils import run_bass_kernel_spmd
from concourse.masks import make_identity

H = 1024
S = 4096
V = 50257
L = 2
NC = 8
VP = 51200          # padded vocab
VS = VP // NC       # 6400 vocab cols per core
MCH = VS // 128     # 50 m-chunks per core
SS = S // NC        # 512 attn cols per core
KH = H // 128       # 8 k-chunks for H contraction
K2H = 2 * H // 128  # 16 k-chunks for 2H contraction
KS = S // 128       # 32 k-chunks for S contraction

F32 = mybir.dt.float32
BF16 = mybir.dt.bfloat16

_CACHE = {}


def _gru_layer(nc, sb, ps, x_sb, h_sb, h_col, wih_sb, whh_sb,
               bi_sb, bh_sb, bsum_sb):
    """One GRU cell step; returns h_new [128,1] (this core's slice)."""
    AF = mybir.ActivationFunctionType
    gi_ps = ps.tile([128, 3], F32, name="gi_ps", tag="scr")
    gh_ps = ps.tile([128, 3], F32, name="gh_ps", tag="scr")
    for g in range(3):
        for k in range(KH):
            nc.tensor.matmul(gi_ps[:, g:g + 1], lhsT=wih_sb[:, k, g, :],
                             rhs=x_sb[:, k:k + 1],
                             start=(k == 0), stop=(k == KH - 1))
    for g in range(3):
        for k in range(KH):
            nc.tensor.matmul(gh_ps[:, g:g + 1], lhsT=whh_sb[:, k, g, :],
                             rhs=h_sb[:, k:k + 1],
                             start=(k == 0), stop=(k == KH - 1))
    gh_sb = sb.tile([128, 3], F32, name="gh_sb")
    nc.vector.tensor_copy(gh_sb[:], gh_ps[:])
    rz_sb = sb.tile([128, 2], F32, name="rz_sb")
    nc.vector.tensor_add(rz_sb[:], gi_ps[:, 0:2], gh_sb[:, 0:2])
    # sigmoid via tanh to stay on the Exp/Tanh activation table:
    # sigmoid(a) = 0.5 + 0.5*tanh(a/2)
    v = sb.tile([128, 1], F32, name="gru_v")  # tanh(r-gate/2)
    u = sb.tile([128, 1], F32, name="gru_u")  # tanh(z-gate/2)
    nc.scalar.activation(v[:], rz_sb[:, 0:1], AF.Tanh,
                         bias=bsum_sb[:, 0:1], scale=0.5)
    nc.scalar.activation(u[:], rz_sb[:, 1:2], AF.Tanh,
                         bias=bsum_sb[:, 1:2], scale=0.5)
    hnb = sb.tile([128, 1], F32, name="gru_hnb")
    nc.vector.tensor_scalar_add(hnb[:], gh_sb[:, 2:3], bh_sb[:, 2:3])
    # rn = sigmoid(r)*hnb = 0.5*(hnb + tanh(r/2)*hnb)
    vh = sb.tile([128, 1], F32, name="gru_vh")
    nc.vector.tensor_mul(vh[:], v[:], hnb[:])
    rh = sb.tile([128, 1], F32, name="gru_rh")
    nc.vector.tensor_add(rh[:], hnb[:], vh[:])
    tin = sb.tile([128, 1], F32, name="gru_tin")
    nc.vector.tensor_scalar(tin[:], rh[:], 0.5, None, op0=mybir.AluOpType.mult)
    nc.vector.tensor_add(tin[:], gi_ps[:, 2:3], tin[:])
    n = sb.tile([128, 1], F32, name="gru_n")
    nc.scalar.activation(n[:], tin[:], AF.Tanh, bias=bi_sb[:, 2:3])
    # h_new = n + sigmoid(z)*(h-n) = n + 0.5*((h-n) + tanh(z/2)*(h-n))
    d = sb.tile([128, 1], F32, name="gru_d")
    nc.vector.tensor_sub(d[:], h_col[:], n[:])
    ud = sb.tile([128, 1], F32, name="gru_ud")
    nc.vector.tensor_mul(ud[:], u[:], d[:])
    e = sb.tile([128, 1], F32, name="gru_e")
    nc.vector.tensor_add(e[:], d[:], ud[:])
    nc.vector.tensor_scalar(e[:], e[:], 0.5, None, op0=mybir.AluOpType.mult)
    h_new = sb.tile([128, 1], F32, name="gru_hnew")
    nc.vector.tensor_add(h_new[:], n[:], e[:])
    return h_new


def build_program():
    nc = bacc.Bacc("TRN2", target_bir_lowering=False, debug=False, num_devices=NC)
    AF = mybir.ActivationFunctionType

    # ---- kernel I/O (per core) ----
    idx8 = nc.dram_tensor("idx8", [KH, 1], mybir.dt.int32, kind="ExternalInput")
    embr = nc.dram_tensor("embr", [V * KH, 128], F32, kind="ExternalInput")
    hidden8 = nc.dram_tensor("hidden8", [KH, 128], F32, kind="ExternalInput")
    h0col = nc.dram_tensor("h0col", [128, 1], F32, kind="ExternalInput")
    attn_w = nc.dram_tensor("attn_w", [2 * H, SS], F32, kind="ExternalInput")
    attn_b = nc.dram_tensor("attn_b", [1, SS], F32, kind="ExternalInput")
    enc = nc.dram_tensor("enc", [S, 128], BF16, kind="ExternalInput")
    comb_w = nc.dram_tensor("comb_w", [2 * H, 128], BF16, kind="ExternalInput")
    comb_b = nc.dram_tensor("comb_b", [1, 128], F32, kind="ExternalInput")
    wih = nc.dram_tensor("wih", [H, 3, 128], F32, kind="ExternalInput")
    whh = nc.dram_tensor("whh", [H, 3, 128], F32, kind="ExternalInput")
    bi = nc.dram_tensor("bi", [128, 3], F32, kind="ExternalInput")
    bh = nc.dram_tensor("bh", [128, 3], F32, kind="ExternalInput")
    ow = nc.dram_tensor("ow", [H, VS], BF16, kind="ExternalInput")
    ob = nc.dram_tensor("ob", [128, MCH], F32, kind="ExternalInput")

    attn_out = nc.dram_tensor("attn_out", [128, KS], F32, kind="ExternalOutput")
    h_out = nc.dram_tensor("h_out", [128, 1], F32, kind="ExternalOutput")
    logp_out = nc.dram_tensor("logp_out", [128, MCH], F32, kind="ExternalOutput")

    rg = [list(range(NC))]

    with tile.TileContext(nc) as tc:
        with (
            tc.tile_pool(name="w", bufs=1) as wp,
            tc.tile_pool(name="sb", bufs=2) as sb,
            tc.tile_pool(name="owp", bufs=KH) as owp,
            tc.tile_pool(name="ps", bufs=4, space="PSUM") as ps,
            tc.tile_pool(name="lgps", bufs=2, space="PSUM") as lgps,
            tc.tile_pool(name="dram", bufs=1, space="DRAM") as dram,
        ):
            # ---- tiny chain-critical loads first (SP queue) ----
            idx_sb = sb.tile([KH, 1], mybir.dt.int32, name="idx_sb", bufs=1)
            nc.sync.dma_start(out=idx_sb[:], in_=idx8[:, :])
            h08 = sb.tile([KH, 128], F32, name="h08", bufs=1)
            nc.sync.dma_start(out=h08[:], in_=hidden8[:, :])

            # ---- weight streams (SP queue, consumption order) ----
            # attn_W chunked so each k-chunk matmul starts as soon as its
            # 256KB slice lands (pipeline DMA with the 16 PE matmuls).
            attn_sb = wp.tile([128, K2H, SS], F32)
            for k in range(K2H):
                nc.sync.dma_start(out=attn_sb[:, k, :],
                                  in_=attn_w[k * 128:(k + 1) * 128, :])
            ab_sb = wp.tile([1, SS], F32)
            nc.sync.dma_start(out=ab_sb[:], in_=attn_b[:, :])
            h0_sb = wp.tile([128, 1], F32)
            nc.sync.dma_start(out=h0_sb[:], in_=h0col[:, :])
            bi_sb = wp.tile([128, 3], F32)
            nc.sync.dma_start(out=bi_sb[:], in_=bi[:, :])
            bh_sb = wp.tile([128, 3], F32)
            nc.sync.dma_start(out=bh_sb[:], in_=bh[:, :])
            cbias_sb = wp.tile([1, 128], F32)
            nc.sync.dma_start(out=cbias_sb[:], in_=comb_b[:, :])
            ob_sb = wp.tile([128, MCH], F32)
            nc.sync.dma_start(out=ob_sb[:], in_=ob[:, :])
            wih_sb = wp.tile([128, KH, 3, 128], F32)
            nc.sync.dma_start(out=wih_sb[:],
                              in_=wih[:, :, :].rearrange("(k p) g n -> p k g n", p=128))
            whh_sb = wp.tile([128, KH, 3, 128], F32)
            nc.sync.dma_start(out=whh_sb[:],
                              in_=whh[:, :, :].rearrange("(k p) g n -> p k g n", p=128))
            enc_sb = wp.tile([128, KS, 128], BF16)
            nc.sync.dma_start(out=enc_sb[:],
                              in_=enc[:, :].rearrange("(k p) n -> p k n", p=128))
            comb_sb = wp.tile([128, K2H, 128], BF16)
            nc.sync.dma_start(out=comb_sb[:],
                              in_=comb_w[:, :].rearrange("(k p) n -> p k n", p=128))
            ow_tiles = []
            for k in range(KH):
                ow_k = owp.tile([128, MCH, 128], BF16, name="ow_k")
                nc.sync.dma_start(
                    out=ow_k[:],
                    in_=ow[k * 128:(k + 1) * 128, :].rearrange("p (m n) -> p m n", n=128))
                ow_tiles.append(ow_k)

            # ---- constants ----
            ident = wp.tile([128, 128], F32)
            make_identity(nc, ident[:])
            ones_p = wp.tile([128, 1], F32)
            nc.vector.memset(ones_p[:], 1.0)
            ones_1 = wp.tile([1, 128], F32)
            nc.vector.memset(ones_1[:], 1.0)
            # prewarm the Exp/Tanh activation table off the critical path
            warm = sb.tile([1, 1], F32, name="warm", bufs=1)
            nc.vector.memset(warm[:], 0.0)
            nc.scalar.activation(warm[:], warm[:], AF.Exp)
            # halved gate bias: tanh(0.5*a + 0.5*(bi+bh)) for the sigmoid trick
            bsum_sb = wp.tile([128, 2], F32)
            nc.vector.tensor_add(bsum_sb[:], bi_sb[:, 0:2], bh_sb[:, 0:2])
            nc.vector.tensor_scalar(bsum_sb[:], bsum_sb[:], 0.5, None,
                                    op0=mybir.AluOpType.mult)

            # ---- embedding row gather (gpsimd) + partition-major transposes --
            er8 = sb.tile([KH, 128], F32, name="er8", bufs=1)
            nc.gpsimd.indirect_dma_start(
                out=er8[:], out_offset=None,
                in_=embr[:, :],
                in_offset=IndirectOffsetOnAxis(ap=idx_sb[:, :1], axis=0))
            tp_e = ps.tile([128, KH], F32, name="tp_e", tag="scr")
            nc.tensor.transpose(tp_e[:], er8[:], ident[:KH, :KH])
            emb_pm = sb.tile([128, KH], F32, name="emb_pm", bufs=1)
            nc.vector.tensor_copy(emb_pm[:], tp_e[:])
            embc_bf = sb.tile([128, KH], BF16, name="embc_bf", bufs=1)
            nc.vector.tensor_copy(embc_bf[:], tp_e[:])
            tp_h0 = ps.tile([128, KH], F32, name="tp_h0", tag="scr")
            nc.tensor.transpose(tp_h0[:], h08[:], ident[:KH, :KH])
            hid_sb = sb.tile([128, KH], F32, name="hid_sb", bufs=1)
            nc.vector.tensor_copy(hid_sb[:], tp_h0[:])

            # ---- attn scores (this core's 512 cols of S) + AG1 ----
            t1_ps = ps.tile([1, SS], F32, name="t1_ps", tag="scr")
            for k in range(K2H):
                lhs = emb_pm[:, k:k + 1] if k < KH else hid_sb[:, k - KH:k - KH + 1]
                nc.tensor.matmul(t1_ps[:], lhsT=lhs, rhs=attn_sb[:, k, :],
                                 start=(k == 0), stop=(k == K2H - 1))
            t1_sb = sb.tile([1, SS], F32, name="t1_sb")
            nc.vector.tensor_add(t1_sb[:], t1_ps[:], ab_sb[:])
            ag1_in = dram.tile([1, SS], F32)
            ag1_out = dram.tile([NC, SS], F32)
            nc.gpsimd.dma_start(out=ag1_in[:], in_=t1_sb[:])
            nc.gpsimd.collective_compute(
                "AllGather", mybir.AluOpType.bypass, replica_groups=rg,
                ins=[ag1_in.opt()], outs=[ag1_out.opt()])

            # ---- softmax over full S (partition-major, no max-sub needed) ----
            t1_32 = sb.tile([KS, 128], F32, name="t1_32")
            nc.gpsimd.dma_start(out=t1_32[:],
                                in_=ag1_out[:].rearrange("a b -> (a b)")
                                .rearrange("(k p) -> k p", p=128))
            tp_t1 = ps.tile([128, KS], F32, name="tp_t1", tag="scr")
            nc.tensor.transpose(tp_t1[:], t1_32[:], ident[:KS, :KS])
            u_sb = sb.tile([128, KS], F32, name="u_sb")
            srow = sb.tile([128, 1], F32, name="srow")
            nc.scalar.activation(u_sb[:], tp_t1[:], AF.Exp, accum_out=srow[:])
            s1_ps = ps.tile([1, 1], F32, name="s1_ps", tag="scr")
            nc.tensor.matmul(s1_ps[:], lhsT=srow[:], rhs=ones_p[:],
                             start=True, stop=True)
            rs_sb = sb.tile([1, 1], F32, name="rs_sb")
            nc.vector.reciprocal(rs_sb[:], s1_ps[:])
            rb_ps = ps.tile([128, 1], F32, name="rb_ps", tag="scr")
            nc.tensor.matmul(rb_ps[:], lhsT=ones_1[:], rhs=rs_sb[:],
                             start=True, stop=True)
            rs_bc = sb.tile([128, 1], F32, name="rs_bc")
            nc.vector.tensor_copy(rs_bc[:], rb_ps[:])
            aw_sb = sb.tile([128, KS], F32, name="aw_sb")
            nc.vector.tensor_scalar_mul(aw_sb[:], u_sb[:], rs_bc[:])
            aw_bf = sb.tile([128, KS], BF16, name="aw_bf")
            nc.vector.tensor_copy(aw_bf[:], aw_sb[:])
            nc.gpsimd.dma_start(out=attn_out[:, :], in_=aw_sb[:])

            # ---- attn_applied (this core's 128 cols of H) + AG2 ----
            aa_ps = ps.tile([1, 128], F32, name="aa_ps", tag="scr")
            for k in range(KS):
                nc.tensor.matmul(aa_ps[:], lhsT=aw_bf[:, k:k + 1],
                                 rhs=enc_sb[:, k, :],
                                 start=(k == 0), stop=(k == KS - 1))
            aa_sb = sb.tile([1, 128], F32, name="aa_sb")
            nc.vector.tensor_copy(aa_sb[:], aa_ps[:])
            ag2_in = dram.tile([1, 128], F32)
            ag2_out = dram.tile([NC, 128], F32)
            nc.gpsimd.dma_start(out=ag2_in[:], in_=aa_sb[:])
            nc.gpsimd.collective_compute(
                "AllGather", mybir.AluOpType.bypass, replica_groups=rg,
                ins=[ag2_in.opt()], outs=[ag2_out.opt()])

            # ---- comb + AG3 ----
            aa8 = sb.tile([KH, 128], F32, name="aa8")
            nc.gpsimd.dma_start(out=aa8[:], in_=ag2_out[:])
            tp_aa = ps.tile([128, KH], F32, name="tp_aa", tag="scr")
            nc.tensor.transpose(tp_aa[:], aa8[:], ident[:KH, :KH])
            aa_bf = sb.tile([128, KH], BF16, name="aa_bf")
            nc.vector.tensor_copy(aa_bf[:], tp_aa[:])
            cb_ps = ps.tile([1, 128], F32, name="cb_ps", tag="scr")
            for k in range(K2H):
                lhs = embc_bf[:, k:k + 1] if k < KH else aa_bf[:, k - KH:k - KH + 1]
                nc.tensor.matmul(cb_ps[:], lhsT=lhs, rhs=comb_sb[:, k, :],
                                 start=(k == 0), stop=(k == K2H - 1))
            cbo_sb = sb.tile([1, 128], F32, name="cbo_sb")
            nc.vector.tensor_add(cbo_sb[:], cb_ps[:], cbias_sb[:])
            ag3_in = dram.tile([1, 128], F32)
            ag3_out = dram.tile([NC, 128], F32)
            nc.gpsimd.dma_start(out=ag3_in[:], in_=cbo_sb[:])
            nc.gpsimd.collective_compute(
                "AllGather", mybir.AluOpType.bypass, replica_groups=rg,
                ins=[ag3_in.opt()], outs=[ag3_out.opt()])

            # ---- GRU layer 1 ----
            x18 = sb.tile([KH, 128], F32, name="x18")
            nc.gpsimd.dma_start(out=x18[:], in_=ag3_out[:])
            tp_x1 = ps.tile([128, KH], F32, name="tp_x1", tag="scr")
            nc.tensor.transpose(tp_x1[:], x18[:], ident[:KH, :KH])
            x1_sb = sb.tile([128, KH], F32, name="x1_sb")
            nc.scalar.activation(x1_sb[:], tp_x1[:], AF.Relu)
            h1_col = _gru_layer(nc, sb, ps, x1_sb, hid_sb, h0_sb,
                                wih_sb, whh_sb, bi_sb, bh_sb, bsum_sb)
            ag4_in = dram.tile([128, 1], F32)
            ag4_out = dram.tile([NC * 128, 1], F32)
            nc.gpsimd.dma_start(out=ag4_in[:], in_=h1_col[:])
            nc.gpsimd.collective_compute(
                "AllGather", mybir.AluOpType.bypass, replica_groups=rg,
                ins=[ag4_in.opt()], outs=[ag4_out.opt()])

            # ---- GRU layer 2 ----
            h18 = sb.tile([KH, 128], F32, name="h18")
            nc.gpsimd.dma_start(out=h18[:],
                                in_=ag4_out[:].rearrange("(a b) o -> a (b o)", b=128))
            tp_h1 = ps.tile([128, KH], F32, name="tp_h1", tag="scr")
            nc.tensor.transpose(tp_h1[:], h18[:], ident[:KH, :KH])
            x2_sb = sb.tile([128, KH], F32, name="x2_sb")
            nc.scalar.activation(x2_sb[:], tp_h1[:], AF.Relu)
            h1f_sb = sb.tile([128, KH], F32, name="h1f_sb")
            nc.vector.tensor_copy(h1f_sb[:], tp_h1[:])
            h2_col = _gru_layer(nc, sb, ps, x2_sb, h1f_sb, h1_col,
                                wih_sb, whh_sb, bi_sb, bh_sb, bsum_sb)
            nc.gpsimd.dma_start(out=h_out[:, :], in_=h2_col[:])
            ag5_in = dram.tile([128, 1], F32)
            ag5_out = dram.tile([NC * 128, 1], F32)
            nc.gpsimd.dma_start(out=ag5_in[:], in_=h2_col[:])
            nc.gpsimd.collective_compute(
                "AllGather", mybir.AluOpType.bypass, replica_groups=rg,
                ins=[ag5_in.opt()], outs=[ag5_out.opt()])

            # ---- logits over this core's 6400 padded vocab cols ----
            h28 = sb.tile([KH, 128], F32, name="h28")
            nc.gpsimd.dma_start(out=h28[:],
                                in_=ag5_out[:].rearrange("(a b) o -> a (b o)", b=128))
            tp_h2 = ps.tile([128, KH], F32, name="tp_h2", tag="scr")
            nc.tensor.transpose(tp_h2[:], h28[:], ident[:KH, :KH])
            h2w_sb = sb.tile([128, KH], BF16, name="h2w_sb")
            nc.vector.tensor_copy(h2w_sb[:], tp_h2[:])

            lg_sb = sb.tile([128, MCH], F32, name="lg_sb", bufs=1)
            nc.vector.tensor_copy(lg_sb[:], ob_sb[:])
            for k in range(KH):
                lg_ps = lgps.tile([128, MCH], F32, name="lg_ps", tag="lg")
                for m in range(MCH):
                    nc.tensor.matmul(lg_ps[:, m:m + 1], lhsT=ow_tiles[k][:, m, :],
                                     rhs=h2w_sb[:, k:k + 1],
                                     start=True, stop=True)
                nc.vector.tensor_add(lg_sb[:], lg_sb[:], lg_ps[:])

            # ---- log-softmax denominator via AG6 ----
            elg = sb.tile([128, MCH], F32, name="elg")
            srl = sb.tile([128, 1], F32, name="srl")
            nc.scalar.activation(elg[:], lg_sb[:], AF.Exp, accum_out=srl[:])
            st_ps = ps.tile([1, 1], F32, name="st_ps", tag="scr")
            nc.tensor.matmul(st_ps[:], lhsT=srl[:], rhs=ones_p[:],
                             start=True, stop=True)
            sc8 = sb.tile([1, 8], F32, name="sc8")
            nc.vector.memset(sc8[:], 0.0)
            nc.vector.tensor_copy(sc8[:, 0:1], st_ps[:])
            ag6_in = dram.tile([1, 8], F32)
            ag6_out = dram.tile([NC, 8], F32)
            nc.gpsimd.dma_start(out=ag6_in[:], in_=sc8[:])
            nc.gpsimd.collective_compute(
                "AllGather", mybir.AluOpType.bypass, replica_groups=rg,
                ins=[ag6_in.opt()], outs=[ag6_out.opt()])
            sg = sb.tile([1, NC, 8], F32, name="sg")
            nc.gpsimd.dma_start(out=sg[:],
                                in_=ag6_out[:].rearrange("a b -> (a b)")[None, :]
                                .rearrange("o (a b) -> o a b", b=8))
            tot = sb.tile([1, 1], F32, name="tot")
            nc.vector.tensor_reduce(tot[:], sg[:, :, 0:1],
                                    axis=mybir.AxisListType.XY, op=mybir.AluOpType.add)
            lse = sb.tile([1, 1], F32, name="lse")
            nc.scalar.activation(lse[:], tot[:], AF.Ln)
            lb_ps = ps.tile([128, 1], F32, name="lb_ps", tag="scr")
            nc.tensor.matmul(lb_ps[:], lhsT=ones_1[:], rhs=lse[:],
                             start=True, stop=True)
            lse_bc = sb.tile([128, 1], F32, name="lse_bc")
            nc.vector.tensor_copy(lse_bc[:], lb_ps[:])
            lp_sb = sb.tile([128, MCH], F32, name="lp_sb")
            nc.vector.tensor_scalar(lp_sb[:], lg_sb[:], lse_bc[:], None,
                                    op0=mybir.AluOpType.subtract)
            nc.sync.dma_start(out=logp_out[:, :], in_=lp_sb[:])

    nc.compile()
    return nc


def shard_inputs(inputs):
    """FULL numpy inputs -> list of 8 per-core input maps."""
    import ml_dtypes
    bf16 = ml_dtypes.bfloat16

    f = lambda k: np.ascontiguousarray(np.asarray(inputs[k], np.float32))
    idx = int(np.asarray(inputs["input"]).ravel()[0])
    emb = f("emb")
    hidden = f("hidden").reshape(H)
    attn_W, attn_b = f("attn_W"), f("attn_b")
    enc = f("encoder_outputs")
    comb_W, comb_b = f("comb_W"), f("comb_b")
    Wih, Whh = f("gru_Wih"), f("gru_Whh")
    bih, bhh = f("gru_bih"), f("gru_bhh")
    out_W, out_b = f("out_W"), f("out_b")

    owp = np.zeros((H, VP), np.float32)
    owp[:, :V] = out_W
    owp = owp.astype(bf16)
    obp = np.full((VP,), -1e30, np.float32)
    obp[:V] = out_b

    A_ih = np.ascontiguousarray(Wih.T.reshape(H, 3, H))
    A_hh = np.ascontiguousarray(Whh.T.reshape(H, 3, H))
    bi3 = bih.reshape(3, H)
    bh3 = bhh.reshape(3, H)

    embr = emb.reshape(V * KH, 128)
    idx8 = (idx * KH + np.arange(KH, dtype=np.int32)).reshape(KH, 1)
    hidden8 = np.ascontiguousarray(hidden.reshape(KH, 128))
    enc_bf = enc.astype(bf16)
    comb_bf = comb_W.astype(bf16)

    in_maps = []
    for c in range(NC):
        sl = slice(c * 128, (c + 1) * 128)
        ssl = slice(c * SS, (c + 1) * SS)
        vsl = slice(c * VS, (c + 1) * VS)
        in_maps.append({
            "idx8": idx8,
            "embr": embr,
            "hidden8": hidden8,
            "h0col": np.ascontiguousarray(hidden[sl, None]),
            "attn_w": np.ascontiguousarray(attn_W[:, ssl]),
            "attn_b": np.ascontiguousarray(attn_b[None, ssl]),
            "enc": np.ascontiguousarray(enc_bf[:, sl]),
            "comb_w": np.ascontiguousarray(comb_bf[:, sl]),
            "comb_b": np.ascontiguousarray(comb_b[None, sl]),
            "wih": np.ascontiguousarray(A_ih[:, :, sl]),
            "whh": np.ascontiguousarray(A_hh[:, :, sl]),
            "bi": np.ascontiguousarray(bi3[:, sl].T),
            "bh": np.ascontiguousarray(bh3[:, sl].T),
            "ow": np.ascontiguousarray(owp[:, vsl]),
            "ob": np.ascontiguousarray(obp[vsl].reshape(MCH, 128).T),
        })
    return in_maps


def unshard_outputs(results):
    """list of per-core {name: array} -> (log_probs, hidden, attn_weights)."""
    logp = np.concatenate(
        [np.asarray(r["logp_out"]).T.reshape(-1) for r in results])[:V][None, :]
    h = np.concatenate([np.asarray(r["h_out"]).ravel() for r in results])[None, None, :]
    aw = np.asarray(results[0]["attn_out"]).T.reshape(-1)[None, :]
    return (np.ascontiguousarray(logp, np.float32),
            np.ascontiguousarray(h, np.float32),
            np.ascontiguousarray(aw, np.float32))


def _get_program():
    if "nc" not in _CACHE:
        _CACHE["nc"] = build_program()
    return _CACHE["nc"]


def kernel(**inputs):
    nc = _get_program()
    in_maps = shard_inputs(inputs)
    res = run_bass_kernel_spmd(nc, in_maps, core_ids=list(range(NC)), trace=False)
    return unshard_outputs(res.results)


# revision 14
# speedup vs baseline: 1.8216x; 1.0457x over previous
"""AttnDecoderRNN step on 8 Trainium2 NeuronCores (Bass/Tile, tensor-parallel).

Sharding (8 cores, stitched with AllGather where a full vector is needed):
  - embedding: table replicated (viewed [V*8, 128]); each core indirect-DMAs
    the token row as 8 chunk-rows (4KB HBM read) -- no collective.
  - attn scores: attn_W sharded over S columns (512/core); AG1 -> softmax
    computed partition-major on every core.
  - attn_applied: encoder_outputs sharded over H columns (128/core); AG2.
  - comb: comb_W sharded over H columns; AG3.
  - GRU (x2, shared weights): gate outputs sharded over H (128/core for each
    of the 3 gates); h re-gathered with AG4/AG5 for the next contraction.
  - logits: out_W sharded over vocab columns (6400/core after padding V to
    51200); log_softmax denominator from an AG of per-core exp-sums (AG6);
    each core writes its own log-prob shard, the host concatenates.

Precision: f32 everywhere except out_W / encoder_outputs / comb_W (and the
activation vectors feeding those three matmuls), which run in bf16.
Verified end-to-end error vs the f32 reference: log_probs ~2e-4, hidden and
attn_weights ~2e-5 (max relative).

Queue plan: all weight streaming on the sync (SP) DMA queue in consumption
order; collective bounce-buffer traffic and gather loads on the gpsimd queue
so they interleave with the collectives; compute engines stay DMA-free.
"""

import numpy as np

import concourse.bacc as bacc
import concourse.bass as bass
import concourse.tile as tile
from concourse import mybir
from concourse.bass import IndirectOffsetOnAxis
from concourse.bass_utils import run_bass_kernel_spmd
from concourse.masks import make_identity

H = 1024
S = 4096
V = 50257
L = 2
NC = 8
VP = 51200          # padded vocab
VS = VP // NC       # 6400 vocab cols per core
MCH = VS // 128     # 50 m-chunks per core
SS = S // NC        # 512 attn cols per core
KH = H // 128       # 8 k-chunks for H contraction
K2H = 2 * H // 128  # 16 k-chunks for 2H contraction
KS = S // 128       # 32 k-chunks for S contraction

F32 = mybir.dt.float32
BF16 = mybir.dt.bfloat16

_CACHE = {}


def _gru_layer(nc, sb, ps, x_sb, h_sb, h_col, wih_sb, whh_sb,
               bi_sb, bh_sb, bsum_sb):
    """One GRU cell step; returns h_new [128,1] (this core's slice)."""
    AF = mybir.ActivationFunctionType
    gi_ps = ps.tile([128, 3], F32, name="gi_ps", tag="scr")
    gh_ps = ps.tile([128, 3], F32, name="gh_ps", tag="scr")
    for g in range(3):
        for k in range(KH):
            nc.tensor.matmul(gi_ps[:, g:g + 1], lhsT=wih_sb[:, k, g, :],
                             rhs=x_sb[:, k:k + 1],
                             start=(k == 0), stop=(k == KH - 1))
    for g in range(3):
        for k in range(KH):
            nc.tensor.matmul(gh_ps[:, g:g + 1], lhsT=whh_sb[:, k, g, :],
                             rhs=h_sb[:, k:k + 1],
                             start=(k == 0), stop=(k == KH - 1))
    gh_sb = sb.tile([128, 3], F32, name="gh_sb")
    nc.vector.tensor_copy(gh_sb[:], gh_ps[:])
    rz_sb = sb.tile([128, 2], F32, name="rz_sb")
    nc.vector.tensor_add(rz_sb[:], gi_ps[:, 0:2], gh_sb[:, 0:2])
    # sigmoid via tanh to stay on the Exp/Tanh activation table:
    # sigmoid(a) = 0.5 + 0.5*tanh(a/2)
    v = sb.tile([128, 1], F32, name="gru_v")  # tanh(r-gate/2)
    u = sb.tile([128, 1], F32, name="gru_u")  # tanh(z-gate/2)
    nc.scalar.activation(v[:], rz_sb[:, 0:1], AF.Tanh,
                         bias=bsum_sb[:, 0:1], scale=0.5)
    nc.scalar.activation(u[:], rz_sb[:, 1:2], AF.Tanh,
                         bias=bsum_sb[:, 1:2], scale=0.5)
    hnb = sb.tile([128, 1], F32, name="gru_hnb")
    nc.vector.tensor_scalar_add(hnb[:], gh_sb[:, 2:3], bh_sb[:, 2:3])
    # rn = sigmoid(r)*hnb = 0.5*(hnb + tanh(r/2)*hnb)
    vh = sb.tile([128, 1], F32, name="gru_vh")
    nc.vector.tensor_mul(vh[:], v[:], hnb[:])
    rh = sb.tile([128, 1], F32, name="gru_rh")
    nc.vector.tensor_add(rh[:], hnb[:], vh[:])
    tin = sb.tile([128, 1], F32, name="gru_tin")
    nc.vector.tensor_scalar(tin[:], rh[:], 0.5, None, op0=mybir.AluOpType.mult)
    nc.vector.tensor_add(tin[:], gi_ps[:, 2:3], tin[:])
    n = sb.tile([128, 1], F32, name="gru_n")
    nc.scalar.activation(n[:], tin[:], AF.Tanh, bias=bi_sb[:, 2:3])
    # h_new = n + sigmoid(z)*(h-n) = n + 0.5*((h-n) + tanh(z/2)*(h-n))
    d = sb.tile([128, 1], F32, name="gru_d")
    nc.vector.tensor_sub(d[:], h_col[:], n[:])
    ud = sb.tile([128, 1], F32, name="gru_ud")
    nc.vector.tensor_mul(ud[:], u[:], d[:])
    e = sb.tile([128, 1], F32, name="gru_e")
    nc.vector.tensor_add(e[:], d[:], ud[:])
    nc.vector.tensor_scalar(e[:], e[:], 0.5, None, op0=mybir.AluOpType.mult)
    h_new = sb.tile([128, 1], F32, name="gru_hnew")
    nc.vector.tensor_add(h_new[:], n[:], e[:])
    return h_new


def build_program():
    nc = bacc.Bacc("TRN2", target_bir_lowering=False, debug=False, num_devices=NC)
    AF = mybir.ActivationFunctionType

    # ---- kernel I/O (per core) ----
    idx8 = nc.dram_tensor("idx8", [KH, 1], mybir.dt.int32, kind="ExternalInput")
    embr = nc.dram_tensor("embr", [V * KH, 128], F32, kind="ExternalInput")
    hidden8 = nc.dram_tensor("hidden8", [KH, 128], F32, kind="ExternalInput")
    h0col = nc.dram_tensor("h0col", [128, 1], F32, kind="ExternalInput")
    attn_w = nc.dram_tensor("attn_w", [2 * H, SS], F32, kind="ExternalInput")
    attn_b = nc.dram_tensor("attn_b", [4, 128], F32, kind="ExternalInput")
    enc = nc.dram_tensor("enc", [S, 128], BF16, kind="ExternalInput")
    comb_w = nc.dram_tensor("comb_w", [2 * H, 128], BF16, kind="ExternalInput")
    comb_b = nc.dram_tensor("comb_b", [1, 128], F32, kind="ExternalInput")
    wih = nc.dram_tensor("wih", [H, 3, 128], F32, kind="ExternalInput")
    whh = nc.dram_tensor("whh", [H, 3, 128], F32, kind="ExternalInput")
    bi = nc.dram_tensor("bi", [128, 3], F32, kind="ExternalInput")
    bh = nc.dram_tensor("bh", [128, 3], F32, kind="ExternalInput")
    ow = nc.dram_tensor("ow", [H, VS], BF16, kind="ExternalInput")
    ob = nc.dram_tensor("ob", [128, MCH], F32, kind="ExternalInput")

    attn_out = nc.dram_tensor("attn_out", [128, KS], F32, kind="ExternalOutput")
    h_out = nc.dram_tensor("h_out", [128, 1], F32, kind="ExternalOutput")
    logp_out = nc.dram_tensor("logp_out", [128, MCH], F32, kind="ExternalOutput")

    rg = [list(range(NC))]

    with tile.TileContext(nc) as tc:
        with (
            tc.tile_pool(name="w", bufs=1) as wp,
            tc.tile_pool(name="sb", bufs=2) as sb,
            tc.tile_pool(name="owp", bufs=KH) as owp,
            tc.tile_pool(name="ps", bufs=4, space="PSUM") as ps,
            tc.tile_pool(name="lgps", bufs=2, space="PSUM") as lgps,
            tc.tile_pool(name="dram", bufs=1, space="DRAM") as dram,
        ):
            # ---- tiny chain-critical loads first (SP queue) ----
            idx_sb = sb.tile([KH, 1], mybir.dt.int32, name="idx_sb", bufs=1)
            nc.sync.dma_start(out=idx_sb[:], in_=idx8[:, :])
            h08 = sb.tile([KH, 128], F32, name="h08", bufs=1)
            nc.sync.dma_start(out=h08[:], in_=hidden8[:, :])

            # ---- weight streams (SP queue, consumption order) ----
            # attn_W chunked so each k-chunk matmul starts as soon as its
            # 256KB slice lands (pipeline DMA with the 16 PE matmuls).
            attn_sb = wp.tile([128, K2H, SS], F32)
            for k in range(K2H):
                nc.sync.dma_start(out=attn_sb[:, k, :],
                                  in_=attn_w[k * 128:(k + 1) * 128, :])
            ab_sb = wp.tile([4, 128], F32)
            nc.sync.dma_start(out=ab_sb[:], in_=attn_b[:, :])
            h0_sb = wp.tile([128, 1], F32)
            nc.sync.dma_start(out=h0_sb[:], in_=h0col[:, :])
            bi_sb = wp.tile([128, 3], F32)
            nc.sync.dma_start(out=bi_sb[:], in_=bi[:, :])
            bh_sb = wp.tile([128, 3], F32)
            nc.sync.dma_start(out=bh_sb[:], in_=bh[:, :])
            cbias_sb = wp.tile([1, 128], F32)
            nc.sync.dma_start(out=cbias_sb[:], in_=comb_b[:, :])
            ob_sb = wp.tile([128, MCH], F32)
            nc.sync.dma_start(out=ob_sb[:], in_=ob[:, :])
            wih_sb = wp.tile([128, KH, 3, 128], F32)
            nc.sync.dma_start(out=wih_sb[:],
                              in_=wih[:, :, :].rearrange("(k p) g n -> p k g n", p=128))
            whh_sb = wp.tile([128, KH, 3, 128], F32)
            nc.sync.dma_start(out=whh_sb[:],
                              in_=whh[:, :, :].rearrange("(k p) g n -> p k g n", p=128))
            enc_sb = wp.tile([128, KS, 128], BF16)
            nc.sync.dma_start(out=enc_sb[:],
                              in_=enc[:, :].rearrange("(k p) n -> p k n", p=128))
            comb_sb = wp.tile([128, K2H, 128], BF16)
            nc.sync.dma_start(out=comb_sb[:],
                              in_=comb_w[:, :].rearrange("(k p) n -> p k n", p=128))
            ow_tiles = []
            for k in range(KH):
                ow_k = owp.tile([128, MCH, 128], BF16, name="ow_k")
                nc.sync.dma_start(
                    out=ow_k[:],
                    in_=ow[k * 128:(k + 1) * 128, :].rearrange("p (m n) -> p m n", n=128))
                ow_tiles.append(ow_k)

            # ---- constants ----
            ident = wp.tile([128, 128], F32)
            make_identity(nc, ident[:])
            ones_p = wp.tile([128, 1], F32)
            nc.vector.memset(ones_p[:], 1.0)
            ones_1 = wp.tile([1, 128], F32)
            nc.vector.memset(ones_1[:], 1.0)
            # prewarm the Exp/Tanh activation table off the critical path
            warm = sb.tile([1, 1], F32, name="warm", bufs=1)
            nc.vector.memset(warm[:], 0.0)
            nc.scalar.activation(warm[:], warm[:], AF.Exp)
            # keep the PE busy from t=0 so the clock is ramped before the
            # first real matmuls
            pe_warm = ps.tile([128, 128], F32, name="pe_warm", tag="scr")
            for _ in range(10):
                nc.tensor.transpose(pe_warm[:], ident[:], ident[:])
            # halved gate bias: tanh(0.5*a + 0.5*(bi+bh)) for the sigmoid trick
            bsum_sb = wp.tile([128, 2], F32)
            nc.vector.tensor_add(bsum_sb[:], bi_sb[:, 0:2], bh_sb[:, 0:2])
            nc.vector.tensor_scalar(bsum_sb[:], bsum_sb[:], 0.5, None,
                                    op0=mybir.AluOpType.mult)

            # ---- embedding row gather (gpsimd) + partition-major transposes --
            er8 = sb.tile([KH, 128], F32, name="er8", bufs=1)
            nc.gpsimd.indirect_dma_start(
                out=er8[:], out_offset=None,
                in_=embr[:, :],
                in_offset=IndirectOffsetOnAxis(ap=idx_sb[:, :1], axis=0))
            tp_e = ps.tile([128, KH], F32, name="tp_e", tag="scr")
            nc.tensor.transpose(tp_e[:], er8[:], ident[:KH, :KH])
            emb_pm = sb.tile([128, KH], F32, name="emb_pm", bufs=1)
            nc.vector.tensor_copy(emb_pm[:], tp_e[:])
            embc_bf = sb.tile([128, KH], BF16, name="embc_bf", bufs=1)
            nc.vector.tensor_copy(embc_bf[:], tp_e[:])
            tp_h0 = ps.tile([128, KH], F32, name="tp_h0", tag="scr")
            nc.tensor.transpose(tp_h0[:], h08[:], ident[:KH, :KH])
            hid_sb = sb.tile([128, KH], F32, name="hid_sb", bufs=1)
            nc.vector.tensor_copy(hid_sb[:], tp_h0[:])

            # ---- attn scores (this core's 512 cols of S) + AG1 ----
            # M-orientation: attn_W chunk is the stationary operand, the
            # activation column streams (N=1); scores land partition-major
            # [128, 4] and are transposed back to linear for the AllGather.
            # Interleaved per-column psum groups are element-disjoint (safe);
            # the group checker only tracks regions, hence skip_group_check.
            t1_ps = ps.tile([128, 4], F32, name="t1_ps", tag="scr")
            for k in range(K2H):
                rhsv = emb_pm[:, k:k + 1] if k < KH else hid_sb[:, k - KH:k - KH + 1]
                for m in range(4):
                    nc.tensor.matmul(t1_ps[:, m:m + 1],
                                     lhsT=attn_sb[:, k, m * 128:(m + 1) * 128],
                                     rhs=rhsv,
                                     start=(k == 0), stop=(k == K2H - 1),
                                     skip_group_check=True)
            t1cp = sb.tile([128, 4], F32, name="t1cp")
            nc.vector.tensor_copy(t1cp[:], t1_ps[:])
            t1t_ps = ps.tile([4, 128], F32, name="t1t_ps", tag="scr")
            nc.tensor.transpose(t1t_ps[:], t1cp[:], ident[:])
            t1_sb = sb.tile([4, 128], F32, name="t1_sb")
            nc.vector.tensor_add(t1_sb[:], t1t_ps[:], ab_sb[:])
            ag1_in = dram.tile([4, 128], F32)
            ag1_out = dram.tile([KS, 128], F32)
            nc.gpsimd.dma_start(out=ag1_in[:], in_=t1_sb[:])
            nc.gpsimd.collective_compute(
                "AllGather", mybir.AluOpType.bypass, replica_groups=rg,
                ins=[ag1_in.opt()], outs=[ag1_out.opt()])

            # ---- softmax over full S (partition-major, no max-sub needed) ----
            t1_32 = sb.tile([KS, 128], F32, name="t1_32")
            nc.gpsimd.dma_start(out=t1_32[:], in_=ag1_out[:])
            tp_t1 = ps.tile([128, KS], F32, name="tp_t1", tag="scr")
            nc.tensor.transpose(tp_t1[:], t1_32[:], ident[:KS, :KS])
            u_sb = sb.tile([128, KS], F32, name="u_sb")
            srow = sb.tile([128, 1], F32, name="srow")
            nc.scalar.activation(u_sb[:], tp_t1[:], AF.Exp, accum_out=srow[:])
            u_bf = sb.tile([128, KS], BF16, name="u_bf")
            nc.vector.tensor_copy(u_bf[:], u_sb[:])

            # ---- attn_applied with unnormalized weights; scaled after the
            # matmul so the reciprocal chain overlaps the PE work ----
            aa_ps = ps.tile([1, 128], F32, name="aa_ps", tag="scr")
            for k in range(KS):
                nc.tensor.matmul(aa_ps[:], lhsT=u_bf[:, k:k + 1],
                                 rhs=enc_sb[:, k, :],
                                 start=(k == 0), stop=(k == KS - 1))
            s1_ps = ps.tile([1, 1], F32, name="s1_ps", tag="scr")
            nc.tensor.matmul(s1_ps[:], lhsT=srow[:], rhs=ones_p[:],
                             start=True, stop=True)
            rs_sb = sb.tile([1, 1], F32, name="rs_sb")
            nc.vector.reciprocal(rs_sb[:], s1_ps[:])
            rb_ps = ps.tile([128, 1], F32, name="rb_ps", tag="scr")
            nc.tensor.matmul(rb_ps[:], lhsT=ones_1[:], rhs=rs_sb[:],
                             start=True, stop=True)
            rs_bc = sb.tile([128, 1], F32, name="rs_bc")
            nc.vector.tensor_copy(rs_bc[:], rb_ps[:])
            aw_sb = sb.tile([128, KS], F32, name="aw_sb")
            nc.vector.tensor_scalar_mul(aw_sb[:], u_sb[:], rs_bc[:])
            nc.gpsimd.dma_start(out=attn_out[:, :], in_=aw_sb[:])
            aa_sb = sb.tile([1, 128], F32, name="aa_sb")
            nc.vector.tensor_scalar_mul(aa_sb[:], aa_ps[:], rs_sb[:])
            ag2_in = dram.tile([1, 128], F32)
            ag2_out = dram.tile([NC, 128], F32)
            nc.gpsimd.dma_start(out=ag2_in[:], in_=aa_sb[:])
            nc.gpsimd.collective_compute(
                "AllGather", mybir.AluOpType.bypass, replica_groups=rg,
                ins=[ag2_in.opt()], outs=[ag2_out.opt()])

            # ---- comb + AG3 ----
            aa8 = sb.tile([KH, 128], F32, name="aa8")
            nc.gpsimd.dma_start(out=aa8[:], in_=ag2_out[:])
            tp_aa = ps.tile([128, KH], F32, name="tp_aa", tag="scr")
            nc.tensor.transpose(tp_aa[:], aa8[:], ident[:KH, :KH])
            aa_bf = sb.tile([128, KH], BF16, name="aa_bf")
            nc.vector.tensor_copy(aa_bf[:], tp_aa[:])
            cb_ps = ps.tile([1, 128], F32, name="cb_ps", tag="scr")
            for k in range(K2H):
                lhs = embc_bf[:, k:k + 1] if k < KH else aa_bf[:, k - KH:k - KH + 1]
                nc.tensor.matmul(cb_ps[:], lhsT=lhs, rhs=comb_sb[:, k, :],
                                 start=(k == 0), stop=(k == K2H - 1))
            cbo_sb = sb.tile([1, 128], F32, name="cbo_sb")
            nc.vector.tensor_add(cbo_sb[:], cb_ps[:], cbias_sb[:])
            ag3_in = dram.tile([1, 128], F32)
            ag3_out = dram.tile([NC, 128], F32)
            nc.gpsimd.dma_start(out=ag3_in[:], in_=cbo_sb[:])
            nc.gpsimd.collective_compute(
                "AllGather", mybir.AluOpType.bypass, replica_groups=rg,
                ins=[ag3_in.opt()], outs=[ag3_out.opt()])

            # ---- GRU layer 1 ----
            x18 = sb.tile([KH, 128], F32, name="x18")
            nc.gpsimd.dma_start(out=x18[:], in_=ag3_out[:])
            tp_x1 = ps.tile([128, KH], F32, name="tp_x1", tag="scr")
            nc.tensor.transpose(tp_x1[:], x18[:], ident[:KH, :KH])
            x1_sb = sb.tile([128, KH], F32, name="x1_sb")
            nc.scalar.activation(x1_sb[:], tp_x1[:], AF.Relu)
            h1_col = _gru_layer(nc, sb, ps, x1_sb, hid_sb, h0_sb,
                                wih_sb, whh_sb, bi_sb, bh_sb, bsum_sb)
            ag4_in = dram.tile([128, 1], F32)
            ag4_out = dram.tile([NC * 128, 1], F32)
            nc.gpsimd.dma_start(out=ag4_in[:], in_=h1_col[:])
            nc.gpsimd.collective_compute(
                "AllGather", mybir.AluOpType.bypass, replica_groups=rg,
                ins=[ag4_in.opt()], outs=[ag4_out.opt()])

            # ---- GRU layer 2 ----
            h18 = sb.tile([KH, 128], F32, name="h18")
            nc.gpsimd.dma_start(out=h18[:],
                                in_=ag4_out[:].rearrange("(a b) o -> a (b o)", b=128))
            tp_h1 = ps.tile([128, KH], F32, name="tp_h1", tag="scr")
            nc.tensor.transpose(tp_h1[:], h18[:], ident[:KH, :KH])
            x2_sb = sb.tile([128, KH], F32, name="x2_sb")
            nc.scalar.activation(x2_sb[:], tp_h1[:], AF.Relu)
            h1f_sb = sb.tile([128, KH], F32, name="h1f_sb")
            nc.vector.tensor_copy(h1f_sb[:], tp_h1[:])
            h2_col = _gru_layer(nc, sb, ps, x2_sb, h1f_sb, h1_col,
                                wih_sb, whh_sb, bi_sb, bh_sb, bsum_sb)
            nc.gpsimd.dma_start(out=h_out[:, :], in_=h2_col[:])
            ag5_in = dram.tile([128, 1], F32)
            ag5_out = dram.tile([NC * 128, 1], F32)
            nc.gpsimd.dma_start(out=ag5_in[:], in_=h2_col[:])
            nc.gpsimd.collective_compute(
                "AllGather", mybir.AluOpType.bypass, replica_groups=rg,
                ins=[ag5_in.opt()], outs=[ag5_out.opt()])

            # ---- logits over this core's 6400 padded vocab cols ----
            h28 = sb.tile([KH, 128], F32, name="h28")
            nc.gpsimd.dma_start(out=h28[:],
                                in_=ag5_out[:].rearrange("(a b) o -> a (b o)", b=128))
            tp_h2 = ps.tile([128, KH], F32, name="tp_h2", tag="scr")
            nc.tensor.transpose(tp_h2[:], h28[:], ident[:KH, :KH])
            h2w_sb = sb.tile([128, KH], BF16, name="h2w_sb")
            nc.vector.tensor_copy(h2w_sb[:], tp_h2[:])

            lg_ps = lgps.tile([128, MCH], F32, name="lg_ps", tag="lg")
            for k in range(KH):
                for m in range(MCH):
                    nc.tensor.matmul(lg_ps[:, m:m + 1], lhsT=ow_tiles[k][:, m, :],
                                     rhs=h2w_sb[:, k:k + 1],
                                     start=(k == 0), stop=(k == KH - 1),
                                     skip_group_check=True)
            lg_sb = sb.tile([128, MCH], F32, name="lg_sb", bufs=1)
            nc.vector.tensor_add(lg_sb[:], lg_ps[:], ob_sb[:])

            # ---- log-softmax denominator via AG6 ----
            elg = sb.tile([128, MCH], F32, name="elg")
            srl = sb.tile([128, 1], F32, name="srl")
            nc.scalar.activation(elg[:], lg_sb[:], AF.Exp, accum_out=srl[:])
            st_ps = ps.tile([1, 1], F32, name="st_ps", tag="scr")
            nc.tensor.matmul(st_ps[:], lhsT=srl[:], rhs=ones_p[:],
                             start=True, stop=True)
            sc8 = sb.tile([1, 8], F32, name="sc8")
            nc.vector.memset(sc8[:], 0.0)
            nc.vector.tensor_copy(sc8[:, 0:1], st_ps[:])
            ag6_in = dram.tile([1, 8], F32)
            ag6_out = dram.tile([NC, 8], F32)
            nc.gpsimd.dma_start(out=ag6_in[:], in_=sc8[:])
            nc.gpsimd.collective_compute(
                "AllGather", mybir.AluOpType.bypass, replica_groups=rg,
                ins=[ag6_in.opt()], outs=[ag6_out.opt()])
            sg = sb.tile([1, NC, 8], F32, name="sg")
            nc.gpsimd.dma_start(out=sg[:],
                                in_=ag6_out[:].rearrange("a b -> (a b)")[None, :]
                                .rearrange("o (a b) -> o a b", b=8))
            tot = sb.tile([1, 1], F32, name="tot")
            nc.vector.tensor_reduce(tot[:], sg[:, :, 0:1],
                                    axis=mybir.AxisListType.XY, op=mybir.AluOpType.add)
            lse = sb.tile([1, 1], F32, name="lse")
            nc.scalar.activation(lse[:], tot[:], AF.Ln)
            lb_ps = ps.tile([128, 1], F32, name="lb_ps", tag="scr")
            nc.tensor.matmul(lb_ps[:], lhsT=ones_1[:], rhs=lse[:],
                             start=True, stop=True)
            lse_bc = sb.tile([128, 1], F32, name="lse_bc")
            nc.vector.tensor_copy(lse_bc[:], lb_ps[:])
            lp_sb = sb.tile([128, MCH], F32, name="lp_sb")
            nc.vector.tensor_scalar(lp_sb[:], lg_sb[:], lse_bc[:], None,
                                    op0=mybir.AluOpType.subtract)
            nc.sync.dma_start(out=logp_out[:, :], in_=lp_sb[:])

    nc.compile()
    return nc


def shard_inputs(inputs):
    """FULL numpy inputs -> list of 8 per-core input maps."""
    import ml_dtypes
    bf16 = ml_dtypes.bfloat16

    f = lambda k: np.ascontiguousarray(np.asarray(inputs[k], np.float32))
    idx = int(np.asarray(inputs["input"]).ravel()[0])
    emb = f("emb")
    hidden = f("hidden").reshape(H)
    attn_W, attn_b = f("attn_W"), f("attn_b")
    enc = f("encoder_outputs")
    comb_W, comb_b = f("comb_W"), f("comb_b")
    Wih, Whh = f("gru_Wih"), f("gru_Whh")
    bih, bhh = f("gru_bih"), f("gru_bhh")
    out_W, out_b = f("out_W"), f("out_b")

    owp = np.zeros((H, VP), np.float32)
    owp[:, :V] = out_W
    owp = owp.astype(bf16)
    obp = np.full((VP,), -1e30, np.float32)
    obp[:V] = out_b

    A_ih = np.ascontiguousarray(Wih.T.reshape(H, 3, H))
    A_hh = np.ascontiguousarray(Whh.T.reshape(H, 3, H))
    bi3 = bih.reshape(3, H)
    bh3 = bhh.reshape(3, H)

    embr = emb.reshape(V * KH, 128)
    idx8 = (idx * KH + np.arange(KH, dtype=np.int32)).reshape(KH, 1)
    hidden8 = np.ascontiguousarray(hidden.reshape(KH, 128))
    enc_bf = enc.astype(bf16)
    comb_bf = comb_W.astype(bf16)

    in_maps = []
    for c in range(NC):
        sl = slice(c * 128, (c + 1) * 128)
        ssl = slice(c * SS, (c + 1) * SS)
        vsl = slice(c * VS, (c + 1) * VS)
        in_maps.append({
            "idx8": idx8,
            "embr": embr,
            "hidden8": hidden8,
            "h0col": np.ascontiguousarray(hidden[sl, None]),
            "attn_w": np.ascontiguousarray(attn_W[:, ssl]),
            "attn_b": np.ascontiguousarray(attn_b[ssl].reshape(4, 128)),
            "enc": np.ascontiguousarray(enc_bf[:, sl]),
            "comb_w": np.ascontiguousarray(comb_bf[:, sl]),
            "comb_b": np.ascontiguousarray(comb_b[None, sl]),
            "wih": np.ascontiguousarray(A_ih[:, :, sl]),
            "whh": np.ascontiguousarray(A_hh[:, :, sl]),
            "bi": np.ascontiguousarray(bi3[:, sl].T),
            "bh": np.ascontiguousarray(bh3[:, sl].T),
            "ow": np.ascontiguousarray(owp[:, vsl]),
            "ob": np.ascontiguousarray(obp[vsl].reshape(MCH, 128).T),
        })
    return in_maps


def unshard_outputs(results):
    """list of per-core {name: array} -> (log_probs, hidden, attn_weights)."""
    logp = np.concatenate(
        [np.asarray(r["logp_out"]).T.reshape(-1) for r in results])[:V][None, :]
    h = np.concatenate([np.asarray(r["h_out"]).ravel() for r in results])[None, None, :]
    aw = np.asarray(results[0]["attn_out"]).T.reshape(-1)[None, :]
    return (np.ascontiguousarray(logp, np.float32),
            np.ascontiguousarray(h, np.float32),
            np.ascontiguousarray(aw, np.float32))


def _get_program():
    if "nc" not in _CACHE:
        _CACHE["nc"] = build_program()
    return _CACHE["nc"]


def kernel(**inputs):
    nc = _get_program()
    in_maps = shard_inputs(inputs)
    res = run_bass_kernel_spmd(nc, in_maps, core_ids=list(range(NC)), trace=False)
    return unshard_outputs(res.results)


# revision 15
# speedup vs baseline: 1.8233x; 1.0009x over previous
"""AttnDecoderRNN step on 8 Trainium2 NeuronCores (Bass/Tile, tensor-parallel).

Sharding (8 cores, stitched with AllGather where a full vector is needed):
  - embedding: table replicated (viewed [V*8, 128]); each core indirect-DMAs
    the token row as 8 chunk-rows (4KB HBM read) -- no collective.
  - attn scores: attn_W sharded over S columns (512/core); AG1 -> softmax
    computed partition-major on every core.
  - attn_applied: encoder_outputs sharded over H columns (128/core); AG2.
  - comb: comb_W sharded over H columns; AG3.
  - GRU (x2, shared weights): gate outputs sharded over H (128/core for each
    of the 3 gates); h re-gathered with AG4/AG5 for the next contraction.
  - logits: out_W sharded over vocab columns (6400/core after padding V to
    51200); log_softmax denominator from an AG of per-core exp-sums (AG6);
    each core writes its own log-prob shard, the host concatenates.

Precision: f32 everywhere except out_W / encoder_outputs / comb_W (and the
activation vectors feeding those three matmuls), which run in bf16.
Verified end-to-end error vs the f32 reference: log_probs ~2e-4, hidden and
attn_weights ~2e-5 (max relative).

Queue plan: all weight streaming on the sync (SP) DMA queue in consumption
order; collective bounce-buffer traffic and gather loads on the gpsimd queue
so they interleave with the collectives; compute engines stay DMA-free.
"""

import numpy as np

import concourse.bacc as bacc
import concourse.bass as bass
import concourse.tile as tile
from concourse import mybir
from concourse.bass import IndirectOffsetOnAxis
from concourse.bass_utils import run_bass_kernel_spmd
from concourse.masks import make_identity

H = 1024
S = 4096
V = 50257
L = 2
NC = 8
VP = 51200          # padded vocab
VS = VP // NC       # 6400 vocab cols per core
MCH = VS // 128     # 50 m-chunks per core
SS = S // NC        # 512 attn cols per core
KH = H // 128       # 8 k-chunks for H contraction
K2H = 2 * H // 128  # 16 k-chunks for 2H contraction
KS = S // 128       # 32 k-chunks for S contraction

F32 = mybir.dt.float32
BF16 = mybir.dt.bfloat16

_CACHE = {}


def _gru_layer(nc, sb, ps, x_sb, h_sb, h_col, wih_sb, whh_sb,
               bi_sb, bh_sb, bsum_sb):
    """One GRU cell step; returns h_new [128,1] (this core's slice)."""
    AF = mybir.ActivationFunctionType
    gi_ps = ps.tile([128, 3], F32, name="gi_ps", tag="scr")
    gh_ps = ps.tile([128, 3], F32, name="gh_ps", tag="scr")
    for g in range(3):
        for k in range(KH):
            nc.tensor.matmul(gi_ps[:, g:g + 1], lhsT=wih_sb[:, k, g, :],
                             rhs=x_sb[:, k:k + 1],
                             start=(k == 0), stop=(k == KH - 1))
    for g in range(3):
        for k in range(KH):
            nc.tensor.matmul(gh_ps[:, g:g + 1], lhsT=whh_sb[:, k, g, :],
                             rhs=h_sb[:, k:k + 1],
                             start=(k == 0), stop=(k == KH - 1))
    gh_sb = sb.tile([128, 3], F32, name="gh_sb")
    nc.vector.tensor_copy(gh_sb[:], gh_ps[:])
    rz_sb = sb.tile([128, 2], F32, name="rz_sb")
    nc.vector.tensor_add(rz_sb[:], gi_ps[:, 0:2], gh_sb[:, 0:2])
    # sigmoid via tanh to stay on the Exp/Tanh activation table:
    # sigmoid(a) = 0.5 + 0.5*tanh(a/2)
    v = sb.tile([128, 1], F32, name="gru_v")  # tanh(r-gate/2)
    u = sb.tile([128, 1], F32, name="gru_u")  # tanh(z-gate/2)
    nc.scalar.activation(v[:], rz_sb[:, 0:1], AF.Tanh,
                         bias=bsum_sb[:, 0:1], scale=0.5)
    nc.scalar.activation(u[:], rz_sb[:, 1:2], AF.Tanh,
                         bias=bsum_sb[:, 1:2], scale=0.5)
    hnb = sb.tile([128, 1], F32, name="gru_hnb")
    nc.vector.tensor_scalar_add(hnb[:], gh_sb[:, 2:3], bh_sb[:, 2:3])
    # rn = sigmoid(r)*hnb = 0.5*(hnb + tanh(r/2)*hnb)
    vh = sb.tile([128, 1], F32, name="gru_vh")
    nc.vector.tensor_mul(vh[:], v[:], hnb[:])
    rh = sb.tile([128, 1], F32, name="gru_rh")
    nc.vector.tensor_add(rh[:], hnb[:], vh[:])
    tin = sb.tile([128, 1], F32, name="gru_tin")
    nc.vector.tensor_scalar(tin[:], rh[:], 0.5, None, op0=mybir.AluOpType.mult)
    nc.vector.tensor_add(tin[:], gi_ps[:, 2:3], tin[:])
    n = sb.tile([128, 1], F32, name="gru_n")
    nc.scalar.activation(n[:], tin[:], AF.Tanh, bias=bi_sb[:, 2:3])
    # h_new = n + sigmoid(z)*(h-n) = n + 0.5*((h-n) + tanh(z/2)*(h-n))
    d = sb.tile([128, 1], F32, name="gru_d")
    nc.vector.tensor_sub(d[:], h_col[:], n[:])
    ud = sb.tile([128, 1], F32, name="gru_ud")
    nc.vector.tensor_mul(ud[:], u[:], d[:])
    e = sb.tile([128, 1], F32, name="gru_e")
    nc.vector.tensor_add(e[:], d[:], ud[:])
    nc.vector.tensor_scalar(e[:], e[:], 0.5, None, op0=mybir.AluOpType.mult)
    h_new = sb.tile([128, 1], F32, name="gru_hnew")
    nc.vector.tensor_add(h_new[:], n[:], e[:])
    return h_new


def build_program():
    nc = bacc.Bacc("TRN2", target_bir_lowering=False, debug=False, num_devices=NC)
    AF = mybir.ActivationFunctionType

    # ---- kernel I/O (per core) ----
    idx8 = nc.dram_tensor("idx8", [KH, 1], mybir.dt.int32, kind="ExternalInput")
    embr = nc.dram_tensor("embr", [V * KH, 128], F32, kind="ExternalInput")
    hidden8 = nc.dram_tensor("hidden8", [KH, 128], F32, kind="ExternalInput")
    h0col = nc.dram_tensor("h0col", [128, 1], F32, kind="ExternalInput")
    attn_w = nc.dram_tensor("attn_w", [2 * H, SS], F32, kind="ExternalInput")
    attn_b = nc.dram_tensor("attn_b", [4, 128], F32, kind="ExternalInput")
    enc = nc.dram_tensor("enc", [S, 128], BF16, kind="ExternalInput")
    comb_w = nc.dram_tensor("comb_w", [2 * H, 128], BF16, kind="ExternalInput")
    comb_b = nc.dram_tensor("comb_b", [1, 128], F32, kind="ExternalInput")
    wih = nc.dram_tensor("wih", [H, 3, 128], F32, kind="ExternalInput")
    whh = nc.dram_tensor("whh", [H, 3, 128], F32, kind="ExternalInput")
    bi = nc.dram_tensor("bi", [128, 3], F32, kind="ExternalInput")
    bh = nc.dram_tensor("bh", [128, 3], F32, kind="ExternalInput")
    ow = nc.dram_tensor("ow", [H, VS], BF16, kind="ExternalInput")
    ob = nc.dram_tensor("ob", [128, MCH], F32, kind="ExternalInput")

    attn_out = nc.dram_tensor("attn_out", [128, KS], F32, kind="ExternalOutput")
    h_out = nc.dram_tensor("h_out", [128, 1], F32, kind="ExternalOutput")
    logp_out = nc.dram_tensor("logp_out", [128, MCH], F32, kind="ExternalOutput")

    rg = [list(range(NC))]

    with tile.TileContext(nc) as tc:
        with (
            tc.tile_pool(name="w", bufs=1) as wp,
            tc.tile_pool(name="sb", bufs=2) as sb,
            tc.tile_pool(name="owp", bufs=KH) as owp,
            tc.tile_pool(name="ps", bufs=4, space="PSUM") as ps,
            tc.tile_pool(name="lgps", bufs=2, space="PSUM") as lgps,
            tc.tile_pool(name="dram", bufs=1, space="DRAM") as dram,
        ):
            # ---- tiny chain-critical loads first (SP queue) ----
            idx_sb = sb.tile([KH, 1], mybir.dt.int32, name="idx_sb", bufs=1)
            nc.sync.dma_start(out=idx_sb[:], in_=idx8[:, :])
            h08 = sb.tile([KH, 128], F32, name="h08", bufs=1)
            nc.sync.dma_start(out=h08[:], in_=hidden8[:, :])

            # ---- weight streams (SP queue, consumption order) ----
            # attn_W chunked so each k-chunk matmul starts as soon as its
            # 256KB slice lands (pipeline DMA with the 16 PE matmuls).
            attn_sb = wp.tile([128, K2H, SS], F32)
            for k in range(K2H):
                nc.sync.dma_start(out=attn_sb[:, k, :],
                                  in_=attn_w[k * 128:(k + 1) * 128, :])
            ab_sb = wp.tile([4, 128], F32)
            nc.sync.dma_start(out=ab_sb[:], in_=attn_b[:, :])
            h0_sb = wp.tile([128, 1], F32)
            nc.sync.dma_start(out=h0_sb[:], in_=h0col[:, :])
            bi_sb = wp.tile([128, 3], F32)
            nc.sync.dma_start(out=bi_sb[:], in_=bi[:, :])
            bh_sb = wp.tile([128, 3], F32)
            nc.sync.dma_start(out=bh_sb[:], in_=bh[:, :])
            cbias_sb = wp.tile([1, 128], F32)
            nc.sync.dma_start(out=cbias_sb[:], in_=comb_b[:, :])
            ob_sb = wp.tile([128, MCH], F32)
            nc.sync.dma_start(out=ob_sb[:], in_=ob[:, :])
            wih_sb = wp.tile([128, KH, 3, 128], F32)
            nc.sync.dma_start(out=wih_sb[:],
                              in_=wih[:, :, :].rearrange("(k p) g n -> p k g n", p=128))
            whh_sb = wp.tile([128, KH, 3, 128], F32)
            nc.sync.dma_start(out=whh_sb[:],
                              in_=whh[:, :, :].rearrange("(k p) g n -> p k g n", p=128))
            enc_sb = wp.tile([128, KS, 128], BF16)
            nc.sync.dma_start(out=enc_sb[:],
                              in_=enc[:, :].rearrange("(k p) n -> p k n", p=128))
            comb_sb = wp.tile([128, K2H, 128], BF16)
            nc.sync.dma_start(out=comb_sb[:],
                              in_=comb_w[:, :].rearrange("(k p) n -> p k n", p=128))
            ow_tiles = []
            for k in range(KH):
                ow_k = owp.tile([128, MCH, 128], BF16, name="ow_k")
                nc.sync.dma_start(
                    out=ow_k[:],
                    in_=ow[k * 128:(k + 1) * 128, :].rearrange("p (m n) -> p m n", n=128))
                ow_tiles.append(ow_k)

            # ---- constants ----
            ident = wp.tile([128, 128], F32)
            make_identity(nc, ident[:])
            ones_p = wp.tile([128, 1], F32)
            nc.vector.memset(ones_p[:], 1.0)
            ones_1 = wp.tile([1, 128], F32)
            nc.vector.memset(ones_1[:], 1.0)
            # prewarm the Exp/Tanh activation table off the critical path
            warm = sb.tile([1, 1], F32, name="warm", bufs=1)
            nc.vector.memset(warm[:], 0.0)
            nc.scalar.activation(warm[:], warm[:], AF.Exp)
            # keep the PE busy from t=0 so the clock is ramped before the
            # first real matmuls
            pe_warm = ps.tile([128, 128], F32, name="pe_warm", tag="scr")
            for _ in range(10):
                nc.tensor.transpose(pe_warm[:], ident[:], ident[:])
            # halved gate bias: tanh(0.5*a + 0.5*(bi+bh)) for the sigmoid trick
            bsum_sb = wp.tile([128, 2], F32)
            nc.vector.tensor_add(bsum_sb[:], bi_sb[:, 0:2], bh_sb[:, 0:2])
            nc.vector.tensor_scalar(bsum_sb[:], bsum_sb[:], 0.5, None,
                                    op0=mybir.AluOpType.mult)

            # ---- embedding row gather (gpsimd) + partition-major transposes --
            er8 = sb.tile([KH, 128], F32, name="er8", bufs=1)
            nc.gpsimd.indirect_dma_start(
                out=er8[:], out_offset=None,
                in_=embr[:, :],
                in_offset=IndirectOffsetOnAxis(ap=idx_sb[:, :1], axis=0))
            tp_e = ps.tile([128, KH], F32, name="tp_e", tag="scr")
            nc.tensor.transpose(tp_e[:], er8[:], ident[:KH, :KH])
            emb_pm = sb.tile([128, KH], F32, name="emb_pm", bufs=1)
            nc.vector.tensor_copy(emb_pm[:], tp_e[:])
            embc_bf = sb.tile([128, KH], BF16, name="embc_bf", bufs=1)
            nc.vector.tensor_copy(embc_bf[:], tp_e[:])
            tp_h0 = ps.tile([128, KH], F32, name="tp_h0", tag="scr")
            nc.tensor.transpose(tp_h0[:], h08[:], ident[:KH, :KH])
            hid_sb = sb.tile([128, KH], F32, name="hid_sb", bufs=1)
            nc.vector.tensor_copy(hid_sb[:], tp_h0[:])

            # ---- attn scores (this core's 512 cols of S) + AG1 ----
            # M-orientation: attn_W chunk is the stationary operand, the
            # activation column streams (N=1); scores land partition-major
            # [128, 4] and are transposed back to linear for the AllGather.
            # Interleaved per-column psum groups are element-disjoint (safe);
            # the group checker only tracks regions, hence skip_group_check.
            t1_tiles = [ps.tile([128, 1], F32, name=f"t1_ps{m}", tag="scr")
                        for m in range(4)]
            for k in range(K2H):
                rhsv = emb_pm[:, k:k + 1] if k < KH else hid_sb[:, k - KH:k - KH + 1]
                for m in range(4):
                    nc.tensor.matmul(t1_tiles[m][:],
                                     lhsT=attn_sb[:, k, m * 128:(m + 1) * 128],
                                     rhs=rhsv,
                                     start=(k == 0), stop=(k == K2H - 1))
            t1cp = sb.tile([128, 4], F32, name="t1cp")
            for m in range(4):
                nc.vector.tensor_copy(t1cp[:, m:m + 1], t1_tiles[m][:])
            t1t_ps = ps.tile([4, 128], F32, name="t1t_ps", tag="scr")
            nc.tensor.transpose(t1t_ps[:], t1cp[:], ident[:])
            t1_sb = sb.tile([4, 128], F32, name="t1_sb")
            nc.vector.tensor_add(t1_sb[:], t1t_ps[:], ab_sb[:])
            ag1_in = dram.tile([4, 128], F32)
            ag1_out = dram.tile([KS, 128], F32)
            nc.gpsimd.dma_start(out=ag1_in[:], in_=t1_sb[:])
            nc.gpsimd.collective_compute(
                "AllGather", mybir.AluOpType.bypass, replica_groups=rg,
                ins=[ag1_in.opt()], outs=[ag1_out.opt()])

            # ---- softmax over full S (partition-major, no max-sub needed) ----
            t1_32 = sb.tile([KS, 128], F32, name="t1_32")
            nc.gpsimd.dma_start(out=t1_32[:], in_=ag1_out[:])
            tp_t1 = ps.tile([128, KS], F32, name="tp_t1", tag="scr")
            nc.tensor.transpose(tp_t1[:], t1_32[:], ident[:KS, :KS])
            u_sb = sb.tile([128, KS], F32, name="u_sb")
            srow = sb.tile([128, 1], F32, name="srow")
            nc.scalar.activation(u_sb[:], tp_t1[:], AF.Exp, accum_out=srow[:])
            u_bf = sb.tile([128, KS], BF16, name="u_bf")
            nc.vector.tensor_copy(u_bf[:], u_sb[:])

            # ---- attn_applied with unnormalized weights; scaled after the
            # matmul so the reciprocal chain overlaps the PE work ----
            aa_ps = ps.tile([1, 128], F32, name="aa_ps", tag="scr")
            for k in range(KS):
                nc.tensor.matmul(aa_ps[:], lhsT=u_bf[:, k:k + 1],
                                 rhs=enc_sb[:, k, :],
                                 start=(k == 0), stop=(k == KS - 1))
            s1_ps = ps.tile([1, 1], F32, name="s1_ps", tag="scr")
            nc.tensor.matmul(s1_ps[:], lhsT=srow[:], rhs=ones_p[:],
                             start=True, stop=True)
            rs_sb = sb.tile([1, 1], F32, name="rs_sb")
            nc.vector.reciprocal(rs_sb[:], s1_ps[:])
            rb_ps = ps.tile([128, 1], F32, name="rb_ps", tag="scr")
            nc.tensor.matmul(rb_ps[:], lhsT=ones_1[:], rhs=rs_sb[:],
                             start=True, stop=True)
            rs_bc = sb.tile([128, 1], F32, name="rs_bc")
            nc.vector.tensor_copy(rs_bc[:], rb_ps[:])
            aw_sb = sb.tile([128, KS], F32, name="aw_sb")
            nc.vector.tensor_scalar_mul(aw_sb[:], u_sb[:], rs_bc[:])
            nc.gpsimd.dma_start(out=attn_out[:, :], in_=aw_sb[:])
            aa_sb = sb.tile([1, 128], F32, name="aa_sb")
            nc.vector.tensor_scalar_mul(aa_sb[:], aa_ps[:], rs_sb[:])
            ag2_in = dram.tile([1, 128], F32)
            ag2_out = dram.tile([NC, 128], F32)
            nc.gpsimd.dma_start(out=ag2_in[:], in_=aa_sb[:])
            nc.gpsimd.collective_compute(
                "AllGather", mybir.AluOpType.bypass, replica_groups=rg,
                ins=[ag2_in.opt()], outs=[ag2_out.opt()])

            # ---- comb + AG3 ----
            aa8 = sb.tile([KH, 128], F32, name="aa8")
            nc.gpsimd.dma_start(out=aa8[:], in_=ag2_out[:])
            tp_aa = ps.tile([128, KH], F32, name="tp_aa", tag="scr")
            nc.tensor.transpose(tp_aa[:], aa8[:], ident[:KH, :KH])
            aa_bf = sb.tile([128, KH], BF16, name="aa_bf")
            nc.vector.tensor_copy(aa_bf[:], tp_aa[:])
            cb_ps = ps.tile([1, 128], F32, name="cb_ps", tag="scr")
            for k in range(K2H):
                lhs = embc_bf[:, k:k + 1] if k < KH else aa_bf[:, k - KH:k - KH + 1]
                nc.tensor.matmul(cb_ps[:], lhsT=lhs, rhs=comb_sb[:, k, :],
                                 start=(k == 0), stop=(k == K2H - 1))
            cbo_sb = sb.tile([1, 128], F32, name="cbo_sb")
            nc.vector.tensor_add(cbo_sb[:], cb_ps[:], cbias_sb[:])
            ag3_in = dram.tile([1, 128], F32)
            ag3_out = dram.tile([NC, 128], F32)
            nc.gpsimd.dma_start(out=ag3_in[:], in_=cbo_sb[:])
            nc.gpsimd.collective_compute(
                "AllGather", mybir.AluOpType.bypass, replica_groups=rg,
                ins=[ag3_in.opt()], outs=[ag3_out.opt()])

            # ---- GRU layer 1 ----
            x18 = sb.tile([KH, 128], F32, name="x18")
            nc.gpsimd.dma_start(out=x18[:], in_=ag3_out[:])
            tp_x1 = ps.tile([128, KH], F32, name="tp_x1", tag="scr")
            nc.tensor.transpose(tp_x1[:], x18[:], ident[:KH, :KH])
            x1_sb = sb.tile([128, KH], F32, name="x1_sb")
            nc.scalar.activation(x1_sb[:], tp_x1[:], AF.Relu)
            h1_col = _gru_layer(nc, sb, ps, x1_sb, hid_sb, h0_sb,
                                wih_sb, whh_sb, bi_sb, bh_sb, bsum_sb)
            ag4_in = dram.tile([128, 1], F32)
            ag4_out = dram.tile([NC * 128, 1], F32)
            nc.gpsimd.dma_start(out=ag4_in[:], in_=h1_col[:])
            nc.gpsimd.collective_compute(
                "AllGather", mybir.AluOpType.bypass, replica_groups=rg,
                ins=[ag4_in.opt()], outs=[ag4_out.opt()])

            # ---- GRU layer 2 ----
            h18 = sb.tile([KH, 128], F32, name="h18")
            nc.gpsimd.dma_start(out=h18[:],
                                in_=ag4_out[:].rearrange("(a b) o -> a (b o)", b=128))
            tp_h1 = ps.tile([128, KH], F32, name="tp_h1", tag="scr")
            nc.tensor.transpose(tp_h1[:], h18[:], ident[:KH, :KH])
            x2_sb = sb.tile([128, KH], F32, name="x2_sb")
            nc.scalar.activation(x2_sb[:], tp_h1[:], AF.Relu)
            h1f_sb = sb.tile([128, KH], F32, name="h1f_sb")
            nc.vector.tensor_copy(h1f_sb[:], tp_h1[:])
            h2_col = _gru_layer(nc, sb, ps, x2_sb, h1f_sb, h1_col,
                                wih_sb, whh_sb, bi_sb, bh_sb, bsum_sb)
            nc.gpsimd.dma_start(out=h_out[:, :], in_=h2_col[:])
            ag5_in = dram.tile([128, 1], F32)
            ag5_out = dram.tile([NC * 128, 1], F32)
            nc.gpsimd.dma_start(out=ag5_in[:], in_=h2_col[:])
            nc.gpsimd.collective_compute(
                "AllGather", mybir.AluOpType.bypass, replica_groups=rg,
                ins=[ag5_in.opt()], outs=[ag5_out.opt()])

            # ---- logits over this core's 6400 padded vocab cols ----
            h28 = sb.tile([KH, 128], F32, name="h28")
            nc.gpsimd.dma_start(out=h28[:],
                                in_=ag5_out[:].rearrange("(a b) o -> a (b o)", b=128))
            tp_h2 = ps.tile([128, KH], F32, name="tp_h2", tag="scr")
            nc.tensor.transpose(tp_h2[:], h28[:], ident[:KH, :KH])
            h2w_sb = sb.tile([128, KH], BF16, name="h2w_sb")
            nc.vector.tensor_copy(h2w_sb[:], tp_h2[:])

            lg_ps = lgps.tile([128, MCH], F32, name="lg_ps", tag="lg")
            for m in range(MCH):
                for k in range(KH):
                    nc.tensor.matmul(lg_ps[:, m:m + 1], lhsT=ow_tiles[k][:, m, :],
                                     rhs=h2w_sb[:, k:k + 1],
                                     start=(k == 0), stop=(k == KH - 1))
            lg_sb = sb.tile([128, MCH], F32, name="lg_sb", bufs=1)
            nc.vector.tensor_add(lg_sb[:], lg_ps[:], ob_sb[:])

            # ---- log-softmax denominator via AG6 ----
            elg = sb.tile([128, MCH], F32, name="elg")
            srl = sb.tile([128, 1], F32, name="srl")
            nc.scalar.activation(elg[:], lg_sb[:], AF.Exp, accum_out=srl[:])
            st_ps = ps.tile([1, 1], F32, name="st_ps", tag="scr")
            nc.tensor.matmul(st_ps[:], lhsT=srl[:], rhs=ones_p[:],
                             start=True, stop=True)
            sc8 = sb.tile([1, 8], F32, name="sc8")
            nc.vector.memset(sc8[:], 0.0)
            nc.vector.tensor_copy(sc8[:, 0:1], st_ps[:])
            ag6_in = dram.tile([1, 8], F32)
            ag6_out = dram.tile([NC, 8], F32)
            nc.gpsimd.dma_start(out=ag6_in[:], in_=sc8[:])
            nc.gpsimd.collective_compute(
                "AllGather", mybir.AluOpType.bypass, replica_groups=rg,
                ins=[ag6_in.opt()], outs=[ag6_out.opt()])
            sg = sb.tile([1, NC, 8], F32, name="sg")
            nc.gpsimd.dma_start(out=sg[:],
                                in_=ag6_out[:].rearrange("a b -> (a b)")[None, :]
                                .rearrange("o (a b) -> o a b", b=8))
            tot = sb.tile([1, 1], F32, name="tot")
            nc.vector.tensor_reduce(tot[:], sg[:, :, 0:1],
                                    axis=mybir.AxisListType.XY, op=mybir.AluOpType.add)
            lse = sb.tile([1, 1], F32, name="lse")
            nc.scalar.activation(lse[:], tot[:], AF.Ln)
            lb_ps = ps.tile([128, 1], F32, name="lb_ps", tag="scr")
            nc.tensor.matmul(lb_ps[:], lhsT=ones_1[:], rhs=lse[:],
                             start=True, stop=True)
            lse_bc = sb.tile([128, 1], F32, name="lse_bc")
            nc.vector.tensor_copy(lse_bc[:], lb_ps[:])
            lp_sb = sb.tile([128, MCH], F32, name="lp_sb")
            nc.vector.tensor_scalar(lp_sb[:], lg_sb[:], lse_bc[:], None,
                                    op0=mybir.AluOpType.subtract)
            nc.sync.dma_start(out=logp_out[:, :], in_=lp_sb[:])

    nc.compile()
    return nc


def shard_inputs(inputs):
    """FULL numpy inputs -> list of 8 per-core input maps."""
    import ml_dtypes
    bf16 = ml_dtypes.bfloat16

    f = lambda k: np.ascontiguousarray(np.asarray(inputs[k], np.float32))
    idx = int(np.asarray(inputs["input"]).ravel()[0])
    emb = f("emb")
    hidden = f("hidden").reshape(H)
    attn_W, attn_b = f("attn_W"), f("attn_b")
    enc = f("encoder_outputs")
    comb_W, comb_b = f("comb_W"), f("comb_b")
    Wih, Whh = f("gru_Wih"), f("gru_Whh")
    bih, bhh = f("gru_bih"), f("gru_bhh")
    out_W, out_b = f("out_W"), f("out_b")

    owp = np.zeros((H, VP), np.float32)
    owp[:, :V] = out_W
    owp = owp.astype(bf16)
    obp = np.full((VP,), -1e30, np.float32)
    obp[:V] = out_b

    A_ih = np.ascontiguousarray(Wih.T.reshape(H, 3, H))
    A_hh = np.ascontiguousarray(Whh.T.reshape(H, 3, H))
    bi3 = bih.reshape(3, H)
    bh3 = bhh.reshape(3, H)

    embr = emb.reshape(V * KH, 128)
    idx8 = (idx * KH + np.arange(KH, dtype=np.int32)).reshape(KH, 1)
    hidden8 = np.ascontiguousarray(hidden.reshape(KH, 128))
    enc_bf = enc.astype(bf16)
    comb_bf = comb_W.astype(bf16)

    in_maps = []
    for c in range(NC):
        sl = slice(c * 128, (c + 1) * 128)
        ssl = slice(c * SS, (c + 1) * SS)
        vsl = slice(c * VS, (c + 1) * VS)
        in_maps.append({
            "idx8": idx8,
            "embr": embr,
            "hidden8": hidden8,
            "h0col": np.ascontiguousarray(hidden[sl, None]),
            "attn_w": np.ascontiguousarray(attn_W[:, ssl]),
            "attn_b": np.ascontiguousarray(attn_b[ssl].reshape(4, 128)),
            "enc": np.ascontiguousarray(enc_bf[:, sl]),
            "comb_w": np.ascontiguousarray(comb_bf[:, sl]),
            "comb_b": np.ascontiguousarray(comb_b[None, sl]),
            "wih": np.ascontiguousarray(A_ih[:, :, sl]),
            "whh": np.ascontiguousarray(A_hh[:, :, sl]),
            "bi": np.ascontiguousarray(bi3[:, sl].T),
            "bh": np.ascontiguousarray(bh3[:, sl].T),
            "ow": np.ascontiguousarray(owp[:, vsl]),
            "ob": np.ascontiguousarray(obp[vsl].reshape(MCH, 128).T),
        })
    return in_maps


def unshard_outputs(results):
    """list of per-core {name: array} -> (log_probs, hidden, attn_weights)."""
    logp = np.concatenate(
        [np.asarray(r["logp_out"]).T.reshape(-1) for r in results])[:V][None, :]
    h = np.concatenate([np.asarray(r["h_out"]).ravel() for r in results])[None, None, :]
    aw = np.asarray(results[0]["attn_out"]).T.reshape(-1)[None, :]
    return (np.ascontiguousarray(logp, np.float32),
            np.ascontiguousarray(h, np.float32),
            np.ascontiguousarray(aw, np.float32))


def _get_program():
    if "nc" not in _CACHE:
        _CACHE["nc"] = build_program()
    return _CACHE["nc"]


def kernel(**inputs):
    nc = _get_program()
    in_maps = shard_inputs(inputs)
    res = run_bass_kernel_spmd(nc, in_maps, core_ids=list(range(NC)), trace=False)
    return unshard_outputs(res.results)


# revision 17
# speedup vs baseline: 1.9199x; 1.0530x over previous
"""AttnDecoderRNN step on 8 Trainium2 NeuronCores (Bass/Tile, tensor-parallel).

Sharding (8 cores, stitched with AllGather where a full vector is needed):
  - embedding: table replicated (viewed [V*8, 128]); each core indirect-DMAs
    the token row as 8 chunk-rows (4KB HBM read) -- no collective.
  - attn scores: attn_W sharded over S columns (512/core); AG1 -> softmax
    computed partition-major on every core.
  - attn_applied: encoder_outputs sharded over H columns (128/core); AG2.
  - comb: comb_W sharded over H columns; AG3.
  - GRU (x2, shared weights): gate outputs sharded over H (128/core for each
    of the 3 gates); h re-gathered with AG4/AG5 for the next contraction.
  - logits: out_W sharded over vocab columns (6400/core after padding V to
    51200); log_softmax denominator from an AG of per-core exp-sums (AG6);
    each core writes its own log-prob shard, the host concatenates.

Precision: f32 everywhere except out_W / encoder_outputs / comb_W (and the
activation vectors feeding those three matmuls), which run in bf16.
Verified end-to-end error vs the f32 reference: log_probs ~2e-4, hidden and
attn_weights ~2e-5 (max relative).

Queue plan: all weight streaming on the sync (SP) DMA queue in consumption
order; collective bounce-buffer traffic and gather loads on the gpsimd queue
so they interleave with the collectives; compute engines stay DMA-free.
"""

import numpy as np

import concourse.bacc as bacc
import concourse.bass as bass
import concourse.tile as tile
from concourse import mybir
from concourse.bass import IndirectOffsetOnAxis
from concourse.bass_utils import run_bass_kernel_spmd
from concourse.masks import make_identity

H = 1024
S = 4096
V = 50257
L = 2
NC = 8
VP = 51200          # padded vocab
VS = VP // NC       # 6400 vocab cols per core
MCH = VS // 128     # 50 m-chunks per core
SS = S // NC        # 512 attn cols per core
KH = H // 128       # 8 k-chunks for H contraction
K2H = 2 * H // 128  # 16 k-chunks for 2H contraction
KS = S // 128       # 32 k-chunks for S contraction

F32 = mybir.dt.float32
BF16 = mybir.dt.bfloat16

_CACHE = {}


def _gru_layer(nc, sb, ps, x_sb, h_sb, h_col, wih_sb, whh_sb,
               bi_sb, bh_sb, bsum_sb):
    """One GRU cell step; returns h_new [128,1] (this core's slice)."""
    AF = mybir.ActivationFunctionType
    gi_ps = ps.tile([128, 3], F32, name="gi_ps", tag="scr")
    gh_ps = ps.tile([128, 3], F32, name="gh_ps", tag="scr")
    for g in range(3):
        for k in range(KH):
            nc.tensor.matmul(gi_ps[:, g:g + 1], lhsT=wih_sb[:, k, g, :],
                             rhs=x_sb[:, k:k + 1],
                             start=(k == 0), stop=(k == KH - 1))
    for g in range(3):
        for k in range(KH):
            nc.tensor.matmul(gh_ps[:, g:g + 1], lhsT=whh_sb[:, k, g, :],
                             rhs=h_sb[:, k:k + 1],
                             start=(k == 0), stop=(k == KH - 1))
    gh_sb = sb.tile([128, 3], F32, name="gh_sb")
    nc.vector.tensor_copy(gh_sb[:], gh_ps[:])
    rz_sb = sb.tile([128, 2], F32, name="rz_sb")
    nc.vector.tensor_add(rz_sb[:], gi_ps[:, 0:2], gh_sb[:, 0:2])
    # sigmoid via tanh to stay on the Exp/Tanh activation table:
    # sigmoid(a) = 0.5 + 0.5*tanh(a/2)
    v = sb.tile([128, 1], F32, name="gru_v")  # tanh(r-gate/2)
    u = sb.tile([128, 1], F32, name="gru_u")  # tanh(z-gate/2)
    nc.scalar.activation(v[:], rz_sb[:, 0:1], AF.Tanh,
                         bias=bsum_sb[:, 0:1], scale=0.5)
    nc.scalar.activation(u[:], rz_sb[:, 1:2], AF.Tanh,
                         bias=bsum_sb[:, 1:2], scale=0.5)
    hnb = sb.tile([128, 1], F32, name="gru_hnb")
    nc.vector.tensor_scalar_add(hnb[:], gh_sb[:, 2:3], bh_sb[:, 2:3])
    # rn = sigmoid(r)*hnb = 0.5*(hnb + tanh(r/2)*hnb)
    vh = sb.tile([128, 1], F32, name="gru_vh")
    nc.vector.tensor_mul(vh[:], v[:], hnb[:])
    rh = sb.tile([128, 1], F32, name="gru_rh")
    nc.vector.tensor_add(rh[:], hnb[:], vh[:])
    tin = sb.tile([128, 1], F32, name="gru_tin")
    nc.vector.tensor_scalar(tin[:], rh[:], 0.5, None, op0=mybir.AluOpType.mult)
    nc.vector.tensor_add(tin[:], gi_ps[:, 2:3], tin[:])
    n = sb.tile([128, 1], F32, name="gru_n")
    nc.scalar.activation(n[:], tin[:], AF.Tanh, bias=bi_sb[:, 2:3])
    # h_new = n + sigmoid(z)*(h-n) = n + 0.5*((h-n) + tanh(z/2)*(h-n))
    d = sb.tile([128, 1], F32, name="gru_d")
    nc.vector.tensor_sub(d[:], h_col[:], n[:])
    ud = sb.tile([128, 1], F32, name="gru_ud")
    nc.vector.tensor_mul(ud[:], u[:], d[:])
    e = sb.tile([128, 1], F32, name="gru_e")
    nc.vector.tensor_add(e[:], d[:], ud[:])
    nc.vector.tensor_scalar(e[:], e[:], 0.5, None, op0=mybir.AluOpType.mult)
    h_new = sb.tile([128, 1], F32, name="gru_hnew")
    nc.vector.tensor_add(h_new[:], n[:], e[:])
    return h_new


def build_program():
    nc = bacc.Bacc("TRN2", target_bir_lowering=False, debug=False, num_devices=NC)
    AF = mybir.ActivationFunctionType

    # ---- kernel I/O (per core) ----
    idx8 = nc.dram_tensor("idx8", [KH, 1], mybir.dt.int32, kind="ExternalInput")
    embr = nc.dram_tensor("embr", [V * KH, 128], F32, kind="ExternalInput")
    hidden8 = nc.dram_tensor("hidden8", [KH, 128], F32, kind="ExternalInput")
    h0col = nc.dram_tensor("h0col", [128, 1], F32, kind="ExternalInput")
    attn_w = nc.dram_tensor("attn_w", [2 * H, SS], F32, kind="ExternalInput")
    attn_b = nc.dram_tensor("attn_b", [4, 128], F32, kind="ExternalInput")
    enc = nc.dram_tensor("enc", [S, 128], BF16, kind="ExternalInput")
    comb_w = nc.dram_tensor("comb_w", [2 * H, 128], BF16, kind="ExternalInput")
    comb_b = nc.dram_tensor("comb_b", [1, 128], F32, kind="ExternalInput")
    wih = nc.dram_tensor("wih", [H, 3, 128], F32, kind="ExternalInput")
    whh = nc.dram_tensor("whh", [H, 3, 128], F32, kind="ExternalInput")
    bi = nc.dram_tensor("bi", [128, 3], F32, kind="ExternalInput")
    bh = nc.dram_tensor("bh", [128, 3], F32, kind="ExternalInput")
    ow = nc.dram_tensor("ow", [H, VS], BF16, kind="ExternalInput")
    ob = nc.dram_tensor("ob", [128, MCH], F32, kind="ExternalInput")

    attn_out = nc.dram_tensor("attn_out", [128, KS], F32, kind="ExternalOutput")
    h_out = nc.dram_tensor("h_out", [128, 1], F32, kind="ExternalOutput")
    logp_out = nc.dram_tensor("logp_out", [128, MCH], F32, kind="ExternalOutput")

    rg = [list(range(NC))]

    with tile.TileContext(nc) as tc:
        with (
            tc.tile_pool(name="w", bufs=1) as wp,
            tc.tile_pool(name="sb", bufs=2) as sb,
            tc.tile_pool(name="owp", bufs=KH) as owp,
            tc.tile_pool(name="ps", bufs=4, space="PSUM") as ps,
            tc.tile_pool(name="lgps", bufs=2, space="PSUM") as lgps,
            tc.tile_pool(name="dram", bufs=1, space="DRAM") as dram,
        ):
            # ---- tiny chain-critical loads first (SP queue) ----
            idx_sb = sb.tile([KH, 1], mybir.dt.int32, name="idx_sb", bufs=1)
            nc.sync.dma_start(out=idx_sb[:], in_=idx8[:, :])
            h08 = sb.tile([KH, 128], F32, name="h08", bufs=1)
            nc.sync.dma_start(out=h08[:], in_=hidden8[:, :])

            # ---- weight streams (SP queue, consumption order) ----
            # attn_W chunked so each k-chunk matmul starts as soon as its
            # 256KB slice lands (pipeline DMA with the 16 PE matmuls).
            attn_sb = wp.tile([128, K2H, SS], F32)
            for k in range(K2H):
                eng = (nc.sync, nc.scalar, nc.gpsimd)[k % 3]
                eng.dma_start(out=attn_sb[:, k, :],
                              in_=attn_w[k * 128:(k + 1) * 128, :])
            ab_sb = wp.tile([4, 128], F32)
            nc.sync.dma_start(out=ab_sb[:], in_=attn_b[:, :])
            h0_sb = wp.tile([128, 1], F32)
            nc.sync.dma_start(out=h0_sb[:], in_=h0col[:, :])
            bi_sb = wp.tile([128, 3], F32)
            nc.sync.dma_start(out=bi_sb[:], in_=bi[:, :])
            bh_sb = wp.tile([128, 3], F32)
            nc.sync.dma_start(out=bh_sb[:], in_=bh[:, :])
            cbias_sb = wp.tile([1, 128], F32)
            nc.sync.dma_start(out=cbias_sb[:], in_=comb_b[:, :])
            ob_sb = wp.tile([128, MCH], F32)
            nc.sync.dma_start(out=ob_sb[:], in_=ob[:, :])
            wih_sb = wp.tile([128, KH, 3, 128], F32)
            nc.sync.dma_start(out=wih_sb[:],
                              in_=wih[:, :, :].rearrange("(k p) g n -> p k g n", p=128))
            whh_sb = wp.tile([128, KH, 3, 128], F32)
            nc.sync.dma_start(out=whh_sb[:],
                              in_=whh[:, :, :].rearrange("(k p) g n -> p k g n", p=128))
            enc_sb = wp.tile([128, KS, 128], BF16)
            nc.sync.dma_start(out=enc_sb[:],
                              in_=enc[:, :].rearrange("(k p) n -> p k n", p=128))
            comb_sb = wp.tile([128, K2H, 128], BF16)
            nc.sync.dma_start(out=comb_sb[:],
                              in_=comb_w[:, :].rearrange("(k p) n -> p k n", p=128))
            ow_tiles = []
            for k in range(KH):
                ow_k = owp.tile([128, MCH, 128], BF16, name="ow_k")
                nc.sync.dma_start(
                    out=ow_k[:],
                    in_=ow[k * 128:(k + 1) * 128, :].rearrange("p (m n) -> p m n", n=128))
                ow_tiles.append(ow_k)

            # ---- constants ----
            ident = wp.tile([128, 128], F32)
            make_identity(nc, ident[:])
            ones_p = wp.tile([128, 1], F32)
            nc.vector.memset(ones_p[:], 1.0)
            ones_1 = wp.tile([1, 128], F32)
            nc.vector.memset(ones_1[:], 1.0)
            # prewarm the Exp/Tanh activation table off the critical path
            warm = sb.tile([1, 1], F32, name="warm", bufs=1)
            nc.vector.memset(warm[:], 0.0)
            nc.scalar.activation(warm[:], warm[:], AF.Exp)
            # keep the PE busy from t=0 so the clock is ramped before the
            # first real matmuls
            pe_warm = ps.tile([128, 128], F32, name="pe_warm", tag="scr")
            for _ in range(10):
                nc.tensor.transpose(pe_warm[:], ident[:], ident[:])
            # halved gate bias: tanh(0.5*a + 0.5*(bi+bh)) for the sigmoid trick
            bsum_sb = wp.tile([128, 2], F32)
            nc.vector.tensor_add(bsum_sb[:], bi_sb[:, 0:2], bh_sb[:, 0:2])
            nc.vector.tensor_scalar(bsum_sb[:], bsum_sb[:], 0.5, None,
                                    op0=mybir.AluOpType.mult)

            # ---- embedding row gather (gpsimd) + partition-major transposes --
            er8 = sb.tile([KH, 128], F32, name="er8", bufs=1)
            nc.gpsimd.indirect_dma_start(
                out=er8[:], out_offset=None,
                in_=embr[:, :],
                in_offset=IndirectOffsetOnAxis(ap=idx_sb[:, :1], axis=0))
            tp_e = ps.tile([128, KH], F32, name="tp_e", tag="scr")
            nc.tensor.transpose(tp_e[:], er8[:], ident[:KH, :KH])
            emb_pm = sb.tile([128, KH], F32, name="emb_pm", bufs=1)
            nc.vector.tensor_copy(emb_pm[:], tp_e[:])
            embc_bf = sb.tile([128, KH], BF16, name="embc_bf", bufs=1)
            nc.vector.tensor_copy(embc_bf[:], tp_e[:])
            tp_h0 = ps.tile([128, KH], F32, name="tp_h0", tag="scr")
            nc.tensor.transpose(tp_h0[:], h08[:], ident[:KH, :KH])
            hid_sb = sb.tile([128, KH], F32, name="hid_sb", bufs=1)
            nc.vector.tensor_copy(hid_sb[:], tp_h0[:])

            # ---- attn scores (this core's 512 cols of S) + AG1 ----
            # M-orientation: attn_W chunk is the stationary operand, the
            # activation column streams (N=1); scores land partition-major
            # [128, 4] and are transposed back to linear for the AllGather.
            # Interleaved per-column psum groups are element-disjoint (safe);
            # the group checker only tracks regions, hence skip_group_check.
            t1_tiles = [ps.tile([128, 1], F32, name=f"t1_ps{m}", tag="scr")
                        for m in range(4)]
            for k in range(K2H):
                rhsv = emb_pm[:, k:k + 1] if k < KH else hid_sb[:, k - KH:k - KH + 1]
                for m in range(4):
                    nc.tensor.matmul(t1_tiles[m][:],
                                     lhsT=attn_sb[:, k, m * 128:(m + 1) * 128],
                                     rhs=rhsv,
                                     start=(k == 0), stop=(k == K2H - 1))
            t1cp = sb.tile([128, 4], F32, name="t1cp")
            for m in range(4):
                nc.vector.tensor_copy(t1cp[:, m:m + 1], t1_tiles[m][:])
            t1t_ps = ps.tile([4, 128], F32, name="t1t_ps", tag="scr")
            nc.tensor.transpose(t1t_ps[:], t1cp[:], ident[:])
            t1_sb = sb.tile([4, 128], F32, name="t1_sb")
            nc.vector.tensor_add(t1_sb[:], t1t_ps[:], ab_sb[:])
            ag1_in = dram.tile([4, 128], F32)
            ag1_out = dram.tile([KS, 128], F32)
            nc.gpsimd.dma_start(out=ag1_in[:], in_=t1_sb[:])
            nc.gpsimd.collective_compute(
                "AllGather", mybir.AluOpType.bypass, replica_groups=rg,
                ins=[ag1_in.opt()], outs=[ag1_out.opt()])

            # ---- softmax over full S (partition-major, no max-sub needed) ----
            t1_32 = sb.tile([KS, 128], F32, name="t1_32")
            nc.gpsimd.dma_start(out=t1_32[:], in_=ag1_out[:])
            tp_t1 = ps.tile([128, KS], F32, name="tp_t1", tag="scr")
            nc.tensor.transpose(tp_t1[:], t1_32[:], ident[:KS, :KS])
            u_sb = sb.tile([128, KS], F32, name="u_sb")
            srow = sb.tile([128, 1], F32, name="srow")
            nc.scalar.activation(u_sb[:], tp_t1[:], AF.Exp, accum_out=srow[:])
            u_bf = sb.tile([128, KS], BF16, name="u_bf")
            nc.vector.tensor_copy(u_bf[:], u_sb[:])

            # ---- attn_applied with unnormalized weights; scaled after the
            # matmul so the reciprocal chain overlaps the PE work ----
            aa_ps = ps.tile([1, 128], F32, name="aa_ps", tag="scr")
            for k in range(KS):
                nc.tensor.matmul(aa_ps[:], lhsT=u_bf[:, k:k + 1],
                                 rhs=enc_sb[:, k, :],
                                 start=(k == 0), stop=(k == KS - 1))
            s1_ps = ps.tile([1, 1], F32, name="s1_ps", tag="scr")
            nc.tensor.matmul(s1_ps[:], lhsT=srow[:], rhs=ones_p[:],
                             start=True, stop=True)
            rs_sb = sb.tile([1, 1], F32, name="rs_sb")
            nc.vector.reciprocal(rs_sb[:], s1_ps[:])
            rb_ps = ps.tile([128, 1], F32, name="rb_ps", tag="scr")
            nc.tensor.matmul(rb_ps[:], lhsT=ones_1[:], rhs=rs_sb[:],
                             start=True, stop=True)
            rs_bc = sb.tile([128, 1], F32, name="rs_bc")
            nc.vector.tensor_copy(rs_bc[:], rb_ps[:])
            aw_sb = sb.tile([128, KS], F32, name="aw_sb")
            nc.vector.tensor_scalar_mul(aw_sb[:], u_sb[:], rs_bc[:])
            nc.gpsimd.dma_start(out=attn_out[:, :], in_=aw_sb[:])
            aa_sb = sb.tile([1, 128], F32, name="aa_sb")
            nc.vector.tensor_scalar_mul(aa_sb[:], aa_ps[:], rs_sb[:])
            ag2_in = dram.tile([1, 128], F32)
            ag2_out = dram.tile([NC, 128], F32)
            nc.gpsimd.dma_start(out=ag2_in[:], in_=aa_sb[:])
            nc.gpsimd.collective_compute(
                "AllGather", mybir.AluOpType.bypass, replica_groups=rg,
                ins=[ag2_in.opt()], outs=[ag2_out.opt()])

            # ---- comb + AG3 ----
            aa8 = sb.tile([KH, 128], F32, name="aa8")
            nc.gpsimd.dma_start(out=aa8[:], in_=ag2_out[:])
            tp_aa = ps.tile([128, KH], F32, name="tp_aa", tag="scr")
            nc.tensor.transpose(tp_aa[:], aa8[:], ident[:KH, :KH])
            aa_bf = sb.tile([128, KH], BF16, name="aa_bf")
            nc.vector.tensor_copy(aa_bf[:], tp_aa[:])
            cb_ps = ps.tile([1, 128], F32, name="cb_ps", tag="scr")
            for k in range(K2H):
                lhs = embc_bf[:, k:k + 1] if k < KH else aa_bf[:, k - KH:k - KH + 1]
                nc.tensor.matmul(cb_ps[:], lhsT=lhs, rhs=comb_sb[:, k, :],
                                 start=(k == 0), stop=(k == K2H - 1))
            cbo_sb = sb.tile([1, 128], F32, name="cbo_sb")
            nc.vector.tensor_add(cbo_sb[:], cb_ps[:], cbias_sb[:])
            ag3_in = dram.tile([1, 128], F32)
            ag3_out = dram.tile([NC, 128], F32)
            nc.gpsimd.dma_start(out=ag3_in[:], in_=cbo_sb[:])
            nc.gpsimd.collective_compute(
                "AllGather", mybir.AluOpType.bypass, replica_groups=rg,
                ins=[ag3_in.opt()], outs=[ag3_out.opt()])

            # ---- GRU layer 1 ----
            x18 = sb.tile([KH, 128], F32, name="x18")
            nc.gpsimd.dma_start(out=x18[:], in_=ag3_out[:])
            tp_x1 = ps.tile([128, KH], F32, name="tp_x1", tag="scr")
            nc.tensor.transpose(tp_x1[:], x18[:], ident[:KH, :KH])
            x1_sb = sb.tile([128, KH], F32, name="x1_sb")
            nc.scalar.activation(x1_sb[:], tp_x1[:], AF.Relu)
            h1_col = _gru_layer(nc, sb, ps, x1_sb, hid_sb, h0_sb,
                                wih_sb, whh_sb, bi_sb, bh_sb, bsum_sb)
            ag4_in = dram.tile([128, 1], F32)
            ag4_out = dram.tile([NC * 128, 1], F32)
            nc.gpsimd.dma_start(out=ag4_in[:], in_=h1_col[:])
            nc.gpsimd.collective_compute(
                "AllGather", mybir.AluOpType.bypass, replica_groups=rg,
                ins=[ag4_in.opt()], outs=[ag4_out.opt()])

            # ---- GRU layer 2 ----
            h18 = sb.tile([KH, 128], F32, name="h18")
            nc.gpsimd.dma_start(out=h18[:],
                                in_=ag4_out[:].rearrange("(a b) o -> a (b o)", b=128))
            tp_h1 = ps.tile([128, KH], F32, name="tp_h1", tag="scr")
            nc.tensor.transpose(tp_h1[:], h18[:], ident[:KH, :KH])
            x2_sb = sb.tile([128, KH], F32, name="x2_sb")
            nc.scalar.activation(x2_sb[:], tp_h1[:], AF.Relu)
            h1f_sb = sb.tile([128, KH], F32, name="h1f_sb")
            nc.vector.tensor_copy(h1f_sb[:], tp_h1[:])
            h2_col = _gru_layer(nc, sb, ps, x2_sb, h1f_sb, h1_col,
                                wih_sb, whh_sb, bi_sb, bh_sb, bsum_sb)
            nc.gpsimd.dma_start(out=h_out[:, :], in_=h2_col[:])
            ag5_in = dram.tile([128, 1], F32)
            ag5_out = dram.tile([NC * 128, 1], F32)
            nc.gpsimd.dma_start(out=ag5_in[:], in_=h2_col[:])
            nc.gpsimd.collective_compute(
                "AllGather", mybir.AluOpType.bypass, replica_groups=rg,
                ins=[ag5_in.opt()], outs=[ag5_out.opt()])

            # ---- logits over this core's 6400 padded vocab cols ----
            h28 = sb.tile([KH, 128], F32, name="h28")
            nc.gpsimd.dma_start(out=h28[:],
                                in_=ag5_out[:].rearrange("(a b) o -> a (b o)", b=128))
            tp_h2 = ps.tile([128, KH], F32, name="tp_h2", tag="scr")
            nc.tensor.transpose(tp_h2[:], h28[:], ident[:KH, :KH])
            h2w_sb = sb.tile([128, KH], BF16, name="h2w_sb")
            nc.vector.tensor_copy(h2w_sb[:], tp_h2[:])

            lg_ps = lgps.tile([128, MCH], F32, name="lg_ps", tag="lg")
            for m in range(MCH):
                for k in range(KH):
                    nc.tensor.matmul(lg_ps[:, m:m + 1], lhsT=ow_tiles[k][:, m, :],
                                     rhs=h2w_sb[:, k:k + 1],
                                     start=(k == 0), stop=(k == KH - 1))
            lg_sb = sb.tile([128, MCH], F32, name="lg_sb", bufs=1)
            nc.vector.tensor_add(lg_sb[:], lg_ps[:], ob_sb[:])

            # ---- log-softmax denominator via AG6 ----
            elg = sb.tile([128, MCH], F32, name="elg")
            srl = sb.tile([128, 1], F32, name="srl")
            nc.scalar.activation(elg[:], lg_sb[:], AF.Exp, accum_out=srl[:])
            # pre-switch the ACT table to the Ln set while AG6 is in flight
            nc.scalar.activation(warm[:], warm[:], AF.Ln)
            st_ps = ps.tile([1, 1], F32, name="st_ps", tag="scr")
            nc.tensor.matmul(st_ps[:], lhsT=srl[:], rhs=ones_p[:],
                             start=True, stop=True)
            sc8 = sb.tile([1, 8], F32, name="sc8")
            nc.vector.memset(sc8[:], 0.0)
            nc.vector.tensor_copy(sc8[:, 0:1], st_ps[:])
            ag6_in = dram.tile([1, 8], F32)
            ag6_out = dram.tile([NC, 8], F32)
            nc.gpsimd.dma_start(out=ag6_in[:], in_=sc8[:])
            nc.gpsimd.collective_compute(
                "AllGather", mybir.AluOpType.bypass, replica_groups=rg,
                ins=[ag6_in.opt()], outs=[ag6_out.opt()])
            sg = sb.tile([1, NC, 8], F32, name="sg")
            nc.gpsimd.dma_start(out=sg[:],
                                in_=ag6_out[:].rearrange("a b -> (a b)")[None, :]
                                .rearrange("o (a b) -> o a b", b=8))
            tot = sb.tile([1, 1], F32, name="tot")
            nc.vector.tensor_reduce(tot[:], sg[:, :, 0:1],
                                    axis=mybir.AxisListType.XY, op=mybir.AluOpType.add)
            lse = sb.tile([1, 1], F32, name="lse")
            nc.scalar.activation(lse[:], tot[:], AF.Ln)
            lb_ps = ps.tile([128, 1], F32, name="lb_ps", tag="scr")
            nc.tensor.matmul(lb_ps[:], lhsT=ones_1[:], rhs=lse[:],
                             start=True, stop=True)
            lse_bc = sb.tile([128, 1], F32, name="lse_bc")
            nc.vector.tensor_copy(lse_bc[:], lb_ps[:])
            lp_sb = sb.tile([128, MCH], F32, name="lp_sb")
            nc.vector.tensor_scalar(lp_sb[:], lg_sb[:], lse_bc[:], None,
                                    op0=mybir.AluOpType.subtract)
            nc.sync.dma_start(out=logp_out[:, :], in_=lp_sb[:])

    nc.compile()
    return nc


def shard_inputs(inputs):
    """FULL numpy inputs -> list of 8 per-core input maps."""
    import ml_dtypes
    bf16 = ml_dtypes.bfloat16

    f = lambda k: np.ascontiguousarray(np.asarray(inputs[k], np.float32))
    idx = int(np.asarray(inputs["input"]).ravel()[0])
    emb = f("emb")
    hidden = f("hidden").reshape(H)
    attn_W, attn_b = f("attn_W"), f("attn_b")
    enc = f("encoder_outputs")
    comb_W, comb_b = f("comb_W"), f("comb_b")
    Wih, Whh = f("gru_Wih"), f("gru_Whh")
    bih, bhh = f("gru_bih"), f("gru_bhh")
    out_W, out_b = f("out_W"), f("out_b")

    owp = np.zeros((H, VP), np.float32)
    owp[:, :V] = out_W
    owp = owp.astype(bf16)
    obp = np.full((VP,), -1e30, np.float32)
    obp[:V] = out_b

    A_ih = np.ascontiguousarray(Wih.T.reshape(H, 3, H))
    A_hh = np.ascontiguousarray(Whh.T.reshape(H, 3, H))
    bi3 = bih.reshape(3, H)
    bh3 = bhh.reshape(3, H)

    embr = emb.reshape(V * KH, 128)
    idx8 = (idx * KH + np.arange(KH, dtype=np.int32)).reshape(KH, 1)
    hidden8 = np.ascontiguousarray(hidden.reshape(KH, 128))
    enc_bf = enc.astype(bf16)
    comb_bf = comb_W.astype(bf16)

    in_maps = []
    for c in range(NC):
        sl = slice(c * 128, (c + 1) * 128)
        ssl = slice(c * SS, (c + 1) * SS)
        vsl = slice(c * VS, (c + 1) * VS)
        in_maps.append({
            "idx8": idx8,
            "embr": embr,
            "hidden8": hidden8,
            "h0col": np.ascontiguousarray(hidden[sl, None]),
            "attn_w": np.ascontiguousarray(attn_W[:, ssl]),
            "attn_b": np.ascontiguousarray(attn_b[ssl].reshape(4, 128)),
            "enc": np.ascontiguousarray(enc_bf[:, sl]),
            "comb_w": np.ascontiguousarray(comb_bf[:, sl]),
            "comb_b": np.ascontiguousarray(comb_b[None, sl]),
            "wih": np.ascontiguousarray(A_ih[:, :, sl]),
            "whh": np.ascontiguousarray(A_hh[:, :, sl]),
            "bi": np.ascontiguousarray(bi3[:, sl].T),
            "bh": np.ascontiguousarray(bh3[:, sl].T),
            "ow": np.ascontiguousarray(owp[:, vsl]),
            "ob": np.ascontiguousarray(obp[vsl].reshape(MCH, 128).T),
        })
    return in_maps


def unshard_outputs(results):
    """list of per-core {name: array} -> (log_probs, hidden, attn_weights)."""
    logp = np.concatenate(
        [np.asarray(r["logp_out"]).T.reshape(-1) for r in results])[:V][None, :]
    h = np.concatenate([np.asarray(r["h_out"]).ravel() for r in results])[None, None, :]
    aw = np.asarray(results[0]["attn_out"]).T.reshape(-1)[None, :]
    return (np.ascontiguousarray(logp, np.float32),
            np.ascontiguousarray(h, np.float32),
            np.ascontiguousarray(aw, np.float32))


def _get_program():
    if "nc" not in _CACHE:
        _CACHE["nc"] = build_program()
    return _CACHE["nc"]


def kernel(**inputs):
    nc = _get_program()
    in_maps = shard_inputs(inputs)
    res = run_bass_kernel_spmd(nc, in_maps, core_ids=list(range(NC)), trace=False)
    return unshard_outputs(res.results)


# revision 18
# speedup vs baseline: 1.9878x; 1.0354x over previous
"""AttnDecoderRNN step on 8 Trainium2 NeuronCores (Bass/Tile, tensor-parallel).

Sharding (8 cores, stitched with AllGather where a full vector is needed):
  - embedding: table replicated (viewed [V*8, 128]); each core indirect-DMAs
    the token row as 8 chunk-rows (4KB HBM read) -- no collective.
  - attn scores: attn_W sharded over S columns (512/core); AG1 -> softmax
    computed partition-major on every core.
  - attn_applied: encoder_outputs sharded over H columns (128/core); AG2.
  - comb: comb_W sharded over H columns; AG3.
  - GRU (x2, shared weights): gate outputs sharded over H (128/core for each
    of the 3 gates); h re-gathered with AG4/AG5 for the next contraction.
  - logits: out_W sharded over vocab columns (6400/core after padding V to
    51200); log_softmax denominator from an AG of per-core exp-sums (AG6);
    each core writes its own log-prob shard, the host concatenates.

Precision: f32 everywhere except out_W / encoder_outputs / comb_W (and the
activation vectors feeding those three matmuls), which run in bf16.
Verified end-to-end error vs the f32 reference: log_probs ~2e-4, hidden and
attn_weights ~2e-5 (max relative).

Queue plan: all weight streaming on the sync (SP) DMA queue in consumption
order; collective bounce-buffer traffic and gather loads on the gpsimd queue
so they interleave with the collectives; compute engines stay DMA-free.
"""

import numpy as np

import concourse.bacc as bacc
import concourse.bass as bass
import concourse.tile as tile
from concourse import mybir
from concourse.bass import IndirectOffsetOnAxis
from concourse.bass_utils import run_bass_kernel_spmd
from concourse.masks import make_identity

H = 1024
S = 4096
V = 50257
L = 2
NC = 8
VP = 51200          # padded vocab
VS = VP // NC       # 6400 vocab cols per core
MCH = VS // 128     # 50 m-chunks per core
SS = S // NC        # 512 attn cols per core
KH = H // 128       # 8 k-chunks for H contraction
K2H = 2 * H // 128  # 16 k-chunks for 2H contraction
KS = S // 128       # 32 k-chunks for S contraction

F32 = mybir.dt.float32
BF16 = mybir.dt.bfloat16

_CACHE = {}


def _gru_layer(nc, sb, ps, x_sb, h_sb, h_col, wih_sb, whh_sb,
               bi_sb, bh_sb, bsum_sb):
    """One GRU cell step; returns h_new [128,1] (this core's slice)."""
    AF = mybir.ActivationFunctionType
    gi_ps = ps.tile([128, 3], F32, name="gi_ps", tag="scr")
    gh_ps = ps.tile([128, 3], F32, name="gh_ps", tag="scr")
    for g in range(3):
        for k in range(KH):
            nc.tensor.matmul(gi_ps[:, g:g + 1], lhsT=wih_sb[:, k, g, :],
                             rhs=x_sb[:, k:k + 1],
                             start=(k == 0), stop=(k == KH - 1))
    for g in range(3):
        for k in range(KH):
            nc.tensor.matmul(gh_ps[:, g:g + 1], lhsT=whh_sb[:, k, g, :],
                             rhs=h_sb[:, k:k + 1],
                             start=(k == 0), stop=(k == KH - 1))
    gh_sb = sb.tile([128, 3], F32, name="gh_sb")
    nc.vector.tensor_copy(gh_sb[:], gh_ps[:])
    rz_sb = sb.tile([128, 2], F32, name="rz_sb")
    nc.vector.tensor_add(rz_sb[:], gi_ps[:, 0:2], gh_sb[:, 0:2])
    # sigmoid via tanh to stay on the Exp/Tanh activation table:
    # sigmoid(a) = 0.5 + 0.5*tanh(a/2)
    v = sb.tile([128, 1], F32, name="gru_v")  # tanh(r-gate/2)
    u = sb.tile([128, 1], F32, name="gru_u")  # tanh(z-gate/2)
    nc.scalar.activation(v[:], rz_sb[:, 0:1], AF.Tanh,
                         bias=bsum_sb[:, 0:1], scale=0.5)
    nc.scalar.activation(u[:], rz_sb[:, 1:2], AF.Tanh,
                         bias=bsum_sb[:, 1:2], scale=0.5)
    hnb = sb.tile([128, 1], F32, name="gru_hnb")
    nc.vector.tensor_scalar_add(hnb[:], gh_sb[:, 2:3], bh_sb[:, 2:3])
    # rn = sigmoid(r)*hnb = 0.5*(hnb + tanh(r/2)*hnb)
    vh = sb.tile([128, 1], F32, name="gru_vh")
    nc.vector.tensor_mul(vh[:], v[:], hnb[:])
    rh = sb.tile([128, 1], F32, name="gru_rh")
    nc.vector.tensor_add(rh[:], hnb[:], vh[:])
    tin = sb.tile([128, 1], F32, name="gru_tin")
    nc.vector.tensor_scalar(tin[:], rh[:], 0.5, None, op0=mybir.AluOpType.mult)
    nc.vector.tensor_add(tin[:], gi_ps[:, 2:3], tin[:])
    n = sb.tile([128, 1], F32, name="gru_n")
    nc.scalar.activation(n[:], tin[:], AF.Tanh, bias=bi_sb[:, 2:3])
    # h_new = n + sigmoid(z)*(h-n) = n + 0.5*((h-n) + tanh(z/2)*(h-n))
    d = sb.tile([128, 1], F32, name="gru_d")
    nc.vector.tensor_sub(d[:], h_col[:], n[:])
    ud = sb.tile([128, 1], F32, name="gru_ud")
    nc.vector.tensor_mul(ud[:], u[:], d[:])
    e = sb.tile([128, 1], F32, name="gru_e")
    nc.vector.tensor_add(e[:], d[:], ud[:])
    nc.vector.tensor_scalar(e[:], e[:], 0.5, None, op0=mybir.AluOpType.mult)
    h_new = sb.tile([128, 1], F32, name="gru_hnew")
    nc.vector.tensor_add(h_new[:], n[:], e[:])
    return h_new


def build_program():
    nc = bacc.Bacc("TRN2", target_bir_lowering=False, debug=False, num_devices=NC)
    AF = mybir.ActivationFunctionType

    # ---- kernel I/O (per core) ----
    idx8 = nc.dram_tensor("idx8", [KH, 1], mybir.dt.int32, kind="ExternalInput")
    embr = nc.dram_tensor("embr", [V * KH, 128], F32, kind="ExternalInput")
    hidden8 = nc.dram_tensor("hidden8", [KH, 128], F32, kind="ExternalInput")
    h0col = nc.dram_tensor("h0col", [128, 1], F32, kind="ExternalInput")
    attn_w = nc.dram_tensor("attn_w", [2 * H, SS], F32, kind="ExternalInput")
    attn_b = nc.dram_tensor("attn_b", [4, 128], F32, kind="ExternalInput")
    enc = nc.dram_tensor("enc", [S, 128], BF16, kind="ExternalInput")
    comb_w = nc.dram_tensor("comb_w", [2 * H, 128], BF16, kind="ExternalInput")
    comb_b = nc.dram_tensor("comb_b", [128, 1], F32, kind="ExternalInput")
    wih = nc.dram_tensor("wih", [H, 3, 128], F32, kind="ExternalInput")
    whh = nc.dram_tensor("whh", [H, 3, 128], F32, kind="ExternalInput")
    bi = nc.dram_tensor("bi", [128, 3], F32, kind="ExternalInput")
    bh = nc.dram_tensor("bh", [128, 3], F32, kind="ExternalInput")
    ow = nc.dram_tensor("ow", [H, VS], BF16, kind="ExternalInput")
    ob = nc.dram_tensor("ob", [128, MCH], F32, kind="ExternalInput")

    attn_out = nc.dram_tensor("attn_out", [128, KS], F32, kind="ExternalOutput")
    h_out = nc.dram_tensor("h_out", [128, 1], F32, kind="ExternalOutput")
    logp_out = nc.dram_tensor("logp_out", [128, MCH], F32, kind="ExternalOutput")

    rg = [list(range(NC))]

    with tile.TileContext(nc) as tc:
        with (
            tc.tile_pool(name="w", bufs=1) as wp,
            tc.tile_pool(name="sb", bufs=2) as sb,
            tc.tile_pool(name="owp", bufs=KH) as owp,
            tc.tile_pool(name="ps", bufs=4, space="PSUM") as ps,
            tc.tile_pool(name="lgps", bufs=2, space="PSUM") as lgps,
            tc.tile_pool(name="dram", bufs=1, space="DRAM") as dram,
        ):
            # ---- tiny chain-critical loads first (SP queue) ----
            idx_sb = sb.tile([KH, 1], mybir.dt.int32, name="idx_sb", bufs=1)
            nc.sync.dma_start(out=idx_sb[:], in_=idx8[:, :])
            h08 = sb.tile([KH, 128], F32, name="h08", bufs=1)
            nc.sync.dma_start(out=h08[:], in_=hidden8[:, :])

            # ---- embedding row gather (gpsimd, ahead of its queue's
            # attn-chunk loads) ----
            er8 = sb.tile([KH, 128], F32, name="er8", bufs=1)
            nc.gpsimd.indirect_dma_start(
                out=er8[:], out_offset=None,
                in_=embr[:, :],
                in_offset=IndirectOffsetOnAxis(ap=idx_sb[:, :1], axis=0))

            # ---- weight streams (SP queue, consumption order) ----
            # attn_W chunked so each k-chunk matmul starts as soon as its
            # 256KB slice lands (pipeline DMA with the 16 PE matmuls).
            attn_sb = wp.tile([128, K2H, SS], F32)
            for k in range(K2H):
                eng = (nc.sync, nc.scalar, nc.gpsimd)[k % 3]
                eng.dma_start(out=attn_sb[:, k, :],
                              in_=attn_w[k * 128:(k + 1) * 128, :])
            ab_sb = wp.tile([4, 128], F32)
            nc.sync.dma_start(out=ab_sb[:], in_=attn_b[:, :])
            h0_sb = wp.tile([128, 1], F32)
            nc.sync.dma_start(out=h0_sb[:], in_=h0col[:, :])
            bi_sb = wp.tile([128, 3], F32)
            nc.sync.dma_start(out=bi_sb[:], in_=bi[:, :])
            bh_sb = wp.tile([128, 3], F32)
            nc.sync.dma_start(out=bh_sb[:], in_=bh[:, :])
            cbias_sb = wp.tile([128, 1], F32)
            nc.sync.dma_start(out=cbias_sb[:], in_=comb_b[:, :])
            ob_sb = wp.tile([128, MCH], F32)
            nc.sync.dma_start(out=ob_sb[:], in_=ob[:, :])
            wih_sb = wp.tile([128, KH, 3, 128], F32)
            nc.sync.dma_start(out=wih_sb[:],
                              in_=wih[:, :, :].rearrange("(k p) g n -> p k g n", p=128))
            whh_sb = wp.tile([128, KH, 3, 128], F32)
            nc.sync.dma_start(out=whh_sb[:],
                              in_=whh[:, :, :].rearrange("(k p) g n -> p k g n", p=128))
            enc_sb = wp.tile([128, KS, 128], BF16)
            nc.sync.dma_start(out=enc_sb[:],
                              in_=enc[:, :].rearrange("(k p) n -> p k n", p=128))
            comb_sb = wp.tile([128, K2H, 128], BF16)
            nc.sync.dma_start(out=comb_sb[:],
                              in_=comb_w[:, :].rearrange("(k p) n -> p k n", p=128))
            ow_tiles = []
            for k in range(KH):
                ow_k = owp.tile([128, MCH, 128], BF16, name="ow_k")
                nc.sync.dma_start(
                    out=ow_k[:],
                    in_=ow[k * 128:(k + 1) * 128, :].rearrange("p (m n) -> p m n", n=128))
                ow_tiles.append(ow_k)

            # ---- constants ----
            ident = wp.tile([128, 128], F32)
            make_identity(nc, ident[:])
            ones_p = wp.tile([128, 1], F32)
            nc.vector.memset(ones_p[:], 1.0)
            ones_1 = wp.tile([1, 128], F32)
            nc.vector.memset(ones_1[:], 1.0)
            # prewarm the Exp/Tanh activation table off the critical path
            warm = sb.tile([1, 1], F32, name="warm", bufs=1)
            nc.vector.memset(warm[:], 0.0)
            nc.scalar.activation(warm[:], warm[:], AF.Exp)
            # keep the PE busy from t=0 so the clock is ramped before the
            # first real matmuls
            pe_warm = ps.tile([128, 128], F32, name="pe_warm", tag="scr")
            for _ in range(10):
                nc.tensor.transpose(pe_warm[:], ident[:], ident[:])
            # halved gate bias: tanh(0.5*a + 0.5*(bi+bh)) for the sigmoid trick
            bsum_sb = wp.tile([128, 2], F32)
            nc.vector.tensor_add(bsum_sb[:], bi_sb[:, 0:2], bh_sb[:, 0:2])
            nc.vector.tensor_scalar(bsum_sb[:], bsum_sb[:], 0.5, None,
                                    op0=mybir.AluOpType.mult)

            # ---- partition-major transposes of emb row and hidden ----
            tp_e = ps.tile([128, KH], F32, name="tp_e", tag="scr")
            nc.tensor.transpose(tp_e[:], er8[:], ident[:KH, :KH])
            emb_pm = sb.tile([128, KH], F32, name="emb_pm", bufs=1)
            nc.vector.tensor_copy(emb_pm[:], tp_e[:])
            embc_bf = sb.tile([128, KH], BF16, name="embc_bf", bufs=1)
            nc.vector.tensor_copy(embc_bf[:], tp_e[:])
            tp_h0 = ps.tile([128, KH], F32, name="tp_h0", tag="scr")
            nc.tensor.transpose(tp_h0[:], h08[:], ident[:KH, :KH])
            hid_sb = sb.tile([128, KH], F32, name="hid_sb", bufs=1)
            nc.vector.tensor_copy(hid_sb[:], tp_h0[:])

            # ---- attn scores (this core's 512 cols of S) + AG1 ----
            # M-orientation: attn_W chunk is the stationary operand, the
            # activation column streams (N=1); scores land partition-major
            # [128, 4] and are transposed back to linear for the AllGather.
            # Interleaved per-column psum groups are element-disjoint (safe);
            # the group checker only tracks regions, hence skip_group_check.
            t1_tiles = [ps.tile([128, 1], F32, name=f"t1_ps{m}", tag="scr")
                        for m in range(4)]
            for k in range(K2H):
                rhsv = emb_pm[:, k:k + 1] if k < KH else hid_sb[:, k - KH:k - KH + 1]
                for m in range(4):
                    nc.tensor.matmul(t1_tiles[m][:],
                                     lhsT=attn_sb[:, k, m * 128:(m + 1) * 128],
                                     rhs=rhsv,
                                     start=(k == 0), stop=(k == K2H - 1))
            t1cp = sb.tile([128, 4], F32, name="t1cp")
            for m in range(4):
                nc.vector.tensor_copy(t1cp[:, m:m + 1], t1_tiles[m][:])
            t1t_ps = ps.tile([4, 128], F32, name="t1t_ps", tag="scr")
            nc.tensor.transpose(t1t_ps[:], t1cp[:], ident[:])
            t1_sb = sb.tile([4, 128], F32, name="t1_sb")
            nc.vector.tensor_add(t1_sb[:], t1t_ps[:], ab_sb[:])
            ag1_in = dram.tile([4, 128], F32)
            ag1_out = dram.tile([KS, 128], F32)
            nc.gpsimd.dma_start(out=ag1_in[:], in_=t1_sb[:])
            nc.gpsimd.collective_compute(
                "AllGather", mybir.AluOpType.bypass, replica_groups=rg,
                ins=[ag1_in.opt()], outs=[ag1_out.opt()])

            # ---- softmax over full S (partition-major, no max-sub needed) ----
            t1_32 = sb.tile([KS, 128], F32, name="t1_32")
            nc.gpsimd.dma_start(out=t1_32[:], in_=ag1_out[:])
            tp_t1 = ps.tile([128, KS], F32, name="tp_t1", tag="scr")
            nc.tensor.transpose(tp_t1[:], t1_32[:], ident[:KS, :KS])
            u_sb = sb.tile([128, KS], F32, name="u_sb")
            srow = sb.tile([128, 1], F32, name="srow")
            nc.scalar.activation(u_sb[:], tp_t1[:], AF.Exp, accum_out=srow[:])
            u_bf = sb.tile([128, KS], BF16, name="u_bf")
            nc.vector.tensor_copy(u_bf[:], u_sb[:])

            # ---- attn_applied with unnormalized weights; scaled after the
            # matmul so the reciprocal chain overlaps the PE work.
            # M-orientation: enc chunk stationary, u column streams; the
            # result lands partition-major [128,1] ready for the AllGather.
            aa_ps = ps.tile([128, 1], F32, name="aa_ps", tag="scr")
            for k in range(KS):
                nc.tensor.matmul(aa_ps[:], lhsT=enc_sb[:, k, :],
                                 rhs=u_bf[:, k:k + 1],
                                 start=(k == 0), stop=(k == KS - 1))
            s1_ps = ps.tile([1, 1], F32, name="s1_ps", tag="scr")
            nc.tensor.matmul(s1_ps[:], lhsT=srow[:], rhs=ones_p[:],
                             start=True, stop=True)
            rs_sb = sb.tile([1, 1], F32, name="rs_sb")
            nc.vector.reciprocal(rs_sb[:], s1_ps[:])
            rb_ps = ps.tile([128, 1], F32, name="rb_ps", tag="scr")
            nc.tensor.matmul(rb_ps[:], lhsT=ones_1[:], rhs=rs_sb[:],
                             start=True, stop=True)
            rs_bc = sb.tile([128, 1], F32, name="rs_bc")
            nc.vector.tensor_copy(rs_bc[:], rb_ps[:])
            aw_sb = sb.tile([128, KS], F32, name="aw_sb")
            nc.vector.tensor_scalar_mul(aw_sb[:], u_sb[:], rs_bc[:])
            nc.gpsimd.dma_start(out=attn_out[:, :], in_=aw_sb[:])
            aa_sb = sb.tile([128, 1], F32, name="aa_sb")
            nc.vector.tensor_scalar_mul(aa_sb[:], aa_ps[:], rs_bc[:])
            ag2_in = dram.tile([128, 1], F32)
            ag2_out = dram.tile([NC * 128, 1], F32)
            nc.gpsimd.dma_start(out=ag2_in[:], in_=aa_sb[:])
            nc.gpsimd.collective_compute(
                "AllGather", mybir.AluOpType.bypass, replica_groups=rg,
                ins=[ag2_in.opt()], outs=[ag2_out.opt()])

            # ---- comb + AG3 ----
            aa8 = sb.tile([KH, 128], F32, name="aa8")
            nc.gpsimd.dma_start(out=aa8[:],
                                in_=ag2_out[:].rearrange("(a b) o -> a (b o)", b=128))
            tp_aa = ps.tile([128, KH], F32, name="tp_aa", tag="scr")
            nc.tensor.transpose(tp_aa[:], aa8[:], ident[:KH, :KH])
            aa_bf = sb.tile([128, KH], BF16, name="aa_bf")
            nc.vector.tensor_copy(aa_bf[:], tp_aa[:])
            cb_ps = ps.tile([128, 1], F32, name="cb_ps", tag="scr")
            for k in range(K2H):
                rhsv = embc_bf[:, k:k + 1] if k < KH else aa_bf[:, k - KH:k - KH + 1]
                nc.tensor.matmul(cb_ps[:], lhsT=comb_sb[:, k, :], rhs=rhsv,
                                 start=(k == 0), stop=(k == K2H - 1))
            cbo_sb = sb.tile([128, 1], F32, name="cbo_sb")
            nc.vector.tensor_add(cbo_sb[:], cb_ps[:], cbias_sb[:])
            ag3_in = dram.tile([128, 1], F32)
            ag3_out = dram.tile([NC * 128, 1], F32)
            nc.gpsimd.dma_start(out=ag3_in[:], in_=cbo_sb[:])
            nc.gpsimd.collective_compute(
                "AllGather", mybir.AluOpType.bypass, replica_groups=rg,
                ins=[ag3_in.opt()], outs=[ag3_out.opt()])

            # ---- GRU layer 1 ----
            x18 = sb.tile([KH, 128], F32, name="x18")
            nc.gpsimd.dma_start(out=x18[:],
                                in_=ag3_out[:].rearrange("(a b) o -> a (b o)", b=128))
            tp_x1 = ps.tile([128, KH], F32, name="tp_x1", tag="scr")
            nc.tensor.transpose(tp_x1[:], x18[:], ident[:KH, :KH])
            x1_sb = sb.tile([128, KH], F32, name="x1_sb")
            nc.scalar.activation(x1_sb[:], tp_x1[:], AF.Relu)
            h1_col = _gru_layer(nc, sb, ps, x1_sb, hid_sb, h0_sb,
                                wih_sb, whh_sb, bi_sb, bh_sb, bsum_sb)
            ag4_in = dram.tile([128, 1], F32)
            ag4_out = dram.tile([NC * 128, 1], F32)
            nc.gpsimd.dma_start(out=ag4_in[:], in_=h1_col[:])
            nc.gpsimd.collective_compute(
                "AllGather", mybir.AluOpType.bypass, replica_groups=rg,
                ins=[ag4_in.opt()], outs=[ag4_out.opt()])

            # ---- GRU layer 2 ----
            h18 = sb.tile([KH, 128], F32, name="h18")
            nc.gpsimd.dma_start(out=h18[:],
                                in_=ag4_out[:].rearrange("(a b) o -> a (b o)", b=128))
            tp_h1 = ps.tile([128, KH], F32, name="tp_h1", tag="scr")
            nc.tensor.transpose(tp_h1[:], h18[:], ident[:KH, :KH])
            x2_sb = sb.tile([128, KH], F32, name="x2_sb")
            nc.scalar.activation(x2_sb[:], tp_h1[:], AF.Relu)
            h1f_sb = sb.tile([128, KH], F32, name="h1f_sb")
            nc.vector.tensor_copy(h1f_sb[:], tp_h1[:])
            h2_col = _gru_layer(nc, sb, ps, x2_sb, h1f_sb, h1_col,
                                wih_sb, whh_sb, bi_sb, bh_sb, bsum_sb)
            nc.gpsimd.dma_start(out=h_out[:, :], in_=h2_col[:])
            ag5_in = dram.tile([128, 1], F32)
            ag5_out = dram.tile([NC * 128, 1], F32)
            nc.gpsimd.dma_start(out=ag5_in[:], in_=h2_col[:])
            nc.gpsimd.collective_compute(
                "AllGather", mybir.AluOpType.bypass, replica_groups=rg,
                ins=[ag5_in.opt()], outs=[ag5_out.opt()])

            # ---- logits over this core's 6400 padded vocab cols ----
            h28 = sb.tile([KH, 128], F32, name="h28")
            nc.gpsimd.dma_start(out=h28[:],
                                in_=ag5_out[:].rearrange("(a b) o -> a (b o)", b=128))
            tp_h2 = ps.tile([128, KH], F32, name="tp_h2", tag="scr")
            nc.tensor.transpose(tp_h2[:], h28[:], ident[:KH, :KH])
            h2w_sb = sb.tile([128, KH], BF16, name="h2w_sb")
            nc.vector.tensor_copy(h2w_sb[:], tp_h2[:])

            lg_ps = lgps.tile([128, MCH], F32, name="lg_ps", tag="lg")
            for m in range(MCH):
                for k in range(KH):
                    nc.tensor.matmul(lg_ps[:, m:m + 1], lhsT=ow_tiles[k][:, m, :],
                                     rhs=h2w_sb[:, k:k + 1],
                                     start=(k == 0), stop=(k == KH - 1))
            lg_sb = sb.tile([128, MCH], F32, name="lg_sb", bufs=1)
            nc.vector.tensor_add(lg_sb[:], lg_ps[:], ob_sb[:])

            # ---- log-softmax denominator via AG6 ----
            elg = sb.tile([128, MCH], F32, name="elg")
            srl = sb.tile([128, 1], F32, name="srl")
            nc.scalar.activation(elg[:], lg_sb[:], AF.Exp, accum_out=srl[:])
            # pre-switch the ACT table to the Ln set while AG6 is in flight
            nc.scalar.activation(warm[:], warm[:], AF.Ln)
            st_ps = ps.tile([1, 1], F32, name="st_ps", tag="scr")
            nc.tensor.matmul(st_ps[:], lhsT=srl[:], rhs=ones_p[:],
                             start=True, stop=True)
            sc8 = sb.tile([1, 8], F32, name="sc8")
            nc.vector.memset(sc8[:], 0.0)
            nc.vector.tensor_copy(sc8[:, 0:1], st_ps[:])
            ag6_in = dram.tile([1, 8], F32)
            ag6_out = dram.tile([NC, 8], F32)
            nc.gpsimd.dma_start(out=ag6_in[:], in_=sc8[:])
            nc.gpsimd.collective_compute(
                "AllGather", mybir.AluOpType.bypass, replica_groups=rg,
                ins=[ag6_in.opt()], outs=[ag6_out.opt()])
            sg = sb.tile([1, NC, 8], F32, name="sg")
            nc.gpsimd.dma_start(out=sg[:],
                                in_=ag6_out[:].rearrange("a b -> (a b)")[None, :]
                                .rearrange("o (a b) -> o a b", b=8))
            tot = sb.tile([1, 1], F32, name="tot")
            nc.vector.tensor_reduce(tot[:], sg[:, :, 0:1],
                                    axis=mybir.AxisListType.XY, op=mybir.AluOpType.add)
            lse = sb.tile([1, 1], F32, name="lse")
            nc.scalar.activation(lse[:], tot[:], AF.Ln)
            lb_ps = ps.tile([128, 1], F32, name="lb_ps", tag="scr")
            nc.tensor.matmul(lb_ps[:], lhsT=ones_1[:], rhs=lse[:],
                             start=True, stop=True)
            lse_bc = sb.tile([128, 1], F32, name="lse_bc")
            nc.vector.tensor_copy(lse_bc[:], lb_ps[:])
            lp_sb = sb.tile([128, MCH], F32, name="lp_sb")
            nc.vector.tensor_scalar(lp_sb[:], lg_sb[:], lse_bc[:], None,
                                    op0=mybir.AluOpType.subtract)
            nc.sync.dma_start(out=logp_out[:, :], in_=lp_sb[:])

    nc.compile()
    return nc


def shard_inputs(inputs):
    """FULL numpy inputs -> list of 8 per-core input maps."""
    import ml_dtypes
    bf16 = ml_dtypes.bfloat16

    f = lambda k: np.ascontiguousarray(np.asarray(inputs[k], np.float32))
    idx = int(np.asarray(inputs["input"]).ravel()[0])
    emb = f("emb")
    hidden = f("hidden").reshape(H)
    attn_W, attn_b = f("attn_W"), f("attn_b")
    enc = f("encoder_outputs")
    comb_W, comb_b = f("comb_W"), f("comb_b")
    Wih, Whh = f("gru_Wih"), f("gru_Whh")
    bih, bhh = f("gru_bih"), f("gru_bhh")
    out_W, out_b = f("out_W"), f("out_b")

    owp = np.zeros((H, VP), np.float32)
    owp[:, :V] = out_W
    owp = owp.astype(bf16)
    obp = np.full((VP,), -1e30, np.float32)
    obp[:V] = out_b

    A_ih = np.ascontiguousarray(Wih.T.reshape(H, 3, H))
    A_hh = np.ascontiguousarray(Whh.T.reshape(H, 3, H))
    bi3 = bih.reshape(3, H)
    bh3 = bhh.reshape(3, H)

    embr = emb.reshape(V * KH, 128)
    idx8 = (idx * KH + np.arange(KH, dtype=np.int32)).reshape(KH, 1)
    hidden8 = np.ascontiguousarray(hidden.reshape(KH, 128))
    enc_bf = enc.astype(bf16)
    comb_bf = comb_W.astype(bf16)

    in_maps = []
    for c in range(NC):
        sl = slice(c * 128, (c + 1) * 128)
        ssl = slice(c * SS, (c + 1) * SS)
        vsl = slice(c * VS, (c + 1) * VS)
        in_maps.append({
            "idx8": idx8,
            "embr": embr,
            "hidden8": hidden8,
            "h0col": np.ascontiguousarray(hidden[sl, None]),
            "attn_w": np.ascontiguousarray(attn_W[:, ssl]),
            "attn_b": np.ascontiguousarray(attn_b[ssl].reshape(4, 128)),
            "enc": np.ascontiguousarray(enc_bf[:, sl]),
            "comb_w": np.ascontiguousarray(comb_bf[:, sl]),
            "comb_b": np.ascontiguousarray(comb_b[sl, None]),
            "wih": np.ascontiguousarray(A_ih[:, :, sl]),
            "whh": np.ascontiguousarray(A_hh[:, :, sl]),
            "bi": np.ascontiguousarray(bi3[:, sl].T),
            "bh": np.ascontiguousarray(bh3[:, sl].T),
            "ow": np.ascontiguousarray(owp[:, vsl]),
            "ob": np.ascontiguousarray(obp[vsl].reshape(MCH, 128).T),
        })
    return in_maps


def unshard_outputs(results):
    """list of per-core {name: array} -> (log_probs, hidden, attn_weights)."""
    logp = np.concatenate(
        [np.asarray(r["logp_out"]).T.reshape(-1) for r in results])[:V][None, :]
    h = np.concatenate([np.asarray(r["h_out"]).ravel() for r in results])[None, None, :]
    aw = np.asarray(results[0]["attn_out"]).T.reshape(-1)[None, :]
    return (np.ascontiguousarray(logp, np.float32),
            np.ascontiguousarray(h, np.float32),
            np.ascontiguousarray(aw, np.float32))


def _get_program():
    if "nc" not in _CACHE:
        _CACHE["nc"] = build_program()
    return _CACHE["nc"]


def kernel(**inputs):
    nc = _get_program()
    in_maps = shard_inputs(inputs)
    res = run_bass_kernel_spmd(nc, in_maps, core_ids=list(range(NC)), trace=False)
    return unshard_outputs(res.results)


# revision 19
# speedup vs baseline: 1.9910x; 1.0016x over previous
"""AttnDecoderRNN step on 8 Trainium2 NeuronCores (Bass/Tile, tensor-parallel).

Sharding (8 cores, stitched with AllGather where a full vector is needed):
  - embedding: table replicated (viewed [V*8, 128]); each core indirect-DMAs
    the token row as 8 chunk-rows (4KB HBM read) -- no collective.
  - attn scores: attn_W sharded over S columns (512/core); AG1 -> softmax
    computed partition-major on every core.
  - attn_applied: encoder_outputs sharded over H columns (128/core); AG2.
  - comb: comb_W sharded over H columns; AG3.
  - GRU (x2, shared weights): gate outputs sharded over H (128/core for each
    of the 3 gates); h re-gathered with AG4/AG5 for the next contraction.
  - logits: out_W sharded over vocab columns (6400/core after padding V to
    51200); log_softmax denominator from an AG of per-core exp-sums (AG6);
    each core writes its own log-prob shard, the host concatenates.

Precision: f32 everywhere except out_W / encoder_outputs / comb_W (and the
activation vectors feeding those three matmuls), which run in bf16.
Verified end-to-end error vs the f32 reference: log_probs ~2e-4, hidden and
attn_weights ~2e-5 (max relative).

Queue plan: all weight streaming on the sync (SP) DMA queue in consumption
order; collective bounce-buffer traffic and gather loads on the gpsimd queue
so they interleave with the collectives; compute engines stay DMA-free.
"""

import numpy as np

import concourse.bacc as bacc
import concourse.bass as bass
import concourse.tile as tile
from concourse import mybir
from concourse.bass import IndirectOffsetOnAxis
from concourse.bass_utils import run_bass_kernel_spmd
from concourse.masks import make_identity

H = 1024
S = 4096
V = 50257
L = 2
NC = 8
VP = 51200          # padded vocab
VS = VP // NC       # 6400 vocab cols per core
MCH = VS // 128     # 50 m-chunks per core
SS = S // NC        # 512 attn cols per core
KH = H // 128       # 8 k-chunks for H contraction
K2H = 2 * H // 128  # 16 k-chunks for 2H contraction
KS = S // 128       # 32 k-chunks for S contraction

F32 = mybir.dt.float32
BF16 = mybir.dt.bfloat16

_CACHE = {}


def _gru_layer(nc, sb, ps, x_sb, h_sb, h_col, wih_sb, whh_sb,
               bi_sb, bh_sb, bsum_sb):
    """One GRU cell step; returns h_new [128,1] (this core's slice)."""
    AF = mybir.ActivationFunctionType
    gi_ps = ps.tile([128, 3], F32, name="gi_ps", tag="scr")
    gh_ps = ps.tile([128, 3], F32, name="gh_ps", tag="scr")
    for g in range(3):
        for k in range(KH):
            nc.tensor.matmul(gi_ps[:, g:g + 1], lhsT=wih_sb[:, k, g, :],
                             rhs=x_sb[:, k:k + 1],
                             start=(k == 0), stop=(k == KH - 1))
    for g in range(3):
        for k in range(KH):
            nc.tensor.matmul(gh_ps[:, g:g + 1], lhsT=whh_sb[:, k, g, :],
                             rhs=h_sb[:, k:k + 1],
                             start=(k == 0), stop=(k == KH - 1))
    gh_sb = sb.tile([128, 3], F32, name="gh_sb")
    nc.vector.tensor_copy(gh_sb[:], gh_ps[:])
    rz_sb = sb.tile([128, 2], F32, name="rz_sb")
    nc.vector.tensor_add(rz_sb[:], gi_ps[:, 0:2], gh_sb[:, 0:2])
    # sigmoid via tanh to stay on the Exp/Tanh activation table:
    # sigmoid(a) = 0.5 + 0.5*tanh(a/2)
    v = sb.tile([128, 1], F32, name="gru_v")  # tanh(r-gate/2)
    u = sb.tile([128, 1], F32, name="gru_u")  # tanh(z-gate/2)
    nc.scalar.activation(v[:], rz_sb[:, 0:1], AF.Tanh,
                         bias=bsum_sb[:, 0:1], scale=0.5)
    nc.scalar.activation(u[:], rz_sb[:, 1:2], AF.Tanh,
                         bias=bsum_sb[:, 1:2], scale=0.5)
    hnb = sb.tile([128, 1], F32, name="gru_hnb")
    nc.vector.tensor_scalar_add(hnb[:], gh_sb[:, 2:3], bh_sb[:, 2:3])
    # rn = sigmoid(r)*hnb = 0.5*(hnb + tanh(r/2)*hnb)
    vh = sb.tile([128, 1], F32, name="gru_vh")
    nc.vector.tensor_mul(vh[:], v[:], hnb[:])
    rh = sb.tile([128, 1], F32, name="gru_rh")
    nc.vector.tensor_add(rh[:], hnb[:], vh[:])
    tin = sb.tile([128, 1], F32, name="gru_tin")
    nc.vector.tensor_scalar(tin[:], rh[:], 0.5, None, op0=mybir.AluOpType.mult)
    nc.vector.tensor_add(tin[:], gi_ps[:, 2:3], tin[:])
    n = sb.tile([128, 1], F32, name="gru_n")
    nc.scalar.activation(n[:], tin[:], AF.Tanh, bias=bi_sb[:, 2:3])
    # h_new = n + sigmoid(z)*(h-n) = n + 0.5*((h-n) + tanh(z/2)*(h-n))
    d = sb.tile([128, 1], F32, name="gru_d")
    nc.vector.tensor_sub(d[:], h_col[:], n[:])
    ud = sb.tile([128, 1], F32, name="gru_ud")
    nc.vector.tensor_mul(ud[:], u[:], d[:])
    e = sb.tile([128, 1], F32, name="gru_e")
    nc.vector.tensor_add(e[:], d[:], ud[:])
    nc.vector.tensor_scalar(e[:], e[:], 0.5, None, op0=mybir.AluOpType.mult)
    h_new = sb.tile([128, 1], F32, name="gru_hnew")
    nc.vector.tensor_add(h_new[:], n[:], e[:])
    return h_new


def build_program():
    nc = bacc.Bacc("TRN2", target_bir_lowering=False, debug=False, num_devices=NC)
    AF = mybir.ActivationFunctionType

    # ---- kernel I/O (per core) ----
    idx8 = nc.dram_tensor("idx8", [KH, 1], mybir.dt.int32, kind="ExternalInput")
    embr = nc.dram_tensor("embr", [V * KH, 128], F32, kind="ExternalInput")
    hidden8 = nc.dram_tensor("hidden8", [KH, 128], F32, kind="ExternalInput")
    h0col = nc.dram_tensor("h0col", [128, 1], F32, kind="ExternalInput")
    attn_w = nc.dram_tensor("attn_w", [2 * H, SS], F32, kind="ExternalInput")
    attn_b = nc.dram_tensor("attn_b", [4, 128], F32, kind="ExternalInput")
    enc = nc.dram_tensor("enc", [S, 128], BF16, kind="ExternalInput")
    comb_w = nc.dram_tensor("comb_w", [2 * H, 128], BF16, kind="ExternalInput")
    comb_b = nc.dram_tensor("comb_b", [128, 1], F32, kind="ExternalInput")
    wih = nc.dram_tensor("wih", [H, 3, 128], F32, kind="ExternalInput")
    whh = nc.dram_tensor("whh", [H, 3, 128], F32, kind="ExternalInput")
    bi = nc.dram_tensor("bi", [128, 3], F32, kind="ExternalInput")
    bh = nc.dram_tensor("bh", [128, 3], F32, kind="ExternalInput")
    ow = nc.dram_tensor("ow", [H, VS], BF16, kind="ExternalInput")
    ob = nc.dram_tensor("ob", [128, MCH], F32, kind="ExternalInput")

    attn_out = nc.dram_tensor("attn_out", [128, KS], F32, kind="ExternalOutput")
    h_out = nc.dram_tensor("h_out", [128, 1], F32, kind="ExternalOutput")
    logp_out = nc.dram_tensor("logp_out", [128, MCH], F32, kind="ExternalOutput")

    rg = [list(range(NC))]

    with tile.TileContext(nc) as tc:
        with (
            tc.tile_pool(name="w", bufs=1) as wp,
            tc.tile_pool(name="sb", bufs=2) as sb,
            tc.tile_pool(name="owp", bufs=KH) as owp,
            tc.tile_pool(name="ps", bufs=4, space="PSUM") as ps,
            tc.tile_pool(name="lgps", bufs=2, space="PSUM") as lgps,
            tc.tile_pool(name="dram", bufs=1, space="DRAM") as dram,
        ):
            # ---- tiny chain-critical loads first (SP queue) ----
            idx_sb = sb.tile([KH, 1], mybir.dt.int32, name="idx_sb", bufs=1)
            nc.sync.dma_start(out=idx_sb[:], in_=idx8[:, :])
            h08 = sb.tile([KH, 128], F32, name="h08", bufs=1)
            nc.sync.dma_start(out=h08[:], in_=hidden8[:, :])

            # ---- embedding row gather (gpsimd, ahead of its queue's
            # attn-chunk loads) ----
            er8 = sb.tile([KH, 128], F32, name="er8", bufs=1)
            nc.gpsimd.indirect_dma_start(
                out=er8[:], out_offset=None,
                in_=embr[:, :],
                in_offset=IndirectOffsetOnAxis(ap=idx_sb[:, :1], axis=0))

            # ---- weight streams (SP queue, consumption order) ----
            # attn_W chunked so each k-chunk matmul starts as soon as its
            # 256KB slice lands (pipeline DMA with the 16 PE matmuls).
            attn_sb = wp.tile([128, K2H, SS], F32)
            for k in range(K2H):
                eng = (nc.sync, nc.scalar, nc.gpsimd)[k % 3]
                eng.dma_start(out=attn_sb[:, k, :],
                              in_=attn_w[k * 128:(k + 1) * 128, :])
            ab_sb = wp.tile([4, 128], F32)
            nc.sync.dma_start(out=ab_sb[:], in_=attn_b[:, :])
            h0_sb = wp.tile([128, 1], F32)
            nc.sync.dma_start(out=h0_sb[:], in_=h0col[:, :])
            bi_sb = wp.tile([128, 3], F32)
            nc.sync.dma_start(out=bi_sb[:], in_=bi[:, :])
            bh_sb = wp.tile([128, 3], F32)
            nc.sync.dma_start(out=bh_sb[:], in_=bh[:, :])
            cbias_sb = wp.tile([128, 1], F32)
            nc.sync.dma_start(out=cbias_sb[:], in_=comb_b[:, :])
            ob_sb = wp.tile([128, MCH], F32)
            nc.sync.dma_start(out=ob_sb[:], in_=ob[:, :])
            wih_sb = wp.tile([128, KH, 3, 128], F32)
            nc.sync.dma_start(out=wih_sb[:],
                              in_=wih[:, :, :].rearrange("(k p) g n -> p k g n", p=128))
            whh_sb = wp.tile([128, KH, 3, 128], F32)
            nc.sync.dma_start(out=whh_sb[:],
                              in_=whh[:, :, :].rearrange("(k p) g n -> p k g n", p=128))
            enc_sb = wp.tile([128, KS, 128], BF16)
            nc.sync.dma_start(out=enc_sb[:],
                              in_=enc[:, :].rearrange("(k p) n -> p k n", p=128))
            comb_sb = wp.tile([128, K2H, 128], BF16)
            nc.sync.dma_start(out=comb_sb[:],
                              in_=comb_w[:, :].rearrange("(k p) n -> p k n", p=128))
            ow_tiles = []
            for k in range(KH):
                ow_k = owp.tile([128, MCH, 128], BF16, name="ow_k")
                nc.sync.dma_start(
                    out=ow_k[:],
                    in_=ow[k * 128:(k + 1) * 128, :].rearrange("p (m n) -> p m n", n=128))
                ow_tiles.append(ow_k)

            # ---- constants ----
            ident = wp.tile([128, 128], F32)
            make_identity(nc, ident[:])
            ones_p = wp.tile([128, 1], F32)
            nc.vector.memset(ones_p[:], 1.0)
            ones_1 = wp.tile([1, 128], F32)
            nc.vector.memset(ones_1[:], 1.0)
            # prewarm the Exp/Tanh activation table off the critical path
            warm = sb.tile([1, 1], F32, name="warm", bufs=1)
            nc.vector.memset(warm[:], 0.0)
            nc.scalar.activation(warm[:], warm[:], AF.Exp)
            # keep the PE busy from t=0 so the clock is ramped before the
            # first real matmuls
            pe_warm = ps.tile([128, 128], F32, name="pe_warm", tag="scr")
            for _ in range(10):
                nc.tensor.transpose(pe_warm[:], ident[:], ident[:])
            # halved gate bias: tanh(0.5*a + 0.5*(bi+bh)) for the sigmoid trick
            bsum_sb = wp.tile([128, 2], F32)
            nc.vector.tensor_add(bsum_sb[:], bi_sb[:, 0:2], bh_sb[:, 0:2])
            nc.vector.tensor_scalar(bsum_sb[:], bsum_sb[:], 0.5, None,
                                    op0=mybir.AluOpType.mult)

            # ---- partition-major transposes of emb row and hidden ----
            tp_e = ps.tile([128, KH], F32, name="tp_e", tag="scr")
            nc.tensor.transpose(tp_e[:], er8[:], ident[:KH, :KH])
            emb_pm = sb.tile([128, KH], F32, name="emb_pm", bufs=1)
            nc.vector.tensor_copy(emb_pm[:], tp_e[:])
            embc_bf = sb.tile([128, KH], BF16, name="embc_bf", bufs=1)
            nc.vector.tensor_copy(embc_bf[:], tp_e[:])
            tp_h0 = ps.tile([128, KH], F32, name="tp_h0", tag="scr")
            nc.tensor.transpose(tp_h0[:], h08[:], ident[:KH, :KH])
            hid_sb = sb.tile([128, KH], F32, name="hid_sb", bufs=1)
            nc.vector.tensor_copy(hid_sb[:], tp_h0[:])

            # ---- attn scores (this core's 512 cols of S) + AG1 ----
            # M-orientation: attn_W chunk is the stationary operand, the
            # activation column streams (N=1); scores land partition-major
            # [128, 4] and are transposed back to linear for the AllGather.
            # Interleaved per-column psum groups are element-disjoint (safe);
            # the group checker only tracks regions, hence skip_group_check.
            t1_tiles = [ps.tile([128, 1], F32, name=f"t1_ps{m}", tag="scr")
                        for m in range(4)]
            for k in range(K2H):
                rhsv = emb_pm[:, k:k + 1] if k < KH else hid_sb[:, k - KH:k - KH + 1]
                for m in range(4):
                    nc.tensor.matmul(t1_tiles[m][:],
                                     lhsT=attn_sb[:, k, m * 128:(m + 1) * 128],
                                     rhs=rhsv,
                                     start=(k == 0), stop=(k == K2H - 1))
            t1cp = sb.tile([128, 4], F32, name="t1cp")
            for m in range(4):
                nc.vector.tensor_copy(t1cp[:, m:m + 1], t1_tiles[m][:])
            t1t_ps = ps.tile([4, 128], F32, name="t1t_ps", tag="scr")
            nc.tensor.transpose(t1t_ps[:], t1cp[:], ident[:])
            t1_sb = sb.tile([4, 128], F32, name="t1_sb")
            nc.vector.tensor_add(t1_sb[:], t1t_ps[:], ab_sb[:])
            ag1_in = dram.tile([4, 128], F32)
            ag1_out = dram.tile([KS, 128], F32)
            nc.gpsimd.dma_start(out=ag1_in[:], in_=t1_sb[:])
            nc.gpsimd.collective_compute(
                "AllGather", mybir.AluOpType.bypass, replica_groups=rg,
                ins=[ag1_in.opt()], outs=[ag1_out.opt()])

            # ---- softmax over full S (partition-major, no max-sub needed) ----
            t1_32 = sb.tile([KS, 128], F32, name="t1_32")
            nc.gpsimd.dma_start(out=t1_32[:], in_=ag1_out[:])
            tp_t1 = ps.tile([128, KS], F32, name="tp_t1", tag="scr")
            nc.tensor.transpose(tp_t1[:], t1_32[:], ident[:KS, :KS])
            u_sb = sb.tile([128, KS], F32, name="u_sb")
            srow = sb.tile([128, 1], F32, name="srow")
            nc.scalar.activation(u_sb[:], tp_t1[:], AF.Exp, accum_out=srow[:])
            u_bf = sb.tile([128, KS], BF16, name="u_bf")
            nc.vector.tensor_copy(u_bf[:], u_sb[:])

            # ---- attn_applied with unnormalized weights; scaled after the
            # matmul so the reciprocal chain overlaps the PE work.
            # M-orientation: enc chunk stationary, u column streams; the
            # result lands partition-major [128,1] ready for the AllGather.
            aa_ps = ps.tile([128, 1], F32, name="aa_ps", tag="scr")
            for k in range(KS):
                nc.tensor.matmul(aa_ps[:], lhsT=enc_sb[:, k, :],
                                 rhs=u_bf[:, k:k + 1],
                                 start=(k == 0), stop=(k == KS - 1))
            s1_ps = ps.tile([1, 1], F32, name="s1_ps", tag="scr")
            nc.tensor.matmul(s1_ps[:], lhsT=srow[:], rhs=ones_p[:],
                             start=True, stop=True)
            rs_sb = sb.tile([1, 1], F32, name="rs_sb")
            nc.vector.reciprocal(rs_sb[:], s1_ps[:])
            rb_ps = ps.tile([128, 1], F32, name="rb_ps", tag="scr")
            nc.tensor.matmul(rb_ps[:], lhsT=ones_1[:], rhs=rs_sb[:],
                             start=True, stop=True)
            rs_bc = sb.tile([128, 1], F32, name="rs_bc")
            nc.vector.tensor_copy(rs_bc[:], rb_ps[:])
            aw_sb = sb.tile([128, KS], F32, name="aw_sb")
            nc.vector.tensor_scalar_mul(aw_sb[:], u_sb[:], rs_bc[:])
            nc.gpsimd.dma_start(out=attn_out[:, :], in_=aw_sb[:])
            aa_sb = sb.tile([128, 1], F32, name="aa_sb")
            nc.vector.tensor_copy(aa_sb[:], aa_ps[:])
            ag2_in = dram.tile([128, 1], F32)
            ag2_out = dram.tile([NC * 128, 1], F32)
            nc.gpsimd.dma_start(out=ag2_in[:], in_=aa_sb[:])
            nc.gpsimd.collective_compute(
                "AllGather", mybir.AluOpType.bypass, replica_groups=rg,
                ins=[ag2_in.opt()], outs=[ag2_out.opt()])

            # ---- comb + AG3 ----
            aa8 = sb.tile([KH, 128], F32, name="aa8")
            nc.gpsimd.dma_start(out=aa8[:],
                                in_=ag2_out[:].rearrange("(a b) o -> a (b o)", b=128))
            tp_aa = ps.tile([128, KH], F32, name="tp_aa", tag="scr")
            nc.tensor.transpose(tp_aa[:], aa8[:], ident[:KH, :KH])
            # normalize by the (globally identical) softmax reciprocal here,
            # off the pre-AG2 critical path
            aa_bf = sb.tile([128, KH], BF16, name="aa_bf")
            nc.vector.tensor_scalar_mul(aa_bf[:], tp_aa[:], rs_bc[:])
            cb_ps = ps.tile([128, 1], F32, name="cb_ps", tag="scr")
            for k in range(K2H):
                rhsv = embc_bf[:, k:k + 1] if k < KH else aa_bf[:, k - KH:k - KH + 1]
                nc.tensor.matmul(cb_ps[:], lhsT=comb_sb[:, k, :], rhs=rhsv,
                                 start=(k == 0), stop=(k == K2H - 1))
            cbo_sb = sb.tile([128, 1], F32, name="cbo_sb")
            nc.vector.tensor_add(cbo_sb[:], cb_ps[:], cbias_sb[:])
            ag3_in = dram.tile([128, 1], F32)
            ag3_out = dram.tile([NC * 128, 1], F32)
            nc.gpsimd.dma_start(out=ag3_in[:], in_=cbo_sb[:])
            nc.gpsimd.collective_compute(
                "AllGather", mybir.AluOpType.bypass, replica_groups=rg,
                ins=[ag3_in.opt()], outs=[ag3_out.opt()])

            # ---- GRU layer 1 ----
            x18 = sb.tile([KH, 128], F32, name="x18")
            nc.gpsimd.dma_start(out=x18[:],
                                in_=ag3_out[:].rearrange("(a b) o -> a (b o)", b=128))
            tp_x1 = ps.tile([128, KH], F32, name="tp_x1", tag="scr")
            nc.tensor.transpose(tp_x1[:], x18[:], ident[:KH, :KH])
            x1_sb = sb.tile([128, KH], F32, name="x1_sb")
            nc.scalar.activation(x1_sb[:], tp_x1[:], AF.Relu)
            h1_col = _gru_layer(nc, sb, ps, x1_sb, hid_sb, h0_sb,
                                wih_sb, whh_sb, bi_sb, bh_sb, bsum_sb)
            ag4_in = dram.tile([128, 1], F32)
            ag4_out = dram.tile([NC * 128, 1], F32)
            nc.gpsimd.dma_start(out=ag4_in[:], in_=h1_col[:])
            nc.gpsimd.collective_compute(
                "AllGather", mybir.AluOpType.bypass, replica_groups=rg,
                ins=[ag4_in.opt()], outs=[ag4_out.opt()])

            # ---- GRU layer 2 ----
            h18 = sb.tile([KH, 128], F32, name="h18")
            nc.gpsimd.dma_start(out=h18[:],
                                in_=ag4_out[:].rearrange("(a b) o -> a (b o)", b=128))
            tp_h1 = ps.tile([128, KH], F32, name="tp_h1", tag="scr")
            nc.tensor.transpose(tp_h1[:], h18[:], ident[:KH, :KH])
            x2_sb = sb.tile([128, KH], F32, name="x2_sb")
            nc.scalar.activation(x2_sb[:], tp_h1[:], AF.Relu)
            h1f_sb = sb.tile([128, KH], F32, name="h1f_sb")
            nc.vector.tensor_copy(h1f_sb[:], tp_h1[:])
            h2_col = _gru_layer(nc, sb, ps, x2_sb, h1f_sb, h1_col,
                                wih_sb, whh_sb, bi_sb, bh_sb, bsum_sb)
            nc.gpsimd.dma_start(out=h_out[:, :], in_=h2_col[:])
            ag5_in = dram.tile([128, 1], F32)
            ag5_out = dram.tile([NC * 128, 1], F32)
            nc.gpsimd.dma_start(out=ag5_in[:], in_=h2_col[:])
            nc.gpsimd.collective_compute(
                "AllGather", mybir.AluOpType.bypass, replica_groups=rg,
                ins=[ag5_in.opt()], outs=[ag5_out.opt()])

            # ---- logits over this core's 6400 padded vocab cols ----
            h28 = sb.tile([KH, 128], F32, name="h28")
            nc.gpsimd.dma_start(out=h28[:],
                                in_=ag5_out[:].rearrange("(a b) o -> a (b o)", b=128))
            tp_h2 = ps.tile([128, KH], F32, name="tp_h2", tag="scr")
            nc.tensor.transpose(tp_h2[:], h28[:], ident[:KH, :KH])
            h2w_sb = sb.tile([128, KH], BF16, name="h2w_sb")
            nc.vector.tensor_copy(h2w_sb[:], tp_h2[:])

            lg_ps = lgps.tile([128, MCH], F32, name="lg_ps", tag="lg")
            for m in range(MCH):
                for k in range(KH):
                    nc.tensor.matmul(lg_ps[:, m:m + 1], lhsT=ow_tiles[k][:, m, :],
                                     rhs=h2w_sb[:, k:k + 1],
                                     start=(k == 0), stop=(k == KH - 1))
            lg_sb = sb.tile([128, MCH], F32, name="lg_sb", bufs=1)
            nc.vector.tensor_add(lg_sb[:], lg_ps[:], ob_sb[:])

            # ---- log-softmax denominator via AG6 ----
            elg = sb.tile([128, MCH], F32, name="elg")
            srl = sb.tile([128, 1], F32, name="srl")
            nc.scalar.activation(elg[:], lg_sb[:], AF.Exp, accum_out=srl[:])
            # pre-switch the ACT table to the Ln set while AG6 is in flight
            nc.scalar.activation(warm[:], warm[:], AF.Ln)
            st_ps = ps.tile([1, 1], F32, name="st_ps", tag="scr")
            nc.tensor.matmul(st_ps[:], lhsT=srl[:], rhs=ones_p[:],
                             start=True, stop=True)
            sc8 = sb.tile([1, 8], F32, name="sc8")
            nc.vector.memset(sc8[:], 0.0)
            nc.vector.tensor_copy(sc8[:, 0:1], st_ps[:])
            ag6_in = dram.tile([1, 8], F32)
            ag6_out = dram.tile([NC, 8], F32)
            nc.gpsimd.dma_start(out=ag6_in[:], in_=sc8[:])
            nc.gpsimd.collective_compute(
                "AllGather", mybir.AluOpType.bypass, replica_groups=rg,
                ins=[ag6_in.opt()], outs=[ag6_out.opt()])
            sg = sb.tile([1, NC, 8], F32, name="sg")
            nc.gpsimd.dma_start(out=sg[:],
                                in_=ag6_out[:].rearrange("a b -> (a b)")[None, :]
                                .rearrange("o (a b) -> o a b", b=8))
            tot = sb.tile([1, 1], F32, name="tot")
            nc.vector.tensor_reduce(tot[:], sg[:, :, 0:1],
                                    axis=mybir.AxisListType.XY, op=mybir.AluOpType.add)
            lse = sb.tile([1, 1], F32, name="lse")
            nc.scalar.activation(lse[:], tot[:], AF.Ln)
            lb_ps = ps.tile([128, 1], F32, name="lb_ps", tag="scr")
            nc.tensor.matmul(lb_ps[:], lhsT=ones_1[:], rhs=lse[:],
                             start=True, stop=True)
            lse_bc = sb.tile([128, 1], F32, name="lse_bc")
            nc.vector.tensor_copy(lse_bc[:], lb_ps[:])
            lp_sb = sb.tile([128, MCH], F32, name="lp_sb")
            nc.vector.tensor_scalar(lp_sb[:], lg_sb[:], lse_bc[:], None,
                                    op0=mybir.AluOpType.subtract)
            nc.sync.dma_start(out=logp_out[:, :], in_=lp_sb[:])

    nc.compile()
    return nc


def shard_inputs(inputs):
    """FULL numpy inputs -> list of 8 per-core input maps."""
    import ml_dtypes
    bf16 = ml_dtypes.bfloat16

    f = lambda k: np.ascontiguousarray(np.asarray(inputs[k], np.float32))
    idx = int(np.asarray(inputs["input"]).ravel()[0])
    emb = f("emb")
    hidden = f("hidden").reshape(H)
    attn_W, attn_b = f("attn_W"), f("attn_b")
    enc = f("encoder_outputs")
    comb_W, comb_b = f("comb_W"), f("comb_b")
    Wih, Whh = f("gru_Wih"), f("gru_Whh")
    bih, bhh = f("gru_bih"), f("gru_bhh")
    out_W, out_b = f("out_W"), f("out_b")

    owp = np.zeros((H, VP), np.float32)
    owp[:, :V] = out_W
    owp = owp.astype(bf16)
    obp = np.full((VP,), -1e30, np.float32)
    obp[:V] = out_b

    A_ih = np.ascontiguousarray(Wih.T.reshape(H, 3, H))
    A_hh = np.ascontiguousarray(Whh.T.reshape(H, 3, H))
    bi3 = bih.reshape(3, H)
    bh3 = bhh.reshape(3, H)

    embr = emb.reshape(V * KH, 128)
    idx8 = (idx * KH + np.arange(KH, dtype=np.int32)).reshape(KH, 1)
    hidden8 = np.ascontiguousarray(hidden.reshape(KH, 128))
    enc_bf = enc.astype(bf16)
    comb_bf = comb_W.astype(bf16)

    in_maps = []
    for c in range(NC):
        sl = slice(c * 128, (c + 1) * 128)
        ssl = slice(c * SS, (c + 1) * SS)
        vsl = slice(c * VS, (c + 1) * VS)
        in_maps.append({
            "idx8": idx8,
            "embr": embr,
            "hidden8": hidden8,
            "h0col": np.ascontiguousarray(hidden[sl, None]),
            "attn_w": np.ascontiguousarray(attn_W[:, ssl]),
            "attn_b": np.ascontiguousarray(attn_b[ssl].reshape(4, 128)),
            "enc": np.ascontiguousarray(enc_bf[:, sl]),
            "comb_w": np.ascontiguousarray(comb_bf[:, sl]),
            "comb_b": np.ascontiguousarray(comb_b[sl, None]),
            "wih": np.ascontiguousarray(A_ih[:, :, sl]),
            "whh": np.ascontiguousarray(A_hh[:, :, sl]),
            "bi": np.ascontiguousarray(bi3[:, sl].T),
            "bh": np.ascontiguousarray(bh3[:, sl].T),
            "ow": np.ascontiguousarray(owp[:, vsl]),
            "ob": np.ascontiguousarray(obp[vsl].reshape(MCH, 128).T),
        })
    return in_maps


def unshard_outputs(results):
    """list of per-core {name: array} -> (log_probs, hidden, attn_weights)."""
    logp = np.concatenate(
        [np.asarray(r["logp_out"]).T.reshape(-1) for r in results])[:V][None, :]
    h = np.concatenate([np.asarray(r["h_out"]).ravel() for r in results])[None, None, :]
    aw = np.asarray(results[0]["attn_out"]).T.reshape(-1)[None, :]
    return (np.ascontiguousarray(logp, np.float32),
            np.ascontiguousarray(h, np.float32),
            np.ascontiguousarray(aw, np.float32))


def _get_program():
    if "nc" not in _CACHE:
        _CACHE["nc"] = build_program()
    return _CACHE["nc"]


def kernel(**inputs):
    nc = _get_program()
    in_maps = shard_inputs(inputs)
    res = run_bass_kernel_spmd(nc, in_maps, core_ids=list(range(NC)), trace=False)
    return unshard_outputs(res.results)
